# revision 1
# baseline (speedup 1.0000x reference)
"""Trainium2 Bass kernel for nn_MultiHeadAttention_69106023793143.

Reference computation (B=4, S=2048, D=1024, H=16, HD=64):
    qh = split_heads(q @ Wq + bq); kh, vh likewise
    out = merge_heads(sigmoid((qh @ kh^T) / sqrt(HD)) @ vh)

Sharding (8 cores): core c handles batch b = c//2 and the half = c%2 slice of
the feature axis (512 features = 8 heads).  Projections are tensor-parallel on
the output dim of Wq/Wk/Wv; attention is head-parallel.

Device strategy per core (ACT-engine-paced pipeline):
  - The sigmoid over all 8*2048*2048 scores is the hard floor (ACT processes
    1 elem/cycle/partition @1.2GHz ~= 218us/core); everything else is
    scheduled to hide under the ACT stream.
  - Projections run as split-fp8 DoubleRow matmuls: host ships x and W as
    (hi, lo) fp8e4 pairs (same bytes as bf16) and the product takes the three
    cross terms xh*Wh + xh*Wl + xl*Wh - ~bf16 accuracy at 0.75x the bf16 PE
    cost.  W is host-scaled by 16 so its N(0, 1/1024) entries stay in e4m3's
    normal range; the 1/256 score scale folds into the sigmoid affine and a
    1/16 into the output copy.
  - Q^T/K^T land as [of, tok] with head PAIRS stacked on the 128 partitions,
    V natural [tok, of].
  - Scores use fp8e4 DoubleRow matmuls at 0.5 cycles/row: kh is stored as an
    (hi, lo) fp8 pair - the two DoubleRow K-blocks - so K-side quantization
    error is compensated; qh is plain fp8 broadcast across the two blocks
    (stride-0 AP).  Odd heads live on partitions 64:128 (tile_position
    (64, 0)).  End-to-end max rel err ~1.35e-2 (q-side fp8 only), vs the
    2e-2 budget.
  - Sigmoid on ScalarE in alternating 3-bank/2-bank PSUM waves, writing attn
    directly as bf16 to SBUF.
  - Attention runs in rounds of (head, q-chunk-PAIR) with the k-token axis
    OUTERMOST inside a round: jobs (h, qcp, kt, i).  A round therefore takes
    ~13 ACT waves to sweep the k tokens, which spreads the xk-chunk DMA and
    V-projection deadlines far enough apart that the serial DMA engines and
    the PE projection stream keep up with ACT from the start (a (h, qc)
    round with kt innermost sweeps all 2048 k tokens in 6 waves - the DMA
    can't feed that).
  - AV in bf16 with out[q, d] layout: lhsT = attn^T tile [128k, 128q],
    rhs = v [128k, 64d] -> only 64 free rows per matmul.  Each round
    accumulates its 8 q-tiles (2 q-chunks x 4) as interleaved sub-bank
    groups in ONE PSUM bank: start=True only on the round's first matmul
    (the PSUM zero-region covers the whole 2KB bank and zeroes on first
    touch per byte), stop=True on the last.
  - PSUM: 3+2 score banks + 2 projection banks (double-buffered so the
    projection pipeline never serializes on its DVE drain) + 1 AV bank = 8.
  - Wk/Wq are shipped p-major ([NHP, P, KTC*2*128]) so a head-pair slice is
    one contiguous 2KB-per-partition DMA; input DMAs are emitted up front in
    deadline order (the DMA engines are a serial resource in practice).
  - AV matmuls drain from a FIFO gated on their V-tile's emission so the
    in-order PE queue never head-of-line blocks on a V projection.
  - Nonzero biases fold in via a host-side augmented ones-row (KTC=9).
"""

import sys

if "/opt/trn_rl_repo" not in sys.path:
    sys.path.insert(0, "/opt/trn_rl_repo")

from collections import deque
from contextlib import ExitStack

import numpy as np

import concourse.tile as tile
from concourse import bacc, mybir
from concourse.bass_utils import run_bass_kernel_spmd

B, S, D, H = 4, 2048, 1024, 16
HD = D // H  # 64
OF = D // 2  # 512 features (8 heads) per core
N_CORES = 8
P = 128
NH = 8          # heads per core
NHP = 4         # head pairs per core
QC = 4          # q-chunks of 512
NQCP = 2        # q-chunk pairs
NKT = 16        # k token tiles of 128
TOKC = 4        # x token chunks of 512
RJOBS = 2 * NKT  # jobs per round (2 q-chunks x 16 kt)
ABUFS = 19      # attn (a_t) wave buffers
WS = 16.0       # host-side W scale (keeps fp8 W out of subnormals)

F32 = mybir.dt.float32
BF16 = mybir.dt.bfloat16
FP8 = mybir.dt.float8e4

# the three split-fp8 cross terms (w level, x level)
TERMS = ((0, 0), (0, 1), (1, 0))

_cache: dict = {}
last_results = None


def _build(KTC: int):
    """KTC = contraction k-tiles for the projections (8, or 9 when biases are
    folded in via an augmented ones-row)."""
    nc = bacc.Bacc("TRN2", target_bir_lowering=False, debug=False,
                   num_devices=N_CORES, name="mha_sig4")
    KA = KTC * P
    NDR = KTC // 2   # DoubleRow kt-pairs per term
    AUGK = KTC % 2   # leftover kt (the ones-row) as plain fp8 matmul
    WFREE = KTC * 2 * P  # per-partition elements of one head-pair W slice

    xq = nc.dram_tensor("xq", [KA, 2, S], FP8, kind="ExternalInput")
    xk = nc.dram_tensor("xk", [KA, 2, S], FP8, kind="ExternalInput")
    xv = nc.dram_tensor("xv", [KA, 2, S], FP8, kind="ExternalInput")
    # wq/wk p-major: [head-pair, partition, kt*level*128]
    wq = nc.dram_tensor("wq", [NHP, P, WFREE], FP8, kind="ExternalInput")
    wk = nc.dram_tensor("wk", [NHP, P, WFREE], FP8, kind="ExternalInput")
    wv = nc.dram_tensor("wv", [KA, 2, OF], FP8, kind="ExternalInput")
    o = nc.dram_tensor("o", [S, OF], F32, kind="ExternalOutput")

    xq_r = xq.rearrange("(kt p) l t -> p kt l t", p=P)
    xk_r = xk.rearrange("(kt p) l t -> p kt l t", p=P)
    xv_r = xv.rearrange("(kt p) l t -> p kt l t", p=P)
    wv_r = wv.rearrange("(kt p) l n -> p kt l n", p=P)

    abufs = ABUFS if KTC == 8 else 10

    with tile.TileContext(nc) as tc:
        with ExitStack() as ctx:
            persist = ctx.enter_context(tc.tile_pool(name="persist", bufs=1))
            xvpool = ctx.enter_context(tc.tile_pool(name="xvpool", bufs=2))
            apool = ctx.enter_context(tc.tile_pool(name="apool", bufs=abufs))
            opool = ctx.enter_context(tc.tile_pool(name="opool", bufs=2))
            ps_pool = ctx.enter_context(
                tc.tile_pool(name="ps_pool", bufs=2, space="PSUM"))

            wk_sb = persist.tile([P, NHP, KTC, 2, P], FP8)
            wq_sb = persist.tile([P, NHP, KTC, 2, P], FP8)
            wv_sb = persist.tile([P, KTC, 2, OF], FP8)
            xk_sb = persist.tile([P, KTC, 2, S], FP8)
            xq_sb = persist.tile([P, KTC, 2, S], FP8)
            # kh as (hi, lo) fp8 pair, head pairs stacked on partitions;
            # qh plain fp8; v bf16 [tok, of]
            kh = persist.tile([P, NHP, 2, S], FP8)
            qh = persist.tile([P, NHP, S], FP8)
            v_sb = persist.tile([P, NKT, OF], BF16)

            # ONE PSUM bank for all projections: two [P, 256] slots in a
            # persistent tile, manually rotated. Region-based dep tracking
            # gives WAR/WAW per slot; each half-group's start=True re-marks
            # the whole bank but PSUM reads return raw data for re-marked
            # bytes (hardware-verified), and no other slot is ever
            # mid-accumulation when a start executes (serial emission).
            proj_ps = ps_pool.tile([P, 2, 256], F32, tag="proj", bufs=1)
            _slot = {"i": 0}

            def next_slot():
                s = _slot["i"] % 2
                _slot["i"] += 1
                return proj_ps[:, s, :]

            # ---------- producer closures ----------
            def dma_w_hp(w_sb, w_dram, hp):
                def run():
                    nc.sync.dma_start(
                        w_sb[:, hp].rearrange("p kt l n -> p (kt l n)"),
                        w_dram[hp])
                return run

            def dma_wv():
                def run():
                    nc.sync.dma_start(wv_sb[:], wv_r)
                return run

            def dma_x(x_sb, x_r, lo, hi):
                def run():
                    for lv in range(2):
                        nc.sync.dma_start(
                            x_sb[:, :, lv, lo:hi],
                            x_r[:, :, lv, lo:hi])
                return run

            xv_tiles = {}

            def dma_xv(c, lv):
                def run():
                    if lv == 0:
                        xv_tiles[c] = xvpool.tile([P, KTC, 2, 512], FP8,
                                                  tag="xvchunk",
                                                  name=f"xv_{c}")
                    nc.sync.dma_start(
                        xv_tiles[c][:, :, lv, :],
                        xv_r[:, :, lv, c * 512:(c + 1) * 512])
                return run

            # warm-up: the PE runs at 0.65/1.2GHz until ~3us of continuous
            # execution; burn dummy matmuls during the prefix DMAs so the
            # first projections run at full clock
            wu_sb = persist.tile([HD, 2, 640], FP8)

            def warmup(n_mm):
                def run():
                    if n_mm < 0:
                        nc.vector.memset(wu_sb[:], 0)
                        return
                    st = ps_pool.tile([P, 3, 512], F32, tag="st", bufs=2,
                                      name=f"wu_{n_mm}")
                    for m in range(n_mm):
                        nc.tensor.matmul(
                            st[:, m % 3, :],
                            lhsT=wu_sb[:, :, 0:P],
                            rhs=wu_sb[:, :, P:P + 512],
                            start=True, stop=True,
                            perf_mode=mybir.MatmulPerfMode.DoubleRow,
                            skip_group_check=True,
                        )
                return run

            # split-emission prefix projections: the hi terms only need the
            # lv0 (hi) half of the x chunk, so they start ~1.5us earlier
            _prefix_ps = {}

            def _kq_half(x_sb, w_sb, hp, tsl, ps, terms, first):
                n = 0
                for lw, lx in terms:
                    for t in range(NDR):
                        nc.tensor.matmul(
                            ps,
                            lhsT=w_sb[:, hp, 2 * t:2 * t + 2, lw, :],
                            rhs=x_sb[:, 2 * t:2 * t + 2, lx, tsl],
                            start=(first and n == 0), stop=False,
                            perf_mode=mybir.MatmulPerfMode.DoubleRow,
                            skip_group_check=True,
                        )
                        n += 1

            def _kq_drain(dst, hp, tsl, ps, split_lo):
                if split_lo:
                    nc.vector.tensor_copy(out=dst[:, hp, 0, tsl], in_=ps)
                    nc.vector.tensor_sub(dst[:, hp, 1, tsl], ps,
                                         dst[:, hp, 0, tsl])
                else:
                    nc.vector.tensor_copy(out=dst[:, hp, tsl], in_=ps)

            def proj_kq_hi(x_sb, w_sb, hp, c, key):
                def run():
                    ps = next_slot()
                    _prefix_ps[key] = ps
                    tsl = slice(c * 512, c * 512 + 256)
                    _kq_half(x_sb, w_sb, hp, tsl, ps, ((0, 0), (1, 0)), True)
                return run

            def proj_kq_lo(x_sb, w_sb, hp, c, dst, split_lo, key):
                def run():
                    ps = _prefix_ps.pop(key)
                    tsl = slice(c * 512, c * 512 + 256)
                    _kq_half(x_sb, w_sb, hp, tsl, ps, ((0, 1),), False)
                    _kq_drain(dst, hp, tsl, ps, split_lo)
                    ps2 = next_slot()
                    tsl2 = slice(c * 512 + 256, (c + 1) * 512)
                    _kq_half(x_sb, w_sb, hp, tsl2, ps2,
                             ((0, 0), (1, 0), (0, 1)), True)
                    _kq_drain(dst, hp, tsl2, ps2, split_lo)
                return run

            def dma_x_lv(x_sb, x_r, lo, hi, lv):
                def run():
                    nc.sync.dma_start(
                        x_sb[:, :, lv, lo:hi],
                        x_r[:, :, lv, lo:hi])
                return run

            def proj_kq(x_sb, w_sb, hp, c, dst, split_lo):
                """dst slice [of-pair, tok chunk c] for head-pair hp, as
                two [P, 256] slot halves in the shared proj bank."""
                def run():
                    for half in range(2):
                        ps = next_slot()
                        tsl = slice(c * 512 + half * 256,
                                    c * 512 + (half + 1) * 256)
                        n = 0
                        for lw, lx in TERMS:
                            for t in range(NDR):
                                nc.tensor.matmul(
                                    ps,
                                    lhsT=w_sb[:, hp, 2 * t:2 * t + 2, lw, :],
                                    rhs=x_sb[:, 2 * t:2 * t + 2, lx, tsl],
                                    start=(n == 0), stop=False,
                                    perf_mode=mybir.MatmulPerfMode.DoubleRow,
                                    skip_group_check=True,
                                )
                                n += 1
                            if AUGK:
                                nc.tensor.matmul(
                                    ps,
                                    lhsT=w_sb[:, hp, KTC - 1, lw, :],
                                    rhs=x_sb[:, KTC - 1, lx, tsl],
                                    start=(n == 0), stop=False,
                                    skip_group_check=True,
                                )
                                n += 1
                    # halves always land on slots (0, 1): one wide drain
                    wide = proj_ps[:].rearrange("p s n -> p (s n)")
                    sl = slice(c * 512, (c + 1) * 512)
                    if split_lo:
                        nc.vector.tensor_copy(out=dst[:, hp, 0, sl], in_=wide)
                        nc.vector.tensor_sub(dst[:, hp, 1, sl], wide,
                                             dst[:, hp, 0, sl])
                    else:
                        nc.vector.tensor_copy(out=dst[:, hp, sl], in_=wide)
                return run

            _v_ps = {}

            def proj_v(t, part):
                """v_sb[:, t, :] = x-token-tile t @ Wv ([tok, of]).
                part 'hi' takes the two x-hi terms, 'lo' the x-lo term +
                the PSUM drain (finer interleaving in the in-order PE queue,
                and 'hi' only needs the lv0 half of the xv chunk)."""
                def run():
                    xt = xv_tiles[t // 4]
                    tsl = slice((t % 4) * P, (t % 4 + 1) * P)

                    def v_half(ps, osl, terms, first):
                        n = 0
                        for lw, lx in terms:
                            for u in range(NDR):
                                nc.tensor.matmul(
                                    ps,
                                    lhsT=xt[:, 2 * u:2 * u + 2, lx, tsl],
                                    rhs=wv_sb[:, 2 * u:2 * u + 2, lw, osl],
                                    start=(first and n == 0), stop=False,
                                    perf_mode=mybir.MatmulPerfMode.DoubleRow,
                                    skip_group_check=True,
                                )
                                n += 1
                            if AUGK:
                                nc.tensor.matmul(
                                    ps,
                                    lhsT=xt[:, KTC - 1, lx, tsl],
                                    rhs=wv_sb[:, KTC - 1, lw, osl],
                                    start=(first and n == 0), stop=False,
                                    skip_group_check=True,
                                )
                                n += 1

                    if part == "hi":
                        ps = next_slot()
                        _v_ps[t] = ps
                        v_half(ps, slice(0, 256), ((0, 0), (1, 0)), True)
                    else:
                        ps = _v_ps.pop(t)
                        v_half(ps, slice(0, 256), ((0, 1),), False)
                        ps2 = next_slot()
                        v_half(ps2, slice(256, 512),
                               ((0, 0), (1, 0), (0, 1)), True)
                        nc.vector.tensor_copy(
                            out=v_sb[:, t, :],
                            in_=proj_ps[:].rearrange("p s n -> p (s n)"))
                return run

            # ---------- static schedule ----------
            # jobs: rounds of (h, qcp), k-token axis outermost inside the
            # round: (h, qc = 2*qcp + i, kt); job = h*64 + qcp*32 + kt*2 + i.
            # Round 0 staggers its second q-chunk by 2 k-tiles so the first
            # waves only need qh[qc0] (whose projection finishes first).
            jobs = []
            for kt in range(NKT + 2):
                if kt < NKT:
                    jobs.append((0, 0, kt))
                if kt >= 2:
                    jobs.append((0, 1, kt - 2))
            jobs += [(h, 2 * qcp + i, kt)
                     for h in range(NH) for qcp in range(NQCP)
                     for kt in range(NKT) for i in range(2)
                     if not (h == 0 and qcp == 0)]
            waves = [jobs[i0:i0 + 3] for i0 in range(0, len(jobs), 3)]

            producers = []  # (due_job, closure, vtile_or_None)
            # prefix + all input DMAs in deadline order (the DMA engines are
            # effectively serial; emission order = transfer order)
            if AUGK == 0:
                producers += [
                    (-99.9, warmup(-1), None),
                    (-99.8, warmup(10), None),
                    (-99.0, dma_w_hp(wk_sb, wk, 0), None),
                    (-98.9, dma_x_lv(xk_sb, xk_r, 0, 512, 0), None),
                    (-98.8, dma_w_hp(wq_sb, wq, 0), None),
                    (-98.7, proj_kq_hi(xk_sb, wk_sb, 0, 0, "k00"), None),
                    (-98.6, dma_x_lv(xk_sb, xk_r, 0, 512, 1), None),
                    (-98.5, proj_kq_lo(xk_sb, wk_sb, 0, 0, kh, True, "k00"),
                     None),
                    (-98.4, dma_x_lv(xq_sb, xq_r, 0, 512, 0), None),
                    (-98.3, warmup(4), None),
                    (-98.2, proj_kq_hi(xq_sb, wq_sb, 0, 0, "q00"), None),
                    (-98.1, dma_x_lv(xq_sb, xq_r, 0, 512, 1), None),
                    (-98.0, proj_kq_lo(xq_sb, wq_sb, 0, 0, qh, False, "q00"),
                     None),
                    (-97.9, dma_x_lv(xq_sb, xq_r, 512, 1024, 0), None),
                    (-97.8, proj_kq_hi(xq_sb, wq_sb, 0, 1, "q01"), None),
                    (-97.7, dma_x_lv(xq_sb, xq_r, 512, 1024, 1), None),
                    (-97.6, proj_kq_lo(xq_sb, wq_sb, 0, 1, qh, False, "q01"),
                     None),
                ]
            else:
                producers += [
                    (-99.0, dma_w_hp(wk_sb, wk, 0), None),
                    (-98.8, dma_x(xk_sb, xk_r, 0, 512), None),
                    (-98.6, dma_w_hp(wq_sb, wq, 0), None),
                    (-98.4, dma_x(xq_sb, xq_r, 0, 512), None),
                    (-98.2, proj_kq(xk_sb, wk_sb, 0, 0, kh, True), None),
                    (-98.0, proj_kq(xq_sb, wq_sb, 0, 0, qh, False), None),
                    (-97.8, dma_x(xq_sb, xq_r, 512, 1024), None),
                    (-97.6, proj_kq(xq_sb, wq_sb, 0, 1, qh, False), None),
                ]
            # earliest-deadline-first input stream; K(0, c1/c2) also use
            # split emission so kh is ready ~1.5us after the lv0 half lands
            if AUGK == 0:
                producers += [
                    (-89.8, dma_x_lv(xk_sb, xk_r, 512, 1024, 0), None),
                    (-89.7, proj_kq_hi(xk_sb, wk_sb, 0, 1, "k01"), None),
                    (-89.6, dma_x_lv(xk_sb, xk_r, 512, 1024, 1), None),
                    (-89.5, proj_kq_lo(xk_sb, wk_sb, 0, 1, kh, True, "k01"),
                     None),
                    (-88.8, dma_x_lv(xk_sb, xk_r, 1024, 1536, 0), None),
                    (-88.7, proj_kq_hi(xk_sb, wk_sb, 0, 2, "k02"), None),
                    (-88.6, dma_x_lv(xk_sb, xk_r, 1024, 1536, 1), None),
                    (-88.5, proj_kq_lo(xk_sb, wk_sb, 0, 2, kh, True, "k02"),
                     None),
                ]
            else:
                producers.append((-89, dma_x(xk_sb, xk_r, 512, 1024), None))
                producers.append((-88, dma_x(xk_sb, xk_r, 1024, 1536), None))
                producers.append((2, proj_kq(xk_sb, wk_sb, 0, 1, kh, True),
                                  None))
                producers.append((10, proj_kq(xk_sb, wk_sb, 0, 2, kh, True),
                                  None))
            producers.append((-87, dma_x(xk_sb, xk_r, 1536, 2048), None))
            producers.append((-86, dma_x(xq_sb, xq_r, 1024, 1536), None))
            producers.append((-85, dma_x(xq_sb, xq_r, 1536, 2048), None))
            producers.append((-84, dma_wv(), None))
            producers.append((-83, dma_xv(0, 0), None))
            producers.append((-82.5, dma_xv(0, 1), None))
            producers.append((-82, dma_xv(1, 0), None))
            producers.append((-81.5, dma_xv(1, 1), None))
            producers.append((10, dma_xv(2, 0), None))
            producers.append((10.5, dma_xv(2, 1), None))
            producers.append((20, dma_xv(3, 0), None))
            producers.append((20.5, dma_xv(3, 1), None))
            # kh chunk c3 needed from kt 12 (job ~25)
            producers.append((18, proj_kq(xk_sb, wk_sb, 0, 3, kh, True),
                              None))
            # Q projections for qc2/3: needed from job 32
            producers.append((24, proj_kq(xq_sb, wq_sb, 0, 2, qh, False),
                              None))
            producers.append((25, proj_kq(xq_sb, wq_sb, 0, 3, qh, False),
                              None))
            # V tiles: paced behind their xv chunk's DMA slot
            for t in range(NKT):
                d = 46 + (t // 4) * 6 + (t % 4)
                producers.append((d, proj_v(t, "hi"), None))
                producers.append((d + 0.5, proj_v(t, "lo"), t))
            for hp in range(1, NHP):
                base = 128 * hp
                producers.append((base - 64, dma_w_hp(wk_sb, wk, hp), None))
                producers.append((base - 62, dma_w_hp(wq_sb, wq, hp), None))
                for c in range(TOKC):
                    producers.append((base + 8 * c - 8,
                                      proj_kq(xk_sb, wk_sb, hp, c, kh, True),
                                      None))
                for qc in range(QC):
                    producers.append((base + 32 * (qc // 2) - 8 + (qc % 2),
                                      proj_kq(xq_sb, wq_sb, hp, qc, qh,
                                              False), None))
            producers.sort(key=lambda e: e[0])
            producers = deque(producers)
            v_emit_wave = {}

            # AV bookkeeping
            av_fifo = deque()  # (job_idx, h, qc, kt, a_t, j_in_wave, wave)
            av_state = {"tile": None, "round": -1}

            def finalize_round(r):
                av = av_state["tile"]
                h, qcp = divmod(r, NQCP)
                o_sb = opool.tile([P, 2, QC, HD], F32, tag="o_sb",
                                  name=f"osb_{r}")
                nc.vector.tensor_scalar_mul(
                    o_sb[:],
                    av[:].rearrange("p (i qt d) -> p i qt d", i=2, qt=QC),
                    1.0 / WS)
                for i in range(2):
                    qc = 2 * qcp + i
                    dst = o[qc * 512:(qc + 1) * 512,
                            h * HD:(h + 1) * HD].rearrange(
                                "(qt p) d -> p qt d", p=P)
                    nc.sync.dma_start(dst, o_sb[:, i])
                av_state["tile"] = None

            def drain_avs(cur_wave, final=False):
                budget = 6  # cap per-wave AV emission so a backlog burst
                # never parks in front of the score stream in the in-order
                # PE queue
                while av_fifo:
                    job, h, qc, kt, a_t, j, w = av_fifo[0]
                    if not final:
                        if budget <= 0:
                            break
                        if w >= cur_wave:
                            break
                        vw = v_emit_wave.get(kt)
                        if vw is None or vw >= cur_wave:
                            break
                        budget -= 1
                    av_fifo.popleft()
                    r = job // RJOBS
                    if r != av_state["round"]:
                        if av_state["tile"] is not None:
                            finalize_round(av_state["round"])
                        av_state["tile"] = ps_pool.tile(
                            [P, 512], F32, tag="av", bufs=1, name=f"av_{r}")
                        av_state["round"] = r
                    av = av_state["tile"]
                    i = qc % 2
                    first = (kt == 0 and i == 0)
                    last = (kt == NKT - 1 and i == 1)
                    for qt in range(4):
                        nc.tensor.matmul(
                            av[:, (i * 4 + qt) * HD:(i * 4 + qt + 1) * HD],
                            lhsT=a_t[:, j, qt * P:(qt + 1) * P],
                            rhs=v_sb[:, kt, h * HD:(h + 1) * HD],
                            start=(first and qt == 0),
                            stop=(last and qt == 3),
                            skip_group_check=True,
                        )

            # ---------- main wave loop ----------
            def drain_producers(w, job_base):
                while producers and producers[0][0] <= job_base + 2:
                    due, closure, vtile = producers.popleft()
                    closure()
                    if vtile is not None:
                        v_emit_wave[vtile] = w

            job_base = 0
            for w, wave in enumerate(waves):
                drain_producers(w, job_base)
                g = len(wave)
                st = ps_pool.tile([P, 3, 512], F32, tag="st", bufs=2,
                                  name=f"st_{w}")
                for j, (h, qc, kt) in enumerate(wave):
                    hp, pb = h // 2, (h % 2) * HD
                    lhsT = kh[pb:pb + HD, hp, :, kt * P:(kt + 1) * P]
                    for half in range(2):
                        rhs = qh[pb:pb + HD, hp,
                                 qc * 512 + half * 256:
                                 qc * 512 + (half + 1) * 256]
                        rhs = rhs.unsqueeze(1).broadcast_to([HD, 2, 256])
                        nc.tensor.matmul(
                            st[:, j, half * 256:(half + 1) * 256],
                            lhsT=lhsT,
                            rhs=rhs,
                            start=True,
                            stop=True,
                            perf_mode=mybir.MatmulPerfMode.DoubleRow,
                            tile_position=(pb, 0),
                            skip_group_check=True,
                        )
                a_t = apool.tile([P, 3, 512], BF16, tag="a_t", name=f"a_{w}")
                nc.scalar.activation(
                    out=a_t[:, :g, :],
                    in_=st[:, :g, :],
                    func=mybir.ActivationFunctionType.Sigmoid,
                    scale=0.125 / (WS * WS),
                )
                for j, (h, qc, kt) in enumerate(wave):
                    av_fifo.append((h * 64 + (qc // 2) * 32 + kt * 2
                                    + (qc % 2), h, qc, kt, a_t, j, w))
                drain_avs(w)
                job_base += g
            while producers:
                producers.popleft()[1]()
            drain_avs(0, final=True)
            finalize_round(av_state["round"])

    nc.compile()
    return nc


def _prep_core_inputs(q, k, v, Wq, bq, Wk, bk, Wv, bv, KTC):
    """Host-side shard + transpose + split-fp8 packing. in_maps for 8 cores."""
    import ml_dtypes
    E4 = ml_dtypes.float8_e4m3
    KA = KTC * P
    aug = KA > D

    def split8(a):
        """[R, C] fp32 -> [R, 2, C] fp8 (hi, lo)."""
        hi = a.astype(E4)
        lo = (a - hi.astype(np.float32)).astype(E4)
        return np.ascontiguousarray(np.stack([hi, lo], axis=1))

    def x_t(x_b):  # [S, D] -> [KA, 2, S] fp8
        xt = np.ascontiguousarray(x_b.T)
        if aug:
            pad = np.zeros((KA, S), np.float32)
            pad[:D] = xt
            pad[D] = 1.0
            xt = pad
        return split8(xt)

    def w_kq(W, b, half):  # -> [NHP, P, KTC*2*128] fp8, p-major
        ws = np.ascontiguousarray(W[:, half * OF:(half + 1) * OF]) * WS
        if aug:
            pad = np.zeros((KA, OF), np.float32)
            pad[:D] = ws
            pad[D] = b[half * OF:(half + 1) * OF] * WS
            ws = pad
        s8 = split8(ws)  # [KA, 2, OF]
        pm = s8.reshape(KTC, P, 2, NHP, P).transpose(3, 1, 0, 2, 4)
        return np.ascontiguousarray(pm.reshape(NHP, P, KTC * 2 * P))

    def w_v(W, b, half):  # -> [KA, 2, OF] fp8
        ws = np.ascontiguousarray(W[:, half * OF:(half + 1) * OF]) * WS
        if aug:
            pad = np.zeros((KA, OF), np.float32)
            pad[:D] = ws
            pad[D] = b[half * OF:(half + 1) * OF] * WS
            ws = pad
        return split8(ws)

    xts = {}
    in_maps = []
    for c in range(N_CORES):
        b, half = divmod(c, 2)
        if b not in xts:
            xts[b] = (x_t(q[b]), x_t(k[b]), x_t(v[b]))
        xq_c, xk_c, xv_c = xts[b]
        in_maps.append({
            "xq": xq_c,
            "xk": xk_c,
            "xv": xv_c,
            "wq": w_kq(Wq, bq, half),
            "wk": w_kq(Wk, bk, half),
            "wv": w_v(Wv, bv, half),
        })
    return in_maps


def kernel(q, k, v, Wq, bq, Wk, bk, Wv, bv):
    global last_results
    q = np.ascontiguousarray(np.asarray(q, np.float32))
    k = np.ascontiguousarray(np.asarray(k, np.float32))
    v = np.ascontiguousarray(np.asarray(v, np.float32))
    Wq = np.asarray(Wq, np.float32)
    Wk = np.asarray(Wk, np.float32)
    Wv = np.asarray(Wv, np.float32)
    bq = np.asarray(bq, np.float32)
    bk = np.asarray(bk, np.float32)
    bv = np.asarray(bv, np.float32)

    aug = any(np.any(b_) for b_ in (bq, bk, bv))
    KTC = (D // P) + (1 if aug else 0)

    if KTC not in _cache:
        _cache[KTC] = _build(KTC)
    nc = _cache[KTC]

    in_maps = _prep_core_inputs(q, k, v, Wq, bq, Wk, bk, Wv, bv, KTC)
    res = run_bass_kernel_spmd(nc, in_maps, core_ids=list(range(N_CORES)))
    last_results = res

    out = np.empty((B, S, D), np.float32)
    for c in range(N_CORES):
        b, half = divmod(c, 2)
        out[b, :, half * OF:(half + 1) * OF] = res.results[c]["o"]
    return out



# revision 20
# speedup vs baseline: 1.1521x; 1.1521x over previous
"""Trainium2 Bass kernel for nn_MultiHeadAttention_69106023793143.

Reference computation (B=4, S=2048, D=1024, H=16, HD=64):
    qh = split_heads(q @ Wq + bq); kh, vh likewise
    out = merge_heads(sigmoid((qh @ kh^T) / sqrt(HD)) @ vh)

Sharding (8 cores): core c handles batch b = c//2 and the half = c%2 slice of
the feature axis (512 features = 8 heads).  Projections are tensor-parallel on
the output dim of Wq/Wk/Wv; attention is head-parallel.

Device strategy per core (ACT-engine-paced pipeline):
  - The sigmoid over all 8*2048*2048 scores is the hard floor (ACT processes
    1 elem/cycle/partition @1.2GHz ~= 218us/core); everything else is
    scheduled to hide under the ACT stream.
  - Projections run as split-fp8 DoubleRow matmuls: host ships x and W as
    (hi, lo) fp8e4 pairs (same bytes as bf16) and the product takes the three
    cross terms xh*Wh + xh*Wl + xl*Wh - ~bf16 accuracy at 0.75x the bf16 PE
    cost.  W is host-scaled by 16 so its N(0, 1/1024) entries stay in e4m3's
    normal range; the 1/256 score scale folds into the sigmoid affine and a
    1/16 into the output copy.
  - Q^T/K^T land as [of, tok] with head PAIRS stacked on the 128 partitions,
    V natural [tok, of].
  - Scores use fp8e4 DoubleRow matmuls at 0.5 cycles/row: kh is stored as an
    (hi, lo) fp8 pair - the two DoubleRow K-blocks - so K-side quantization
    error is compensated; qh is plain fp8 broadcast across the two blocks
    (stride-0 AP).  Odd heads live on partitions 64:128 (tile_position
    (64, 0)).  End-to-end max rel err ~1.35e-2 (q-side fp8 only), vs the
    2e-2 budget.
  - Sigmoid on ScalarE in alternating 3-bank/2-bank PSUM waves, writing attn
    directly as bf16 to SBUF.
  - Attention runs in rounds of (head, q-chunk-PAIR) with the k-token axis
    OUTERMOST inside a round: jobs (h, qcp, kt, i).  A round therefore takes
    ~13 ACT waves to sweep the k tokens, which spreads the xk-chunk DMA and
    V-projection deadlines far enough apart that the serial DMA engines and
    the PE projection stream keep up with ACT from the start (a (h, qc)
    round with kt innermost sweeps all 2048 k tokens in 6 waves - the DMA
    can't feed that).
  - AV in bf16 with out[q, d] layout: lhsT = attn^T tile [128k, 128q],
    rhs = v [128k, 64d] -> only 64 free rows per matmul.  Each round
    accumulates its 8 q-tiles (2 q-chunks x 4) as interleaved sub-bank
    groups in ONE PSUM bank: start=True only on the round's first matmul
    (the PSUM zero-region covers the whole 2KB bank and zeroes on first
    touch per byte), stop=True on the last.
  - PSUM: 3+2 score banks + 2 projection banks (double-buffered so the
    projection pipeline never serializes on its DVE drain) + 1 AV bank = 8.
  - Wk/Wq are shipped p-major ([NHP, P, KTC*2*128]) so a head-pair slice is
    one contiguous 2KB-per-partition DMA; input DMAs are emitted up front in
    deadline order (the DMA engines are a serial resource in practice).
  - AV matmuls drain from a FIFO gated on their V-tile's emission so the
    in-order PE queue never head-of-line blocks on a V projection.
  - Nonzero biases fold in via a host-side augmented ones-row (KTC=9).
"""

import sys

if "/opt/trn_rl_repo" not in sys.path:
    sys.path.insert(0, "/opt/trn_rl_repo")

from collections import deque
from contextlib import ExitStack

import numpy as np

import concourse.tile as tile
from concourse import bacc, mybir
from concourse import dve_ops as _dve_ops
from concourse.bass_utils import run_bass_kernel_spmd
from concourse.dve_spec import C0, C1, C2, One, Spec, Src0, Zero, lower, maxx, minn, sq
from concourse.dve_uop import DveOpSpec

# ---- custom DVE op: clamped odd-quintic tanh approximation -----------------
# p(u) = u*(C1 + u^2*(C2 + u^2)), u = clamp(Src0*C0, -1, 1)  [8 ALU stages]
# Approximates tanh(k_fit * s / C0_rel ...): with C0 = K_FIT*raw_scale it
# computes tanh(s_true/2) to 3.5e-3 weighted RMS over s_true ~ N(0,1)
# (max err 0.034 at the |s|~4 clamp shoulder).  The quintic coefficient is
# slaved to 1 in u-units, which keeps the expression inside the DVE's
# 8-stage budget with only 3 scalar slots.
_TANH_NAME = "TANH_PC5_ANT"
K_FIT = 0.25283828
C1_FIT = 1.94641582
C2_FIT = -1.95047264


def _tanh_pc5_ref(in0, in1, s0, s1, imm2):
    u = np.clip(np.asarray(in0, np.float32) * s0, -1.0, 1.0)
    u2 = u * u
    return u * (s1 + u2 * (imm2 + u2))


def _register_tanh_op():
    for op in _dve_ops.OPS:
        if op.name == _TANH_NAME:
            return op
    t = Src0 * C0
    u = maxx(minn(t, One), Zero - One)
    u2 = sq(u)
    spec = Spec(body=u * (C1 + u2 * (C2 + u2)), reference=_tanh_pc5_ref)
    shas = {
        ver: DveOpSpec(name=_TANH_NAME, uops=lower(spec, ver=ver)).sha(ver)
        for ver in ("v3", "v4")
    }
    op = _dve_ops.DveOp(_TANH_NAME, spec, subdim=False, uops_sha=shas)
    _dve_ops.OPS.append(op)
    _dve_ops.CUSTOM_DVE_SPECS[op.name] = spec
    _dve_ops._SUB_OPCODE_FOR_NAME[op.name] = (
        _dve_ops._CUSTOM_DVE_ROW_BASE + len(_dve_ops.OPS) - 1
    )
    return op


TANH_OP = _register_tanh_op()

B, S, D, H = 4, 2048, 1024, 16
HD = D // H  # 64
OF = D // 2  # 512 features (8 heads) per core
N_CORES = 8
P = 128
NH = 8          # heads per core
NHP = 4         # head pairs per core
QC = 4          # q-chunks of 512
NQCP = 2        # q-chunk pairs
NKT = 16        # k token tiles of 128
TOKC = 4        # x token chunks of 512
RJOBS = 2 * NKT  # jobs per round (2 q-chunks x 16 kt)
ABUFS = 24      # attn (a_t) wave buffers
# wave engine pattern: True -> DVE quintic, False -> ACT tanh (5:3 ACT:DVE)
DVE_PAT = (False, False, True, False, False, True, False, True)
WS = 16.0       # host-side W scale (keeps fp8 W out of subnormals)

F32 = mybir.dt.float32
BF16 = mybir.dt.bfloat16
FP8 = mybir.dt.float8e4

# the three split-fp8 cross terms (w level, x level)
TERMS = ((0, 0), (0, 1), (1, 0))

_cache: dict = {}
last_results = None


def _build(KTC: int):
    """KTC = contraction k-tiles for the projections (8, or 9 when biases are
    folded in via an augmented ones-row)."""
    nc = bacc.Bacc("TRN2", target_bir_lowering=False, debug=False,
                   num_devices=N_CORES, name="mha_sig4")
    KA = KTC * P
    NDR = KTC // 2   # DoubleRow kt-pairs per term
    AUGK = KTC % 2   # leftover kt (the ones-row) as plain fp8 matmul
    WFREE = KTC * 2 * P  # per-partition elements of one head-pair W slice

    xq = nc.dram_tensor("xq", [KA, 2, S], FP8, kind="ExternalInput")
    xk = nc.dram_tensor("xk", [KA, 2, S], FP8, kind="ExternalInput")
    xv = nc.dram_tensor("xv", [KA, 2, S], FP8, kind="ExternalInput")
    # wq/wk p-major: [head-pair, partition, kt*level*128]
    wq = nc.dram_tensor("wq", [NHP, P, WFREE], FP8, kind="ExternalInput")
    wk = nc.dram_tensor("wk", [NHP, P, WFREE], FP8, kind="ExternalInput")
    wv = nc.dram_tensor("wv", [KA, 2, OF], FP8, kind="ExternalInput")
    o = nc.dram_tensor("o", [S, OF], F32, kind="ExternalOutput")

    xq_r = xq.rearrange("(kt p) l t -> p kt l t", p=P)
    xk_r = xk.rearrange("(kt p) l t -> p kt l t", p=P)
    xv_r = xv.rearrange("(kt p) l t -> p kt l t", p=P)
    wv_r = wv.rearrange("(kt p) l n -> p kt l n", p=P)

    abufs = ABUFS if KTC == 8 else 10

    with tile.TileContext(nc) as tc:
        with ExitStack() as ctx:
            persist = ctx.enter_context(tc.tile_pool(name="persist", bufs=1))
            xvpool = ctx.enter_context(tc.tile_pool(name="xvpool", bufs=2))
            apool = ctx.enter_context(tc.tile_pool(name="apool", bufs=abufs))
            opool = ctx.enter_context(tc.tile_pool(name="opool", bufs=2))
            ps_pool = ctx.enter_context(
                tc.tile_pool(name="ps_pool", bufs=2, space="PSUM"))

            wk_sb = persist.tile([P, NHP, KTC, 2, P], FP8)
            wq_sb = persist.tile([P, NHP, KTC, 2, P], FP8)
            wv_sb = persist.tile([P, KTC, 2, OF], FP8)
            xk_sb = persist.tile([P, KTC, 2, S], FP8)
            xq_sb = persist.tile([P, KTC, 2, S], FP8)
            # kh as (hi, lo) fp8 pair, head pairs stacked on partitions;
            # qh plain fp8; v bf16 [tok, of]
            kh = persist.tile([P, NHP, 2, S], FP8)
            qh = persist.tile([P, NHP, S], FP8)
            v_sb = persist.tile([P, NKT, OF], BF16)
            # (Σ_k v)·WS·(0.5/WS) per head, broadcast-added at the o drain
            sv_sb = persist.tile([P, NH, HD], BF16)
            ones_sb = persist.tile([P, P], BF16)
            nc.vector.memset(ones_sb[:], 1.0)

            # ONE PSUM bank for all projections: two [P, 256] slots in a
            # persistent tile, manually rotated. Region-based dep tracking
            # gives WAR/WAW per slot; each half-group's start=True re-marks
            # the whole bank but PSUM reads return raw data for re-marked
            # bytes (hardware-verified), and no other slot is ever
            # mid-accumulation when a start executes (serial emission).
            proj_ps = ps_pool.tile([P, 2, 256], F32, tag="proj", bufs=1)
            _slot = {"i": 0}

            def next_slot():
                s = _slot["i"] % 2
                _slot["i"] += 1
                return proj_ps[:, s, :]

            # ---------- engine-balance ledger ----------
            # planned busy ns for ACT / DVE; drains charge DVE (or ACT) at
            # emission so the per-wave greedy pick stays globally balanced
            est = {"ACT": 0.0, "DVE": 0.0}

            def charge(eng, n_elems, ov=None):
                est[eng] += n_elems * 0.833 + 171 if eng == "ACT" \
                    else n_elems * 1.042 + 61

            # ---------- producer closures ----------
            def dma_w_hp(w_sb, w_dram, hp):
                def run():
                    nc.sync.dma_start(
                        w_sb[:, hp].rearrange("p kt l n -> p (kt l n)"),
                        w_dram[hp])
                return run

            def dma_wv():
                def run():
                    nc.sync.dma_start(wv_sb[:], wv_r)
                return run

            def dma_x(x_sb, x_r, lo, hi):
                def run():
                    for lv in range(2):
                        nc.sync.dma_start(
                            x_sb[:, :, lv, lo:hi],
                            x_r[:, :, lv, lo:hi])
                return run

            xv_tiles = {}

            def dma_xv(c, lv):
                def run():
                    if lv == 0:
                        xv_tiles[c] = xvpool.tile([P, KTC, 2, 512], FP8,
                                                  tag="xvchunk",
                                                  name=f"xv_{c}")
                    nc.sync.dma_start(
                        xv_tiles[c][:, :, lv, :],
                        xv_r[:, :, lv, c * 512:(c + 1) * 512])
                return run

            # warm-up: the PE runs at 0.65/1.2GHz until ~3us of continuous
            # execution; burn dummy matmuls during the prefix DMAs so the
            # first projections run at full clock
            wu_sb = persist.tile([HD, 2, 640], FP8)

            def warmup(n_mm):
                def run():
                    if n_mm < 0:
                        nc.vector.memset(wu_sb[:], 0)
                        return
                    st = ps_pool.tile([P, 2, 512], F32, tag="st", bufs=3,
                                      name=f"wu_{n_mm}")
                    for m in range(n_mm):
                        nc.tensor.matmul(
                            st[:, m % 2, :],
                            lhsT=wu_sb[:, :, 0:P],
                            rhs=wu_sb[:, :, P:P + 512],
                            start=True, stop=True,
                            perf_mode=mybir.MatmulPerfMode.DoubleRow,
                            skip_group_check=True,
                        )
                return run

            # split-emission prefix projections: the hi terms only need the
            # lv0 (hi) half of the x chunk, so they start ~1.5us earlier
            _prefix_ps = {}

            def _kq_half(x_sb, w_sb, hp, tsl, ps, terms, first):
                n = 0
                for lw, lx in terms:
                    for t in range(NDR):
                        nc.tensor.matmul(
                            ps,
                            lhsT=w_sb[:, hp, 2 * t:2 * t + 2, lw, :],
                            rhs=x_sb[:, 2 * t:2 * t + 2, lx, tsl],
                            start=(first and n == 0), stop=False,
                            perf_mode=mybir.MatmulPerfMode.DoubleRow,
                            skip_group_check=True,
                        )
                        n += 1

            def _kq_drain(dst, hp, tsl, ps, split_lo):
                charge("DVE", 512 if split_lo else 256)
                if split_lo:
                    charge("DVE", 256)
                    nc.vector.tensor_copy(out=dst[:, hp, 0, tsl], in_=ps)
                    nc.vector.tensor_sub(dst[:, hp, 1, tsl], ps,
                                         dst[:, hp, 0, tsl])
                else:
                    nc.vector.tensor_copy(out=dst[:, hp, tsl], in_=ps)

            def proj_kq_hi(x_sb, w_sb, hp, c, key):
                def run():
                    ps = next_slot()
                    _prefix_ps[key] = ps
                    tsl = slice(c * 512, c * 512 + 256)
                    _kq_half(x_sb, w_sb, hp, tsl, ps, ((0, 0), (1, 0)), True)
                return run

            def proj_kq_lo(x_sb, w_sb, hp, c, dst, split_lo, key):
                def run():
                    ps = _prefix_ps.pop(key)
                    tsl = slice(c * 512, c * 512 + 256)
                    _kq_half(x_sb, w_sb, hp, tsl, ps, ((0, 1),), False)
                    _kq_drain(dst, hp, tsl, ps, split_lo)
                    ps2 = next_slot()
                    tsl2 = slice(c * 512 + 256, (c + 1) * 512)
                    _kq_half(x_sb, w_sb, hp, tsl2, ps2,
                             ((0, 0), (1, 0), (0, 1)), True)
                    _kq_drain(dst, hp, tsl2, ps2, split_lo)
                return run

            def dma_x_lv(x_sb, x_r, lo, hi, lv):
                def run():
                    nc.sync.dma_start(
                        x_sb[:, :, lv, lo:hi],
                        x_r[:, :, lv, lo:hi])
                return run

            def proj_kq(x_sb, w_sb, hp, c, dst, split_lo):
                """dst slice [of-pair, tok chunk c] for head-pair hp, as
                two [P, 256] slot halves in the shared proj bank."""
                def run():
                    for half in range(2):
                        ps = next_slot()
                        tsl = slice(c * 512 + half * 256,
                                    c * 512 + (half + 1) * 256)
                        n = 0
                        for lw, lx in TERMS:
                            for t in range(NDR):
                                nc.tensor.matmul(
                                    ps,
                                    lhsT=w_sb[:, hp, 2 * t:2 * t + 2, lw, :],
                                    rhs=x_sb[:, 2 * t:2 * t + 2, lx, tsl],
                                    start=(n == 0), stop=False,
                                    perf_mode=mybir.MatmulPerfMode.DoubleRow,
                                    skip_group_check=True,
                                )
                                n += 1
                            if AUGK:
                                nc.tensor.matmul(
                                    ps,
                                    lhsT=w_sb[:, hp, KTC - 1, lw, :],
                                    rhs=x_sb[:, KTC - 1, lx, tsl],
                                    start=(n == 0), stop=False,
                                    skip_group_check=True,
                                )
                                n += 1
                    # halves always land on slots (0, 1): one wide drain
                    wide = proj_ps[:].rearrange("p s n -> p (s n)")
                    sl = slice(c * 512, (c + 1) * 512)
                    if split_lo:
                        charge("DVE", 512)
                        charge("DVE", 512)
                        nc.vector.tensor_copy(out=dst[:, hp, 0, sl], in_=wide)
                        nc.vector.tensor_sub(dst[:, hp, 1, sl], wide,
                                             dst[:, hp, 0, sl])
                    else:
                        charge("DVE", 512)
                        nc.vector.tensor_copy(out=dst[:, hp, sl], in_=wide)
                return run

            _v_ps = {}

            def proj_v(t, part):
                """v_sb[:, t, :] = x-token-tile t @ Wv ([tok, of]).
                part 'hi' takes the two x-hi terms, 'lo' the x-lo term +
                the PSUM drain (finer interleaving in the in-order PE queue,
                and 'hi' only needs the lv0 half of the xv chunk)."""
                def run():
                    xt = xv_tiles[t // 4]
                    tsl = slice((t % 4) * P, (t % 4 + 1) * P)

                    def v_half(ps, osl, terms, first):
                        n = 0
                        for lw, lx in terms:
                            for u in range(NDR):
                                nc.tensor.matmul(
                                    ps,
                                    lhsT=xt[:, 2 * u:2 * u + 2, lx, tsl],
                                    rhs=wv_sb[:, 2 * u:2 * u + 2, lw, osl],
                                    start=(first and n == 0), stop=False,
                                    perf_mode=mybir.MatmulPerfMode.DoubleRow,
                                    skip_group_check=True,
                                )
                                n += 1
                            if AUGK:
                                nc.tensor.matmul(
                                    ps,
                                    lhsT=xt[:, KTC - 1, lx, tsl],
                                    rhs=wv_sb[:, KTC - 1, lw, osl],
                                    start=(first and n == 0), stop=False,
                                    skip_group_check=True,
                                )
                                n += 1

                    if part == "hi":
                        ps = next_slot()
                        _v_ps[t] = ps
                        v_half(ps, slice(0, 256), ((0, 0), (1, 0)), True)
                    else:
                        ps = _v_ps.pop(t)
                        v_half(ps, slice(0, 256), ((0, 1),), False)
                        ps2 = next_slot()
                        v_half(ps2, slice(256, 512),
                               ((0, 0), (1, 0), (0, 1)), True)
                        charge("DVE", 512)
                        nc.vector.tensor_copy(
                            out=v_sb[:, t, :],
                            in_=proj_ps[:].rearrange("p s n -> p (s n)"))
                return run

            # ---------- static schedule ----------
            # rounds in head-pair blocks: (2hp,0),(2hp+1,0),(2hp,1),(2hp+1,1)
            # so the early rounds need only qh chunks 0/1 -- xq c2/c3 (2MB)
            # can ship after the whole xk/xv stream.  Round index
            # r(h,qc) = 4*(h//2) + 2*(qc//2) + h%2; job = r*32 + kt*2 + i.
            # Round 0 staggers its second q-chunk by 2 k-tiles so the first
            # waves only need qh[qc0] (whose projection finishes first).
            rounds = [(2 * hp + (s % 2), s // 2)
                      for hp in range(NHP) for s in range(4)]
            jobs = []
            for kt in range(NKT + 2):
                if kt < NKT:
                    jobs.append((0, 0, kt))
                if kt >= 2:
                    jobs.append((0, 1, kt - 2))
            for h, qcp in rounds[1:]:
                jobs += [(h, 2 * qcp + i, kt)
                         for kt in range(NKT) for i in range(2)]
            waves = [jobs[i0:i0 + 2] for i0 in range(0, len(jobs), 2)]

            producers = []  # (due_job, closure, vtile_or_None)
            # prefix + all input DMAs in deadline order (the DMA engines are
            # effectively serial; emission order = transfer order)
            if AUGK == 0:
                producers += [
                    (-99.9, warmup(-1), None),
                    (-99.8, warmup(10), None),
                    (-99.0, dma_w_hp(wk_sb, wk, 0), None),
                    (-98.9, dma_x_lv(xk_sb, xk_r, 0, 512, 0), None),
                    (-98.8, dma_w_hp(wq_sb, wq, 0), None),
                    (-98.7, proj_kq_hi(xk_sb, wk_sb, 0, 0, "k00"), None),
                    (-98.6, dma_x_lv(xk_sb, xk_r, 0, 512, 1), None),
                    (-98.5, proj_kq_lo(xk_sb, wk_sb, 0, 0, kh, True, "k00"),
                     None),
                    (-98.4, dma_x_lv(xq_sb, xq_r, 0, 512, 0), None),
                    (-98.3, warmup(4), None),
                    (-98.2, proj_kq_hi(xq_sb, wq_sb, 0, 0, "q00"), None),
                    (-98.1, dma_x_lv(xq_sb, xq_r, 0, 512, 1), None),
                    (-98.0, proj_kq_lo(xq_sb, wq_sb, 0, 0, qh, False, "q00"),
                     None),
                    (-97.9, dma_x_lv(xq_sb, xq_r, 512, 1024, 0), None),
                    (-97.8, proj_kq_hi(xq_sb, wq_sb, 0, 1, "q01"), None),
                    (-97.7, dma_x_lv(xq_sb, xq_r, 512, 1024, 1), None),
                    (-97.6, proj_kq_lo(xq_sb, wq_sb, 0, 1, qh, False, "q01"),
                     None),
                ]
            else:
                producers += [
                    (-99.0, dma_w_hp(wk_sb, wk, 0), None),
                    (-98.8, dma_x(xk_sb, xk_r, 0, 512), None),
                    (-98.6, dma_w_hp(wq_sb, wq, 0), None),
                    (-98.4, dma_x(xq_sb, xq_r, 0, 512), None),
                    (-98.2, proj_kq(xk_sb, wk_sb, 0, 0, kh, True), None),
                    (-98.0, proj_kq(xq_sb, wq_sb, 0, 0, qh, False), None),
                    (-97.8, dma_x(xq_sb, xq_r, 512, 1024), None),
                    (-97.6, proj_kq(xq_sb, wq_sb, 0, 1, qh, False), None),
                ]
            # Σv per head: ones-lhsT matmuls into the (idle until hp1) proj
            # bank once all v tiles are in SBUF, then one pre-scaled drain.
            def sv_accum():
                def run():
                    wide = proj_ps[:].rearrange("p s n -> p (s n)")
                    n = 0
                    for h in range(NH):
                        for kt in range(NKT):
                            nc.tensor.matmul(
                                wide[:, h * HD:(h + 1) * HD],
                                lhsT=ones_sb[:],
                                rhs=v_sb[:, kt, h * HD:(h + 1) * HD],
                                start=(n == 0), stop=(n == NH * NKT - 1),
                                skip_group_check=True,
                            )
                            n += 1
                return run

            def sv_drain():
                def run():
                    charge("DVE", 512)
                    nc.vector.tensor_scalar_mul(
                        sv_sb[:],
                        proj_ps[:].rearrange("p s (h2 d) -> p (s h2) d", d=HD),
                        0.5 / WS)
                return run

            # input stream in DMA-arrival order (the DMA engines are
            # effectively serial; emission order = transfer order); PE
            # producers positioned at their consumer's job index so the
            # in-order PE queue never inverts the arrival order.
            if AUGK == 0:
                producers += [
                    # xk c1 -> kh c1 (kt 4-7, needed from job ~8)
                    (-89.8, dma_x_lv(xk_sb, xk_r, 512, 1024, 0), None),
                    (-89.7, proj_kq_hi(xk_sb, wk_sb, 0, 1, "k01"), None),
                    (-89.6, dma_x_lv(xk_sb, xk_r, 512, 1024, 1), None),
                    (-89.5, proj_kq_lo(xk_sb, wk_sb, 0, 1, kh, True, "k01"),
                     None),
                ]
            else:
                producers.append((-89.8, dma_x(xk_sb, xk_r, 512, 1024), None))
                producers.append((2, proj_kq(xk_sb, wk_sb, 0, 1, kh, True),
                                  None))
            # V stream: wv + xv c0 right behind xk c1
            producers.append((-89.4, dma_wv(), None))
            producers.append((-89.3, dma_xv(0, 0), None))
            producers.append((-89.2, dma_xv(0, 1), None))
            for t in range(4):
                producers.append((12 + 0.1 * t, proj_v(t, "hi"), None))
                producers.append((12.05 + 0.1 * t, proj_v(t, "lo"), t))
            # xk c2 -> kh c2 (kt 8-11, needed from job ~16)
            if AUGK == 0:
                producers += [
                    (13.0, dma_x_lv(xk_sb, xk_r, 1024, 1536, 0), None),
                    (13.1, proj_kq_hi(xk_sb, wk_sb, 0, 2, "k02"), None),
                    (13.2, dma_x_lv(xk_sb, xk_r, 1024, 1536, 1), None),
                    (13.3, proj_kq_lo(xk_sb, wk_sb, 0, 2, kh, True, "k02"),
                     None),
                ]
            else:
                producers.append((13.0, dma_x(xk_sb, xk_r, 1024, 1536), None))
                producers.append((13.3, proj_kq(xk_sb, wk_sb, 0, 2, kh, True),
                                  None))
            producers.append((13.4, dma_xv(1, 0), None))
            producers.append((13.5, dma_xv(1, 1), None))
            for t in range(4, 8):
                producers.append((16 + 0.1 * t, proj_v(t, "hi"), None))
                producers.append((16.05 + 0.1 * t, proj_v(t, "lo"), t))
            # xk c3 -> kh c3 (kt 12-15, needed from job ~24)
            producers.append((18.0, dma_x(xk_sb, xk_r, 1536, 2048), None))
            producers.append((18.1, proj_kq(xk_sb, wk_sb, 0, 3, kh, True),
                              None))
            producers.append((18.2, dma_xv(2, 0), None))
            producers.append((18.3, dma_xv(2, 1), None))
            for t in range(8, 12):
                producers.append((22 + 0.1 * t, proj_v(t, "hi"), None))
                producers.append((22.05 + 0.1 * t, proj_v(t, "lo"), t))
            producers.append((24.0, dma_xv(3, 0), None))
            producers.append((24.1, dma_xv(3, 1), None))
            for t in range(12, NKT):
                producers.append((26 + 0.1 * t, proj_v(t, "hi"), None))
                producers.append((26.05 + 0.1 * t, proj_v(t, "lo"), t))
            producers.append((30.0, sv_accum(), None))
            producers.append((30.1, sv_drain(), None))
            # xq c2/c3 ship last; qh c2/3 for hp0 first used at job 64
            producers.append((32.0, dma_x(xq_sb, xq_r, 1024, 1536), None))
            producers.append((32.5, dma_x(xq_sb, xq_r, 1536, 2048), None))
            producers.append((48, proj_kq(xq_sb, wq_sb, 0, 2, qh, False),
                              None))
            producers.append((49, proj_kq(xq_sb, wq_sb, 0, 3, qh, False),
                              None))
            # hp1-3: W DMAs + K/Q projections just-in-time for their blocks
            for hp in range(1, NHP):
                base = 128 * hp
                producers.append((base - 40, dma_w_hp(wk_sb, wk, hp), None))
                producers.append((base - 38, dma_w_hp(wq_sb, wq, hp), None))
                for c in range(TOKC):
                    producers.append((base + 8 * c - 14,
                                      proj_kq(xk_sb, wk_sb, hp, c, kh, True),
                                      None))
                producers.append((base - 10,
                                  proj_kq(xq_sb, wq_sb, hp, 0, qh, False),
                                  None))
                producers.append((base - 9,
                                  proj_kq(xq_sb, wq_sb, hp, 1, qh, False),
                                  None))
                producers.append((base + 50,
                                  proj_kq(xq_sb, wq_sb, hp, 2, qh, False),
                                  None))
                producers.append((base + 51,
                                  proj_kq(xq_sb, wq_sb, hp, 3, qh, False),
                                  None))
            producers.sort(key=lambda e: e[0])
            producers = deque(producers)
            v_emit_wave = {}

            # AV bookkeeping
            av_fifo = deque()  # (job_idx, h, qc, kt, a_t, j_in_wave, wave)
            av_state = {"tile": None, "round": -1}

            def finalize_round(r):
                av = av_state["tile"]
                hp, s = divmod(r, 4)
                h, qcp = 2 * hp + (s % 2), s // 2
                o_sb = opool.tile([P, 2, QC, HD], F32, tag="o_sb",
                                  name=f"osb_{r}")
                # o = (Σ attn'·v + Σv)·(1/(2·WS)) with attn' = tanh(s/2):
                # the sv term supplies the +Σv, pre-scaled by 0.5/WS
                charge("DVE", 512)
                nc.vector.affine_then_add(
                    out=o_sb[:].rearrange("p i qt d -> p (i qt) d"),
                    in0=av[:].rearrange("p (g d) -> p g d", d=HD),
                    in1=sv_sb[:, h].unsqueeze(1).broadcast_to([P, 2 * QC, HD]),
                    scale=0.5 / WS,
                    bias=0.0,
                )
                for i in range(2):
                    qc = 2 * qcp + i
                    dst = o[qc * 512:(qc + 1) * 512,
                            h * HD:(h + 1) * HD].rearrange(
                                "(qt p) d -> p qt d", p=P)
                    nc.sync.dma_start(dst, o_sb[:, i])
                av_state["tile"] = None

            def drain_avs(cur_wave, final=False):
                budget = 6  # cap per-wave AV emission so a backlog burst
                # never parks in front of the score stream in the in-order
                # PE queue
                while av_fifo:
                    job, h, qc, kt, a_t, j, w = av_fifo[0]
                    if not final:
                        if budget <= 0:
                            break
                        if w >= cur_wave:
                            break
                        vw = v_emit_wave.get(kt)
                        if vw is None or vw >= cur_wave:
                            break
                        budget -= 1
                    av_fifo.popleft()
                    r = job // RJOBS
                    if r != av_state["round"]:
                        if av_state["tile"] is not None:
                            finalize_round(av_state["round"])
                        av_state["tile"] = ps_pool.tile(
                            [P, 512], F32, tag="av", bufs=1, name=f"av_{r}")
                        av_state["round"] = r
                    av = av_state["tile"]
                    i = qc % 2
                    first = (kt == 0 and i == 0)
                    last = (kt == NKT - 1 and i == 1)
                    for qt in range(4):
                        nc.tensor.matmul(
                            av[:, (i * 4 + qt) * HD:(i * 4 + qt + 1) * HD],
                            lhsT=a_t[:, j, qt * P:(qt + 1) * P],
                            rhs=v_sb[:, kt, h * HD:(h + 1) * HD],
                            start=(first and qt == 0),
                            stop=(last and qt == 3),
                            skip_group_check=True,
                        )

            # ---------- main wave loop ----------
            def drain_producers(w, job_base):
                while producers and producers[0][0] <= job_base + 2:
                    due, closure, vtile = producers.popleft()
                    closure()
                    if vtile is not None:
                        v_emit_wave[vtile] = w

            job_base = 0
            for w, wave in enumerate(waves):
                drain_producers(w, job_base)
                g = len(wave)
                st = ps_pool.tile([P, 2, 512], F32, tag="st", bufs=3,
                                  name=f"st_{w}")
                for j, (h, qc, kt) in enumerate(wave):
                    hp, pb = h // 2, (h % 2) * HD
                    lhsT = kh[pb:pb + HD, hp, :, kt * P:(kt + 1) * P]
                    for half in range(2):
                        rhs = qh[pb:pb + HD, hp,
                                 qc * 512 + half * 256:
                                 qc * 512 + (half + 1) * 256]
                        rhs = rhs.unsqueeze(1).broadcast_to([HD, 2, 256])
                        nc.tensor.matmul(
                            st[:, j, half * 256:(half + 1) * 256],
                            lhsT=lhsT,
                            rhs=rhs,
                            start=True,
                            stop=True,
                            perf_mode=mybir.MatmulPerfMode.DoubleRow,
                            tile_position=(pb, 0),
                            skip_group_check=True,
                        )
                a_t = apool.tile([P, 2, 512], BF16, tag="a_t", name=f"a_{w}")
                n_el = g * 512
                if est["DVE"] + n_el * 1.042 + 61 < est["ACT"] + n_el * 0.833 + 171:
                    # DVE share: clamped odd-quintic ~= tanh(s_true/2)
                    charge("DVE", n_el)
                    nc.vector._custom_dve(
                        TANH_OP,
                        out=a_t[:, :g, :],
                        in0=st[:, :g, :],
                        s0=K_FIT * 0.125 / (WS * WS),
                        s1=C1_FIT,
                        imm2=C2_FIT,
                    )
                else:
                    charge("ACT", n_el)
                    nc.scalar.activation(
                        out=a_t[:, :g, :],
                        in_=st[:, :g, :],
                        func=mybir.ActivationFunctionType.Tanh,
                        scale=0.0625 / (WS * WS),
                    )
                for j, (h, qc, kt) in enumerate(wave):
                    r = 4 * (h // 2) + 2 * (qc // 2) + (h % 2)
                    av_fifo.append((r * RJOBS + kt * 2 + (qc % 2),
                                    h, qc, kt, a_t, j, w))
                drain_avs(w)
                job_base += g
            import os
            if os.environ.get("KDBG"):
                print("EST at end:", est)
            while producers:
                producers.popleft()[1]()
            drain_avs(0, final=True)
            finalize_round(av_state["round"])

    nc.compile()
    return nc


def _prep_core_inputs(q, k, v, Wq, bq, Wk, bk, Wv, bv, KTC):
    """Host-side shard + transpose + split-fp8 packing. in_maps for 8 cores."""
    import ml_dtypes
    E4 = ml_dtypes.float8_e4m3
    KA = KTC * P
    aug = KA > D

    def split8(a):
        """[R, C] fp32 -> [R, 2, C] fp8 (hi, lo)."""
        hi = a.astype(E4)
        lo = (a - hi.astype(np.float32)).astype(E4)
        return np.ascontiguousarray(np.stack([hi, lo], axis=1))

    def x_t(x_b):  # [S, D] -> [KA, 2, S] fp8
        xt = np.ascontiguousarray(x_b.T)
        if aug:
            pad = np.zeros((KA, S), np.float32)
            pad[:D] = xt
            pad[D] = 1.0
            xt = pad
        return split8(xt)

    def w_kq(W, b, half):  # -> [NHP, P, KTC*2*128] fp8, p-major
        ws = np.ascontiguousarray(W[:, half * OF:(half + 1) * OF]) * WS
        if aug:
            pad = np.zeros((KA, OF), np.float32)
            pad[:D] = ws
            pad[D] = b[half * OF:(half + 1) * OF] * WS
            ws = pad
        s8 = split8(ws)  # [KA, 2, OF]
        pm = s8.reshape(KTC, P, 2, NHP, P).transpose(3, 1, 0, 2, 4)
        return np.ascontiguousarray(pm.reshape(NHP, P, KTC * 2 * P))

    def w_v(W, b, half):  # -> [KA, 2, OF] fp8
        ws = np.ascontiguousarray(W[:, half * OF:(half + 1) * OF]) * WS
        if aug:
            pad = np.zeros((KA, OF), np.float32)
            pad[:D] = ws
            pad[D] = b[half * OF:(half + 1) * OF] * WS
            ws = pad
        return split8(ws)

    xts = {}
    in_maps = []
    for c in range(N_CORES):
        b, half = divmod(c, 2)
        if b not in xts:
            xts[b] = (x_t(q[b]), x_t(k[b]), x_t(v[b]))
        xq_c, xk_c, xv_c = xts[b]
        in_maps.append({
            "xq": xq_c,
            "xk": xk_c,
            "xv": xv_c,
            "wq": w_kq(Wq, bq, half),
            "wk": w_kq(Wk, bk, half),
            "wv": w_v(Wv, bv, half),
        })
    return in_maps


def kernel(q, k, v, Wq, bq, Wk, bk, Wv, bv):
    global last_results
    q = np.ascontiguousarray(np.asarray(q, np.float32))
    k = np.ascontiguousarray(np.asarray(k, np.float32))
    v = np.ascontiguousarray(np.asarray(v, np.float32))
    Wq = np.asarray(Wq, np.float32)
    Wk = np.asarray(Wk, np.float32)
    Wv = np.asarray(Wv, np.float32)
    bq = np.asarray(bq, np.float32)
    bk = np.asarray(bk, np.float32)
    bv = np.asarray(bv, np.float32)

    aug = any(np.any(b_) for b_ in (bq, bk, bv))
    KTC = (D // P) + (1 if aug else 0)

    if KTC not in _cache:
        _cache[KTC] = _build(KTC)
    nc = _cache[KTC]

    in_maps = _prep_core_inputs(q, k, v, Wq, bq, Wk, bk, Wv, bv, KTC)
    res = run_bass_kernel_spmd(nc, in_maps, core_ids=list(range(N_CORES)))
    last_results = res

    out = np.empty((B, S, D), np.float32)
    for c in range(N_CORES):
        b, half = divmod(c, 2)
        out[b, :, half * OF:(half + 1) * OF] = res.results[c]["o"]
    return out



# revision 52
# speedup vs baseline: 1.1747x; 1.0197x over previous
"""Trainium2 Bass kernel for nn_MultiHeadAttention_69106023793143.

Reference computation (B=4, S=2048, D=1024, H=16, HD=64):
    qh = split_heads(q @ Wq + bq); kh, vh likewise
    out = merge_heads(sigmoid((qh @ kh^T) / sqrt(HD)) @ vh)

Sharding (8 cores): core c handles batch b = c//2 and the half = c%2 slice of
the feature axis (512 features = 8 heads).  Projections are tensor-parallel on
the output dim of Wq/Wk/Wv; attention is head-parallel.

Device strategy per core (ACT-engine-paced pipeline):
  - The sigmoid over all 8*2048*2048 scores is the hard floor (ACT processes
    1 elem/cycle/partition @1.2GHz ~= 218us/core); everything else is
    scheduled to hide under the ACT stream.
  - Projections run as split-fp8 DoubleRow matmuls: host ships x and W as
    (hi, lo) fp8e4 pairs (same bytes as bf16) and the product takes the three
    cross terms xh*Wh + xh*Wl + xl*Wh - ~bf16 accuracy at 0.75x the bf16 PE
    cost.  W is host-scaled by 16 so its N(0, 1/1024) entries stay in e4m3's
    normal range; the 1/256 score scale folds into the sigmoid affine and a
    1/16 into the output copy.
  - Q^T/K^T land as [of, tok] with head PAIRS stacked on the 128 partitions,
    V natural [tok, of].
  - Scores use fp8e4 DoubleRow matmuls at 0.5 cycles/row: kh is stored as an
    (hi, lo) fp8 pair - the two DoubleRow K-blocks - so K-side quantization
    error is compensated; qh is plain fp8 broadcast across the two blocks
    (stride-0 AP).  Odd heads live on partitions 64:128 (tile_position
    (64, 0)).  End-to-end max rel err ~1.35e-2 (q-side fp8 only), vs the
    2e-2 budget.
  - Sigmoid on ScalarE in alternating 3-bank/2-bank PSUM waves, writing attn
    directly as bf16 to SBUF.
  - Attention runs in rounds of (head, q-chunk-PAIR) with the k-token axis
    OUTERMOST inside a round: jobs (h, qcp, kt, i).  A round therefore takes
    ~13 ACT waves to sweep the k tokens, which spreads the xk-chunk DMA and
    V-projection deadlines far enough apart that the serial DMA engines and
    the PE projection stream keep up with ACT from the start (a (h, qc)
    round with kt innermost sweeps all 2048 k tokens in 6 waves - the DMA
    can't feed that).
  - AV in bf16 with out[q, d] layout: lhsT = attn^T tile [128k, 128q],
    rhs = v [128k, 64d] -> only 64 free rows per matmul.  Each round
    accumulates its 8 q-tiles (2 q-chunks x 4) as interleaved sub-bank
    groups in ONE PSUM bank: start=True only on the round's first matmul
    (the PSUM zero-region covers the whole 2KB bank and zeroes on first
    touch per byte), stop=True on the last.
  - PSUM: 3+2 score banks + 2 projection banks (double-buffered so the
    projection pipeline never serializes on its DVE drain) + 1 AV bank = 8.
  - Wk/Wq are shipped p-major ([NHP, P, KTC*2*128]) so a head-pair slice is
    one contiguous 2KB-per-partition DMA; input DMAs are emitted up front in
    deadline order (the DMA engines are a serial resource in practice).
  - AV matmuls drain from a FIFO gated on their V-tile's emission so the
    in-order PE queue never head-of-line blocks on a V projection.
  - Nonzero biases fold in via a host-side augmented ones-row (KTC=9).
"""

import sys

if "/opt/trn_rl_repo" not in sys.path:
    sys.path.insert(0, "/opt/trn_rl_repo")

from collections import deque
from contextlib import ExitStack

import numpy as np

import concourse.tile as tile
from concourse import bacc, mybir
from concourse import dve_ops as _dve_ops
from concourse.bass_utils import run_bass_kernel_spmd
from concourse.dve_spec import C0, C1, C2, One, Spec, Src0, Zero, lower, maxx, minn, sq
from concourse.dve_uop import DveOpSpec

# ---- custom DVE op: clamped odd-quintic tanh approximation -----------------
# p(u) = u*(C1 + u^2*(C2 + u^2)), u = clamp(Src0*C0, -1, 1)  [8 ALU stages]
# Approximates tanh(k_fit * s / C0_rel ...): with C0 = K_FIT*raw_scale it
# computes tanh(s_true/2) to 3.5e-3 weighted RMS over s_true ~ N(0,1)
# (max err 0.034 at the |s|~4 clamp shoulder).  The quintic coefficient is
# slaved to 1 in u-units, which keeps the expression inside the DVE's
# 8-stage budget with only 3 scalar slots.
_TANH_NAME = "TANH_PC5_ANT"
K_FIT = 0.25283828
C1_FIT = 1.94641582
C2_FIT = -1.95047264


def _tanh_pc5_ref(in0, in1, s0, s1, imm2):
    u = np.clip(np.asarray(in0, np.float32) * s0, -1.0, 1.0)
    u2 = u * u
    return u * (s1 + u2 * (imm2 + u2))


def _register_tanh_op():
    for op in _dve_ops.OPS:
        if op.name == _TANH_NAME:
            return op
    t = Src0 * C0
    u = maxx(minn(t, One), Zero - One)
    u2 = sq(u)
    spec = Spec(body=u * (C1 + u2 * (C2 + u2)), reference=_tanh_pc5_ref)
    shas = {
        ver: DveOpSpec(name=_TANH_NAME, uops=lower(spec, ver=ver)).sha(ver)
        for ver in ("v3", "v4")
    }
    op = _dve_ops.DveOp(_TANH_NAME, spec, subdim=False, uops_sha=shas)
    _dve_ops.OPS.append(op)
    _dve_ops.CUSTOM_DVE_SPECS[op.name] = spec
    _dve_ops._SUB_OPCODE_FOR_NAME[op.name] = (
        _dve_ops._CUSTOM_DVE_ROW_BASE + len(_dve_ops.OPS) - 1
    )
    return op


TANH_OP = _register_tanh_op()

B, S, D, H = 4, 2048, 1024, 16
HD = D // H  # 64
OF = D // 2  # 512 features (8 heads) per core
N_CORES = 8
P = 128
NH = 8          # heads per core
NHP = 4         # head pairs per core
QC = 4          # q-chunks of 512
NQCP = 2        # q-chunk pairs
NKT = 16        # k token tiles of 128
TOKC = 4        # x token chunks of 512
RJOBS = 2 * NKT  # jobs per round (2 q-chunks x 16 kt)
ABUFS = 28      # attn (a_t) wave buffers
# wave engine pattern: True -> DVE quintic, False -> ACT tanh (5:3 ACT:DVE)
DVE_PAT = (False, False, True, False, False, True, False, True)
WS = 16.0       # host-side W scale (keeps fp8 W out of subnormals)

F32 = mybir.dt.float32
BF16 = mybir.dt.bfloat16
FP8 = mybir.dt.float8e4

# the three split-fp8 cross terms (w level, x level)
TERMS = ((0, 0), (0, 1), (1, 0))

_cache: dict = {}
last_results = None


def _build(KTC: int):
    """KTC = contraction k-tiles for the projections (8, or 9 when biases are
    folded in via an augmented ones-row)."""
    nc = bacc.Bacc("TRN2", target_bir_lowering=False, debug=False,
                   num_devices=N_CORES, name="mha_sig4")
    KA = KTC * P
    NDR = KTC // 2   # DoubleRow kt-pairs per term
    AUGK = KTC % 2   # leftover kt (the ones-row) as plain fp8 matmul
    WFREE = KTC * 2 * P  # per-partition elements of one head-pair W slice

    xq = nc.dram_tensor("xq", [KA, 2, S], FP8, kind="ExternalInput")
    xk = nc.dram_tensor("xk", [KA, 2, S], FP8, kind="ExternalInput")
    xv = nc.dram_tensor("xv", [KA, 2, S], FP8, kind="ExternalInput")
    # wq/wk p-major: [head-pair, partition, kt*level*128]
    wq = nc.dram_tensor("wq", [NHP, P, WFREE], FP8, kind="ExternalInput")
    wk = nc.dram_tensor("wk", [NHP, P, WFREE], FP8, kind="ExternalInput")
    wv = nc.dram_tensor("wv", [KA, 2, OF], FP8, kind="ExternalInput")
    o = nc.dram_tensor("o", [S, OF], F32, kind="ExternalOutput")

    xq_r = xq.rearrange("(kt p) l t -> p kt l t", p=P)
    xk_r = xk.rearrange("(kt p) l t -> p kt l t", p=P)
    xv_r = xv.rearrange("(kt p) l t -> p kt l t", p=P)
    wv_r = wv.rearrange("(kt p) l n -> p kt l n", p=P)

    abufs = ABUFS if KTC == 8 else 10

    with tile.TileContext(nc) as tc:
        with ExitStack() as ctx:
            persist = ctx.enter_context(tc.tile_pool(name="persist", bufs=1))
            xvpool = ctx.enter_context(tc.tile_pool(name="xvpool", bufs=2))
            apool = ctx.enter_context(tc.tile_pool(name="apool", bufs=abufs))
            opool = ctx.enter_context(tc.tile_pool(name="opool", bufs=2))
            ps_pool = ctx.enter_context(
                tc.tile_pool(name="ps_pool", bufs=2, space="PSUM"))

            wk_sb = persist.tile([P, NHP, KTC, 2, P], FP8)
            wq_sb = persist.tile([P, NHP, KTC, 2, P], FP8)
            wv_sb = persist.tile([P, KTC, 2, OF], FP8)
            xk_sb = persist.tile([P, KTC, 2, S], FP8)
            xq_sb = persist.tile([P, KTC, 2, S], FP8)
            # kh as (hi, lo) fp8 pair, head pairs stacked on partitions;
            # qh plain fp8; v bf16 [tok, of]
            kh = persist.tile([P, NHP, 2, S], FP8)
            qh = persist.tile([P, NHP, S], FP8)
            v_sb = persist.tile([P, NKT, OF], BF16)
            # (Σ_k v)·WS·(0.5/WS) per head, broadcast-added at the o drain
            sv_sb = persist.tile([P, NH, HD], BF16)
            ones_sb = persist.tile([P, P], BF16)
            nc.vector.memset(ones_sb[:], 1.0)

            # ONE PSUM bank for all projections: two [P, 256] slots in a
            # persistent tile, manually rotated. Region-based dep tracking
            # gives WAR/WAW per slot; each half-group's start=True re-marks
            # the whole bank but PSUM reads return raw data for re-marked
            # bytes (hardware-verified), and no other slot is ever
            # mid-accumulation when a start executes (serial emission).
            proj_ps = ps_pool.tile([P, 2, 256], F32, tag="proj", bufs=1)

            class _ProjView:
                """Adapter exposing the proj bank as tile[:, 0, cols]:
                cols 0:256 -> slot 0, 256:512 -> slot 1, full -> wide."""

                def __getitem__(self, idx):
                    c = idx[2]
                    if c == slice(None):
                        return proj_ps[:].rearrange("p s n -> p (s n)")
                    return proj_ps[:, 0 if c.start == 0 else 1, :]

            def proj_tile():
                return _ProjView()

            # ---------- engine-balance ledger ----------
            # planned busy ns for ACT / DVE; drains charge DVE (or ACT) at
            # emission so the per-wave greedy pick stays globally balanced
            est = {"ACT": 0.0, "DVE": 0.0}

            def charge(eng, n_elems, ov=None):
                est[eng] += n_elems * 0.833 + 171 if eng == "ACT" \
                    else n_elems * 1.042 + 61

            def bal_copy(out, in_, n, scale=None):
                """PSUM->SBUF copy (optionally scaled) on whichever of
                ACT/DVE the ledger says is less loaded."""
                if est["ACT"] + n * 0.833 + 171 <= est["DVE"] + n * 1.042 + 61:
                    charge("ACT", n)
                    nc.scalar.activation(
                        out=out, in_=in_,
                        func=mybir.ActivationFunctionType.Copy,
                        scale=1.0 if scale is None else scale)
                else:
                    charge("DVE", n)
                    if scale is None:
                        nc.vector.tensor_copy(out=out, in_=in_)
                    else:
                        nc.vector.tensor_scalar_mul(out, in_, scale)

            # ---------- producer closures ----------
            def dma_w_hp(w_sb, w_dram, hp):
                def run():
                    nc.sync.dma_start(
                        w_sb[:, hp].rearrange("p kt l n -> p (kt l n)"),
                        w_dram[hp])
                return run

            def dma_wv():
                def run():
                    nc.sync.dma_start(wv_sb[:], wv_r)
                return run

            def dma_x(x_sb, x_r, lo, hi):
                def run():
                    for lv in range(2):
                        nc.sync.dma_start(
                            x_sb[:, :, lv, lo:hi],
                            x_r[:, :, lv, lo:hi])
                return run

            xv_tiles = {}

            def dma_xv(c, lv):
                def run():
                    if lv == 0:
                        xv_tiles[c] = xvpool.tile([P, KTC, 2, 512], FP8,
                                                  tag="xvchunk",
                                                  name=f"xv_{c}")
                    nc.sync.dma_start(
                        xv_tiles[c][:, :, lv, :],
                        xv_r[:, :, lv, c * 512:(c + 1) * 512])
                return run

            # warm-up: the PE runs at 0.65/1.2GHz until ~3us of continuous
            # execution; burn dummy matmuls during the prefix DMAs so the
            # first projections run at full clock
            wu_sb = persist.tile([HD, 2, 640], FP8)

            def warmup(n_mm):
                def run():
                    if n_mm < 0:
                        nc.vector.memset(wu_sb[:], 0)
                        return
                    st = ps_pool.tile([P, 2, 512], F32, tag="st", bufs=3,
                                      name=f"wu_{n_mm}")
                    for m in range(n_mm):
                        nc.tensor.matmul(
                            st[:, m % 2, :],
                            lhsT=wu_sb[:, :, 0:P],
                            rhs=wu_sb[:, :, P:P + 512],
                            start=True, stop=True,
                            perf_mode=mybir.MatmulPerfMode.DoubleRow,
                            skip_group_check=True,
                        )
                return run

            # split-emission prefix projections: the hi terms only need the
            # lv0 (hi) half of the x chunk, so they start ~1.5us earlier
            _prefix_ps = {}

            def _kq_half(x_sb, w_sb, hp, tsl, ps, terms, first):
                n = 0
                for lw, lx in terms:
                    for t in range(NDR):
                        nc.tensor.matmul(
                            ps,
                            lhsT=w_sb[:, hp, 2 * t:2 * t + 2, lw, :],
                            rhs=x_sb[:, 2 * t:2 * t + 2, lx, tsl],
                            start=(first and n == 0), stop=False,
                            perf_mode=mybir.MatmulPerfMode.DoubleRow,
                            skip_group_check=True,
                        )
                        n += 1

            def _kq_drain(dst, hp, tsl, ps, split_lo):
                charge("DVE", 512 if split_lo else 256)
                if split_lo:
                    charge("DVE", 256)
                    nc.vector.tensor_copy(out=dst[:, hp, 0, tsl], in_=ps)
                    nc.vector.tensor_sub(dst[:, hp, 1, tsl], ps,
                                         dst[:, hp, 0, tsl])
                else:
                    nc.vector.tensor_copy(out=dst[:, hp, tsl], in_=ps)

            def proj_kq_hi(x_sb, w_sb, hp, c, key):
                def run():
                    tl = proj_tile()
                    _prefix_ps[key] = tl
                    tsl = slice(c * 512, c * 512 + 256)
                    _kq_half(x_sb, w_sb, hp, tsl, tl[:, 0, 0:256],
                             ((0, 0), (1, 0)), True)
                return run

            def proj_kq_lo(x_sb, w_sb, hp, c, dst, split_lo, key):
                def run():
                    tl = _prefix_ps.pop(key)
                    tsl = slice(c * 512, c * 512 + 256)
                    _kq_half(x_sb, w_sb, hp, tsl, tl[:, 0, 0:256],
                             ((0, 1),), False)
                    _kq_drain(dst, hp, tsl, tl[:, 0, 0:256], split_lo)
                    tsl2 = slice(c * 512 + 256, (c + 1) * 512)
                    _kq_half(x_sb, w_sb, hp, tsl2, tl[:, 0, 256:512],
                             ((0, 0), (1, 0), (0, 1)), True)
                    _kq_drain(dst, hp, tsl2, tl[:, 0, 256:512], split_lo)
                return run

            def dma_x_lv(x_sb, x_r, lo, hi, lv):
                def run():
                    nc.sync.dma_start(
                        x_sb[:, :, lv, lo:hi],
                        x_r[:, :, lv, lo:hi])
                return run

            _kq_state = {}

            def _kq_matmuls(x_sb, w_sb, hp, c, tl, half):
                ps = tl[:, 0, half * 256:(half + 1) * 256]
                tsl = slice(c * 512 + half * 256,
                            c * 512 + (half + 1) * 256)
                n = 0
                for lw, lx in TERMS:
                    for t in range(NDR):
                        nc.tensor.matmul(
                            ps,
                            lhsT=w_sb[:, hp, 2 * t:2 * t + 2, lw, :],
                            rhs=x_sb[:, 2 * t:2 * t + 2, lx, tsl],
                            start=(n == 0), stop=False,
                            perf_mode=mybir.MatmulPerfMode.DoubleRow,
                            skip_group_check=True,
                        )
                        n += 1
                    if AUGK:
                        nc.tensor.matmul(
                            ps,
                            lhsT=w_sb[:, hp, KTC - 1, lw, :],
                            rhs=x_sb[:, KTC - 1, lx, tsl],
                            start=(n == 0), stop=False,
                            skip_group_check=True,
                        )
                        n += 1

            def proj_kq_a(x_sb, w_sb, hp, c, key):
                """first 256-half of a K/Q chunk projection -- emitted two
                jobs before the second half so score fills slip between"""
                def run():
                    tl = proj_tile()
                    _kq_state[key] = tl
                    _kq_matmuls(x_sb, w_sb, hp, c, tl, 0)
                return run

            def proj_kq_b(x_sb, w_sb, hp, c, dst, split_lo, key):
                def run():
                    tl = _kq_state.pop(key)
                    _kq_matmuls(x_sb, w_sb, hp, c, tl, 1)
                    wide = tl[:, 0, :]
                    _kq_finish(wide, dst, hp, c, split_lo)
                return run

            def _kq_finish(wide, dst, hp, c, split_lo):
                if True:
                    sl = slice(c * 512, (c + 1) * 512)
                    if split_lo:
                        charge("DVE", 512)
                        charge("DVE", 512)
                        nc.vector.tensor_copy(out=dst[:, hp, 0, sl], in_=wide)
                        nc.vector.tensor_sub(dst[:, hp, 1, sl], wide,
                                             dst[:, hp, 0, sl])
                    else:
                        charge("DVE", 512)
                        nc.vector.tensor_copy(out=dst[:, hp, sl], in_=wide)

            def proj_kq(x_sb, w_sb, hp, c, dst, split_lo):
                def run():
                    tl = proj_tile()
                    _kq_matmuls(x_sb, w_sb, hp, c, tl, 0)
                    _kq_matmuls(x_sb, w_sb, hp, c, tl, 1)
                    _kq_finish(tl[:, 0, :], dst, hp, c, split_lo)
                return run

            _v_ps = {}

            def proj_v(t, part):
                """v_sb[:, t, :] = x-token-tile t @ Wv ([tok, of]).
                part 'hi' takes the two x-hi terms, 'lo' the x-lo term +
                the PSUM drain (finer interleaving in the in-order PE queue,
                and 'hi' only needs the lv0 half of the xv chunk)."""
                def run():
                    xt = xv_tiles[t // 4]
                    tsl = slice((t % 4) * P, (t % 4 + 1) * P)

                    def v_half(ps, osl, terms, first):
                        n = 0
                        for lw, lx in terms:
                            for u in range(NDR):
                                nc.tensor.matmul(
                                    ps,
                                    lhsT=xt[:, 2 * u:2 * u + 2, lx, tsl],
                                    rhs=wv_sb[:, 2 * u:2 * u + 2, lw, osl],
                                    start=(first and n == 0), stop=False,
                                    perf_mode=mybir.MatmulPerfMode.DoubleRow,
                                    skip_group_check=True,
                                )
                                n += 1
                            if AUGK:
                                nc.tensor.matmul(
                                    ps,
                                    lhsT=xt[:, KTC - 1, lx, tsl],
                                    rhs=wv_sb[:, KTC - 1, lw, osl],
                                    start=(first and n == 0), stop=False,
                                    skip_group_check=True,
                                )
                                n += 1

                    if part == "hi":
                        tl = proj_tile()
                        _v_ps[t] = tl
                        v_half(tl[:, 0, 0:256], slice(0, 256),
                               ((0, 0), (1, 0)), True)
                    else:
                        tl = _v_ps.pop(t)
                        v_half(tl[:, 0, 0:256], slice(0, 256),
                               ((0, 1),), False)
                        v_half(tl[:, 0, 256:512], slice(256, 512),
                               ((0, 0), (1, 0), (0, 1)), True)
                        charge("DVE", 512)
                        nc.vector.tensor_copy(
                            out=v_sb[:, t, :], in_=tl[:, 0, :])
                return run

            # ---------- static schedule ----------
            # rounds in head-pair blocks: (2hp,0),(2hp+1,0),(2hp,1),(2hp+1,1)
            # so the early rounds need only qh chunks 0/1 -- xq c2/c3 (2MB)
            # can ship after the whole xk/xv stream.  Round index
            # r(h,qc) = 4*(h//2) + 2*(qc//2) + h%2; job = r*32 + kt*2 + i.
            # Round 0 staggers its second q-chunk by 2 k-tiles so the first
            # waves only need qh[qc0] (whose projection finishes first).
            rounds = [(2 * hp + (s % 2), s // 2)
                      for hp in range(NHP) for s in range(4)]
            jobs = []
            for kt in range(NKT + 2):
                if kt < NKT:
                    jobs.append((0, 0, kt))
                if kt >= 2:
                    jobs.append((0, 1, kt - 2))
            for h, qcp in rounds[1:]:
                jobs += [(h, 2 * qcp + i, kt)
                         for kt in range(NKT) for i in range(2)]
            waves = [jobs[i0:i0 + 2] for i0 in range(0, len(jobs), 2)]

            producers = []  # (due_job, closure, vtile_or_None)
            # prefix + all input DMAs in deadline order (the DMA engines are
            # effectively serial; emission order = transfer order)
            if AUGK == 0:
                producers += [
                    (-99.9, warmup(-1), None),
                    (-99.8, warmup(10), None),
                    (-99.0, dma_w_hp(wk_sb, wk, 0), None),
                    (-98.9, dma_x_lv(xk_sb, xk_r, 0, 512, 0), None),
                    (-98.8, dma_w_hp(wq_sb, wq, 0), None),
                    (-98.7, proj_kq_hi(xk_sb, wk_sb, 0, 0, "k00"), None),
                    (-98.6, dma_x_lv(xk_sb, xk_r, 0, 512, 1), None),
                    (-98.5, proj_kq_lo(xk_sb, wk_sb, 0, 0, kh, True, "k00"),
                     None),
                    (-98.4, dma_x_lv(xq_sb, xq_r, 0, 512, 0), None),
                    (-98.3, warmup(4), None),
                    (-98.2, proj_kq_hi(xq_sb, wq_sb, 0, 0, "q00"), None),
                    (-98.1, dma_x_lv(xq_sb, xq_r, 0, 512, 1), None),
                    (-98.0, proj_kq_lo(xq_sb, wq_sb, 0, 0, qh, False, "q00"),
                     None),
                    (-97.9, dma_x_lv(xq_sb, xq_r, 512, 1024, 0), None),
                    (-97.8, proj_kq_hi(xq_sb, wq_sb, 0, 1, "q01"), None),
                    (-97.7, dma_x_lv(xq_sb, xq_r, 512, 1024, 1), None),
                    (-97.6, proj_kq_lo(xq_sb, wq_sb, 0, 1, qh, False, "q01"),
                     None),
                ]
            else:
                producers += [
                    (-99.0, dma_w_hp(wk_sb, wk, 0), None),
                    (-98.8, dma_x(xk_sb, xk_r, 0, 512), None),
                    (-98.6, dma_w_hp(wq_sb, wq, 0), None),
                    (-98.4, dma_x(xq_sb, xq_r, 0, 512), None),
                    (-98.2, proj_kq(xk_sb, wk_sb, 0, 0, kh, True), None),
                    (-98.0, proj_kq(xq_sb, wq_sb, 0, 0, qh, False), None),
                    (-97.8, dma_x(xq_sb, xq_r, 512, 1024), None),
                    (-97.6, proj_kq(xq_sb, wq_sb, 0, 1, qh, False), None),
                ]
            # Σv per head: ones-lhsT matmuls into the (idle until hp1) proj
            # bank once all v tiles are in SBUF, then one pre-scaled drain.
            _sv = {}

            def sv_accum():
                def run():
                    _sv["tl"] = proj_tile()
                    wide = _sv["tl"][:, 0, :]
                    n = 0
                    for h in range(NH):
                        for kt in range(NKT):
                            nc.tensor.matmul(
                                wide[:, h * HD:(h + 1) * HD],
                                lhsT=ones_sb[:],
                                rhs=v_sb[:, kt, h * HD:(h + 1) * HD],
                                start=(n == 0), stop=(n == NH * NKT - 1),
                                skip_group_check=True,
                            )
                            n += 1
                return run

            def sv_drain():
                def run():
                    charge("DVE", 512)
                    nc.vector.tensor_scalar_mul(
                        sv_sb[:],
                        _sv.pop("tl")[:, 0, :].rearrange(
                            "p (h2 d) -> p h2 d", d=HD),
                        0.5 / WS)
                    _sv["done"] = True
                return run

            # input stream in DMA-arrival order (the DMA engines are
            # effectively serial; emission order = transfer order); PE
            # producers positioned at their consumer's job index so the
            # in-order PE queue never inverts the arrival order.
            if AUGK == 0:
                producers += [
                    # xk c1 -> kh c1 (kt 4-7, needed from job ~8)
                    (-89.8, dma_x_lv(xk_sb, xk_r, 512, 1024, 0), None),
                    (-89.7, proj_kq_hi(xk_sb, wk_sb, 0, 1, "k01"), None),
                    (-89.6, dma_x_lv(xk_sb, xk_r, 512, 1024, 1), None),
                    (-89.5, proj_kq_lo(xk_sb, wk_sb, 0, 1, kh, True, "k01"),
                     None),
                ]
            else:
                producers.append((-89.8, dma_x(xk_sb, xk_r, 512, 1024), None))
                producers.append((2, proj_kq(xk_sb, wk_sb, 0, 1, kh, True),
                                  None))
            # V stream: wv + xv c0 right behind xk c1
            producers.append((-89.4, dma_wv(), None))
            producers.append((-89.3, dma_xv(0, 0), None))
            producers.append((-89.2, dma_xv(0, 1), None))
            for t in range(4):
                producers.append((12 + 0.1 * t, proj_v(t, "hi"), None))
                producers.append((12.05 + 0.1 * t, proj_v(t, "lo"), t))
            # xk c2 -> kh c2 (kt 8-11, needed from job ~16)
            if AUGK == 0:
                producers += [
                    (13.0, dma_x_lv(xk_sb, xk_r, 1024, 1536, 0), None),
                    (13.1, proj_kq_hi(xk_sb, wk_sb, 0, 2, "k02"), None),
                    (13.2, dma_x_lv(xk_sb, xk_r, 1024, 1536, 1), None),
                    (13.3, proj_kq_lo(xk_sb, wk_sb, 0, 2, kh, True, "k02"),
                     None),
                ]
            else:
                producers.append((13.0, dma_x(xk_sb, xk_r, 1024, 1536), None))
                producers.append((13.3, proj_kq(xk_sb, wk_sb, 0, 2, kh, True),
                                  None))
            producers.append((13.4, dma_xv(1, 0), None))
            producers.append((13.5, dma_xv(1, 1), None))
            for t in range(4, 8):
                producers.append((16 + 0.1 * t, proj_v(t, "hi"), None))
                producers.append((16.05 + 0.1 * t, proj_v(t, "lo"), t))
            # xk c3 -> kh c3 (kt 12-15, needed from job ~24)
            producers.append((18.0, dma_x(xk_sb, xk_r, 1536, 2048), None))
            producers.append((18.1, proj_kq(xk_sb, wk_sb, 0, 3, kh, True),
                              None))
            producers.append((18.2, dma_xv(2, 0), None))
            producers.append((18.3, dma_xv(2, 1), None))
            for t in range(8, 12):
                producers.append((22 + 0.1 * t, proj_v(t, "hi"), None))
                producers.append((22.05 + 0.1 * t, proj_v(t, "lo"), t))
            producers.append((24.0, dma_xv(3, 0), None))
            producers.append((24.1, dma_xv(3, 1), None))
            for t in range(12, NKT):
                producers.append((26 + 0.1 * t, proj_v(t, "hi"), None))
                producers.append((26.05 + 0.1 * t, proj_v(t, "lo"), t))
            producers.append((30.0, sv_accum(), None))
            producers.append((30.1, sv_drain(), None))
            # xq c2/c3 ship last; qh c2/3 for hp0 first used at job 64
            producers.append((32.0, dma_x(xq_sb, xq_r, 1024, 1536), None))
            producers.append((32.5, dma_x(xq_sb, xq_r, 1536, 2048), None))
            producers.append((46, proj_kq_a(xq_sb, wq_sb, 0, 2, "q02"),
                              None))
            producers.append((48, proj_kq_b(xq_sb, wq_sb, 0, 2, qh, False,
                                            "q02"), None))
            producers.append((50, proj_kq_a(xq_sb, wq_sb, 0, 3, "q03"),
                              None))
            producers.append((52, proj_kq_b(xq_sb, wq_sb, 0, 3, qh, False,
                                            "q03"), None))
            # hp1-3: W DMAs + K/Q projections just-in-time for their blocks
            for hp in range(1, NHP):
                base = 128 * hp
                producers.append((base - 40, dma_w_hp(wk_sb, wk, hp), None))
                producers.append((base - 38, dma_w_hp(wq_sb, wq, hp), None))
                # A/B pairs of one projection share the single proj
                # PSUM bank -- their deadlines must never interleave with
                # another projection's pair (fractional offsets keep each
                # pair adjacent in the producer stream)
                for c in range(TOKC):
                    producers.append((base + 8 * c - 16.0,
                                      proj_kq_a(xk_sb, wk_sb, hp, c,
                                                f"k{hp}{c}"), None))
                    producers.append((base + 8 * c - 15.9,
                                      proj_kq_b(xk_sb, wk_sb, hp, c, kh, True,
                                                f"k{hp}{c}"), None))
                producers.append((base - 13.8,
                                  proj_kq_a(xq_sb, wq_sb, hp, 0, f"q{hp}0"),
                                  None))
                producers.append((base - 13.7,
                                  proj_kq_b(xq_sb, wq_sb, hp, 0, qh, False,
                                            f"q{hp}0"), None))
                producers.append((base - 11.8,
                                  proj_kq_a(xq_sb, wq_sb, hp, 1, f"q{hp}1"),
                                  None))
                producers.append((base - 11.7,
                                  proj_kq_b(xq_sb, wq_sb, hp, 1, qh, False,
                                            f"q{hp}1"), None))
                producers.append((base + 48.0,
                                  proj_kq_a(xq_sb, wq_sb, hp, 2, f"q{hp}2"),
                                  None))
                producers.append((base + 48.1,
                                  proj_kq_b(xq_sb, wq_sb, hp, 2, qh, False,
                                            f"q{hp}2"), None))
                producers.append((base + 52.0,
                                  proj_kq_a(xq_sb, wq_sb, hp, 3, f"q{hp}3"),
                                  None))
                producers.append((base + 52.1,
                                  proj_kq_b(xq_sb, wq_sb, hp, 3, qh, False,
                                            f"q{hp}3"), None))
            producers.sort(key=lambda e: e[0])
            producers = deque(producers)
            v_emit_wave = {}

            # AV bookkeeping
            av_fifo = deque()  # (job_idx, h, qc, kt, a_t, j_in_wave, wave)
            av_state = {"tile": None, "round": -1, "cool": -1}

            def finalize_round(r):
                av = av_state["tile"]
                hp, s = divmod(r, 4)
                h, qcp = 2 * hp + (s % 2), s // 2
                o_sb = opool.tile([P, 2, QC, HD], F32, tag="o_sb",
                                  name=f"osb_{r}")
                # o = (Σ attn'·v + Σv)·(1/(2·WS)) with attn' = tanh(s/2):
                # the sv term supplies the +Σv, pre-scaled by 0.5/WS
                charge("DVE", 512)
                nc.vector.affine_then_add(
                    out=o_sb[:].rearrange("p i qt d -> p (i qt) d"),
                    in0=av[:].rearrange("p (g d) -> p g d", d=HD),
                    in1=sv_sb[:, h].unsqueeze(1).broadcast_to([P, 2 * QC, HD]),
                    scale=0.5 / WS,
                    bias=0.0,
                )
                for i in range(2):
                    qc = 2 * qcp + i
                    dst = o[qc * 512:(qc + 1) * 512,
                            h * HD:(h + 1) * HD].rearrange(
                                "(qt p) d -> p qt d", p=P)
                    nc.sync.dma_start(dst, o_sb[:, i])
                av_state["tile"] = None

            def drain_avs(cur_wave, final=False):
                budget = 8  # cap per-wave AV emission so a backlog burst
                # never parks in front of the score stream in the in-order
                # PE queue
                continue_outer = False
                while av_fifo and not continue_outer:
                    job, h, qc, kt, a_t, j, w = av_fifo[0]
                    if not final:
                        if budget <= 0:
                            break
                        if w >= cur_wave:
                            break
                        vw = v_emit_wave.get(kt)
                        if vw is None or vw >= cur_wave:
                            break
                        budget -= 1
                    r = job // RJOBS
                    if r != av_state["round"]:
                        if not final and not _sv.get("done"):
                            break
                        if av_state["tile"] is not None:
                            finalize_round(av_state["round"])
                            # cool-down: keep the next round's AVs out of the
                            # in-order PE queue until the o-drain has had two
                            # waves to clear the av bank (they would WAR-block
                            # every score fill emitted behind them)
                            av_state["cool"] = cur_wave + 4
                        if not final and cur_wave < av_state["cool"]:
                            continue_outer = True
                            break
                        av_state["tile"] = ps_pool.tile(
                            [P, 512], F32, tag="av", bufs=1, name=f"av_{r}")
                        av_state["round"] = r
                    av_fifo.popleft()
                    av = av_state["tile"]
                    i = qc % 2
                    first = (kt == 0 and i == 0)
                    last = (kt == NKT - 1 and i == 1)
                    for qt in range(4):
                        nc.tensor.matmul(
                            av[:, (i * 4 + qt) * HD:(i * 4 + qt + 1) * HD],
                            lhsT=a_t[:, j, qt * P:(qt + 1) * P],
                            rhs=v_sb[:, kt, h * HD:(h + 1) * HD],
                            start=(first and qt == 0),
                            stop=(last and qt == 3),
                            skip_group_check=True,
                        )

            # ---------- main wave loop ----------
            def drain_producers(w, job_base):
                while producers and producers[0][0] <= job_base + 2:
                    due, closure, vtile = producers.popleft()
                    closure()
                    if vtile is not None:
                        v_emit_wave[vtile] = w

            job_base = 0
            for w, wave in enumerate(waves):
                drain_producers(w, job_base)
                g = len(wave)
                st = ps_pool.tile([P, 2, 512], F32, tag="st", bufs=3,
                                  name=f"st_{w}")
                for j, (h, qc, kt) in enumerate(wave):
                    hp, pb = h // 2, (h % 2) * HD
                    lhsT = kh[pb:pb + HD, hp, :, kt * P:(kt + 1) * P]
                    # two 256-col halves: the moving AP is [HD, 2, 256] = 512
                    # elements, the PE's MAX_MOVING_FREE_DIM_SIZE
                    for half in range(2):
                        rhs = qh[pb:pb + HD, hp,
                                 qc * 512 + half * 256:
                                 qc * 512 + (half + 1) * 256]
                        rhs = rhs.unsqueeze(1).broadcast_to([HD, 2, 256])
                        nc.tensor.matmul(
                            st[:, j, half * 256:(half + 1) * 256],
                            lhsT=lhsT,
                            rhs=rhs,
                            start=True,
                            stop=True,
                            perf_mode=mybir.MatmulPerfMode.DoubleRow,
                            tile_position=(pb, 0),
                            skip_group_check=True,
                        )
                a_t = apool.tile([P, 2, 512], BF16, tag="a_t", name=f"a_{w}")
                n_el = g * 512
                if est["DVE"] + n_el * 1.042 + 61 < est["ACT"] + n_el * 0.833 + 171:
                    # DVE share: clamped odd-quintic ~= tanh(s_true/2)
                    charge("DVE", n_el)
                    nc.vector._custom_dve(
                        TANH_OP,
                        out=a_t[:, :g, :],
                        in0=st[:, :g, :],
                        s0=K_FIT * 0.125 / (WS * WS),
                        s1=C1_FIT,
                        imm2=C2_FIT,
                    )
                else:
                    charge("ACT", n_el)
                    nc.scalar.activation(
                        out=a_t[:, :g, :],
                        in_=st[:, :g, :],
                        func=mybir.ActivationFunctionType.Tanh,
                        scale=0.0625 / (WS * WS),
                    )
                for j, (h, qc, kt) in enumerate(wave):
                    r = 4 * (h // 2) + 2 * (qc // 2) + (h % 2)
                    av_fifo.append((r * RJOBS + kt * 2 + (qc % 2),
                                    h, qc, kt, a_t, j, w))
                drain_avs(w)
                job_base += g
            import os
            if os.environ.get("KDBG"):
                print("EST at end:", est)
            while producers:
                producers.popleft()[1]()
            drain_avs(0, final=True)
            finalize_round(av_state["round"])

    nc.compile()
    return nc


def _prep_core_inputs(q, k, v, Wq, bq, Wk, bk, Wv, bv, KTC):
    """Host-side shard + transpose + split-fp8 packing. in_maps for 8 cores."""
    import ml_dtypes
    E4 = ml_dtypes.float8_e4m3
    KA = KTC * P
    aug = KA > D

    def split8(a):
        """[R, C] fp32 -> [R, 2, C] fp8 (hi, lo)."""
        hi = a.astype(E4)
        lo = (a - hi.astype(np.float32)).astype(E4)
        return np.ascontiguousarray(np.stack([hi, lo], axis=1))

    def x_t(x_b):  # [S, D] -> [KA, 2, S] fp8
        xt = np.ascontiguousarray(x_b.T)
        if aug:
            pad = np.zeros((KA, S), np.float32)
            pad[:D] = xt
            pad[D] = 1.0
            xt = pad
        return split8(xt)

    def w_kq(W, b, half):  # -> [NHP, P, KTC*2*128] fp8, p-major
        ws = np.ascontiguousarray(W[:, half * OF:(half + 1) * OF]) * WS
        if aug:
            pad = np.zeros((KA, OF), np.float32)
            pad[:D] = ws
            pad[D] = b[half * OF:(half + 1) * OF] * WS
            ws = pad
        s8 = split8(ws)  # [KA, 2, OF]
        pm = s8.reshape(KTC, P, 2, NHP, P).transpose(3, 1, 0, 2, 4)
        return np.ascontiguousarray(pm.reshape(NHP, P, KTC * 2 * P))

    def w_v(W, b, half):  # -> [KA, 2, OF] fp8
        ws = np.ascontiguousarray(W[:, half * OF:(half + 1) * OF]) * WS
        if aug:
            pad = np.zeros((KA, OF), np.float32)
            pad[:D] = ws
            pad[D] = b[half * OF:(half + 1) * OF] * WS
            ws = pad
        return split8(ws)

    xts = {}
    in_maps = []
    for c in range(N_CORES):
        b, half = divmod(c, 2)
        if b not in xts:
            xts[b] = (x_t(q[b]), x_t(k[b]), x_t(v[b]))
        xq_c, xk_c, xv_c = xts[b]
        in_maps.append({
            "xq": xq_c,
            "xk": xk_c,
            "xv": xv_c,
            "wq": w_kq(Wq, bq, half),
            "wk": w_kq(Wk, bk, half),
            "wv": w_v(Wv, bv, half),
        })
    return in_maps


def kernel(q, k, v, Wq, bq, Wk, bk, Wv, bv):
    global last_results
    q = np.ascontiguousarray(np.asarray(q, np.float32))
    k = np.ascontiguousarray(np.asarray(k, np.float32))
    v = np.ascontiguousarray(np.asarray(v, np.float32))
    Wq = np.asarray(Wq, np.float32)
    Wk = np.asarray(Wk, np.float32)
    Wv = np.asarray(Wv, np.float32)
    bq = np.asarray(bq, np.float32)
    bk = np.asarray(bk, np.float32)
    bv = np.asarray(bv, np.float32)

    aug = any(np.any(b_) for b_ in (bq, bk, bv))
    KTC = (D // P) + (1 if aug else 0)

    if KTC not in _cache:
        _cache[KTC] = _build(KTC)
    nc = _cache[KTC]

    in_maps = _prep_core_inputs(q, k, v, Wq, bq, Wk, bk, Wv, bv, KTC)
    res = run_bass_kernel_spmd(nc, in_maps, core_ids=list(range(N_CORES)))
    last_results = res

    out = np.empty((B, S, D), np.float32)
    for c in range(N_CORES):
        b, half = divmod(c, 2)
        out[b, :, half * OF:(half + 1) * OF] = res.results[c]["o"]
    return out



# revision 62
# speedup vs baseline: 1.2001x; 1.0216x over previous
"""Trainium2 Bass kernel for nn_MultiHeadAttention_69106023793143.

Reference computation (B=4, S=2048, D=1024, H=16, HD=64):
    qh = split_heads(q @ Wq + bq); kh, vh likewise
    out = merge_heads(sigmoid((qh @ kh^T) / sqrt(HD)) @ vh)

Sharding (8 cores): core c handles batch b = c//2 and the half = c%2 slice of
the feature axis (512 features = 8 heads).  Projections are tensor-parallel on
the output dim of Wq/Wk/Wv; attention is head-parallel.

Device strategy per core (three-way balanced pipeline):
  - The elementwise nonlinearity over 8*2048*2048 scores is split across
    BOTH elementwise engines via the identity sigma(s) = (1+tanh(s/2))/2:
    out = (SUM tanh(s_k/2) v_k + SUM v_k) / 2.  ACT computes exact tanh
    (1 elem/cyc/partition @1.2GHz); the DVE computes a clamped odd-quintic
    approximation of tanh as ONE fused custom-DVE op (TANH_PC5_ANT,
    registered at import: p(u)=u*(C1+u^2*(C2+u^2)), u=clamp(C0*s,+-1),
    8 ALU stages, N(0,1)-weighted RMS 3.5e-3).  A greedy ledger (est/charge)
    assigns each score wave to whichever engine has less planned busy time,
    counting the PSUM->SBUF drains that only the DVE can do.  This turns the
    218us single-engine ACT floor into ~155us across two engines, leaving
    the PE (~178us busy) as the pacing engine.
  - The +SUM v_k term: ones-lhsT matmuls accumulate SUM v per head once into
    the briefly-idle proj bank (sv_sb, prescaled 0.5/WS); each round's o
    drain is AFFINE_THEN_ADD (one DVE op: o = av*(0.5/WS) + sv broadcast).
  - Projections run as split-fp8 DoubleRow matmuls: host ships x and W as
    (hi, lo) fp8e4 pairs and the product takes the three cross terms
    xh*Wh + xh*Wl + xl*Wh - ~bf16 accuracy at 0.75x the bf16 PE cost.
    W is host-scaled by WS=16 to keep fp8 W normal; scales fold into the
    tanh arg and the o drain.
  - Scores use fp8e4 DoubleRow matmuls at 0.5 cyc/row: kh stored as an
    (hi, lo) fp8 pair on the two DoubleRow K-blocks (k-side compensated),
    qh plain fp8 broadcast (stride-0).  Odd heads at tile_position (64,0).
    Each score matmul moves [HD,2,256]=512 elems = MAX_MOVING_FREE_DIM_SIZE
    (a single 512-col matmul moves 1024 and SILENTLY mis-executes on HW).
  - Rounds are ordered in head-pair blocks (2hp,0),(2hp+1,0),(2hp,1),
    (2hp+1,1) so early rounds need only qh chunks 0/1: the DMA stream is
    wk,xk0,wq,xq0,xq1,xk1,wv,xv0,xk2,xv1,xk3,xv2,xv3,xq2,xq3 - kh chunks
    arrive just-in-time for round 0's kt sweep and V-projections/AV start
    ~15us in, while xq c2/c3 (2MB) ship last.  K/Q projections for hp1-3
    are emitted just-in-time before their blocks as adjacent A/B half-pairs
    (pairs must NEVER interleave: they share the single proj PSUM bank).
  - AV in bf16, out[q,d]: lhsT = attn^T tile [128k,128q], rhs = v[128k,64].
    Each round accumulates 8 q-tiles interleaved in ONE PSUM av bank
    (start=True on first, stop=True on last).  AVs drain from a FIFO gated
    on their V-tile's emission, capped at 8/wave, with a 4-wave cool-down
    after each round boundary so the next round's AVs never WAR-block the
    in-order PE queue on the o drain.
  - PSUM: 3 x 2-bank score-wave tiles (2-job waves) + 1 proj bank + 1 av
    bank = 8.  SBUF: 28 attn wave buffers.
  - Warm-up matmuls ramp the PE p-state (0.65->2.4GHz) during the prefix.
  - Nonzero biases fold in via a host-side augmented ones-row (KTC=9).

End-to-end: max rel err ~1.36e-2 (budget 2e-2); TimelineSim 247797ns
(baseline 297223ns).
"""

import sys

if "/opt/trn_rl_repo" not in sys.path:
    sys.path.insert(0, "/opt/trn_rl_repo")

from collections import deque
from contextlib import ExitStack

import numpy as np

import concourse.tile as tile
from concourse import bacc, mybir
from concourse import dve_ops as _dve_ops
from concourse.bass_utils import run_bass_kernel_spmd
from concourse.dve_spec import C0, C1, C2, One, Spec, Src0, Zero, lower, maxx, minn, sq
from concourse.dve_uop import DveOpSpec

# ---- custom DVE op: clamped odd-quintic tanh approximation -----------------
# p(u) = u*(C1 + u^2*(C2 + u^2)), u = clamp(Src0*C0, -1, 1)  [8 ALU stages]
# Approximates tanh(k_fit * s / C0_rel ...): with C0 = K_FIT*raw_scale it
# computes tanh(s_true/2) to 3.5e-3 weighted RMS over s_true ~ N(0,1)
# (max err 0.034 at the |s|~4 clamp shoulder).  The quintic coefficient is
# slaved to 1 in u-units, which keeps the expression inside the DVE's
# 8-stage budget with only 3 scalar slots.
_TANH_NAME = "TANH_PC5_ANT"
K_FIT = 0.25283828
C1_FIT = 1.94641582
C2_FIT = -1.95047264


def _tanh_pc5_ref(in0, in1, s0, s1, imm2):
    u = np.clip(np.asarray(in0, np.float32) * s0, -1.0, 1.0)
    u2 = u * u
    return u * (s1 + u2 * (imm2 + u2))


def _register_tanh_op():
    for op in _dve_ops.OPS:
        if op.name == _TANH_NAME:
            return op
    t = Src0 * C0
    u = maxx(minn(t, One), Zero - One)
    u2 = sq(u)
    spec = Spec(body=u * (C1 + u2 * (C2 + u2)), reference=_tanh_pc5_ref)
    shas = {
        ver: DveOpSpec(name=_TANH_NAME, uops=lower(spec, ver=ver)).sha(ver)
        for ver in ("v3", "v4")
    }
    op = _dve_ops.DveOp(_TANH_NAME, spec, subdim=False, uops_sha=shas)
    _dve_ops.OPS.append(op)
    _dve_ops.CUSTOM_DVE_SPECS[op.name] = spec
    _dve_ops._SUB_OPCODE_FOR_NAME[op.name] = (
        _dve_ops._CUSTOM_DVE_ROW_BASE + len(_dve_ops.OPS) - 1
    )
    return op


TANH_OP = _register_tanh_op()

B, S, D, H = 4, 2048, 1024, 16
HD = D // H  # 64
OF = D // 2  # 512 features (8 heads) per core
N_CORES = 8
P = 128
NH = 8          # heads per core
NHP = 4         # head pairs per core
QC = 4          # q-chunks of 512
NQCP = 2        # q-chunk pairs
NKT = 16        # k token tiles of 128
TOKC = 4        # x token chunks of 512
RJOBS = 2 * NKT  # jobs per round (2 q-chunks x 16 kt)
ABUFS = 28      # attn (a_t) wave buffers
WS = 16.0       # host-side W scale (keeps fp8 W out of subnormals)

F32 = mybir.dt.float32
BF16 = mybir.dt.bfloat16
FP8 = mybir.dt.float8e4

# the three split-fp8 cross terms (w level, x level)
TERMS = ((0, 0), (0, 1), (1, 0))

_cache: dict = {}
last_results = None


def _build(KTC: int):
    """KTC = contraction k-tiles for the projections (8, or 9 when biases are
    folded in via an augmented ones-row)."""
    nc = bacc.Bacc("TRN2", target_bir_lowering=False, debug=False,
                   num_devices=N_CORES, name="mha_sig4")
    KA = KTC * P
    NDR = KTC // 2   # DoubleRow kt-pairs per term
    AUGK = KTC % 2   # leftover kt (the ones-row) as plain fp8 matmul
    WFREE = KTC * 2 * P  # per-partition elements of one head-pair W slice

    xq = nc.dram_tensor("xq", [KA, 2, S], FP8, kind="ExternalInput")
    xk = nc.dram_tensor("xk", [KA, 2, S], FP8, kind="ExternalInput")
    xv = nc.dram_tensor("xv", [KA, 2, S], FP8, kind="ExternalInput")
    # wq/wk p-major: [head-pair, partition, kt*level*128]
    wq = nc.dram_tensor("wq", [NHP, P, WFREE], FP8, kind="ExternalInput")
    wk = nc.dram_tensor("wk", [NHP, P, WFREE], FP8, kind="ExternalInput")
    wv = nc.dram_tensor("wv", [KA, 2, OF], FP8, kind="ExternalInput")
    o = nc.dram_tensor("o", [S, OF], F32, kind="ExternalOutput")

    xq_r = xq.rearrange("(kt p) l t -> p kt l t", p=P)
    xk_r = xk.rearrange("(kt p) l t -> p kt l t", p=P)
    xv_r = xv.rearrange("(kt p) l t -> p kt l t", p=P)
    wv_r = wv.rearrange("(kt p) l n -> p kt l n", p=P)

    abufs = ABUFS if KTC == 8 else 10

    with tile.TileContext(nc) as tc:
        with ExitStack() as ctx:
            persist = ctx.enter_context(tc.tile_pool(name="persist", bufs=1))
            xvpool = ctx.enter_context(tc.tile_pool(name="xvpool", bufs=2))
            apool = ctx.enter_context(tc.tile_pool(name="apool", bufs=abufs))
            opool = ctx.enter_context(tc.tile_pool(name="opool", bufs=2))
            ps_pool = ctx.enter_context(
                tc.tile_pool(name="ps_pool", bufs=2, space="PSUM"))

            wk_sb = persist.tile([P, NHP, KTC, 2, P], FP8)
            wq_sb = persist.tile([P, NHP, KTC, 2, P], FP8)
            wv_sb = persist.tile([P, KTC, 2, OF], FP8)
            xk_sb = persist.tile([P, KTC, 2, S], FP8)
            xq_sb = persist.tile([P, KTC, 2, S], FP8)
            # kh as (hi, lo) fp8 pair, head pairs stacked on partitions;
            # qh plain fp8; v bf16 [tok, of]
            kh = persist.tile([P, NHP, 2, S], FP8)
            qh = persist.tile([P, NHP, S], FP8)
            v_sb = persist.tile([P, NKT, OF], BF16)
            # (Σ_k v)·WS·(0.5/WS) per head, broadcast-added at the o drain
            sv_sb = persist.tile([P, NH, HD], BF16)
            ones_sb = persist.tile([P, P], BF16)
            nc.vector.memset(ones_sb[:], 1.0)

            # ONE PSUM bank for all projections: two [P, 256] slots in a
            # persistent tile, manually rotated. Region-based dep tracking
            # gives WAR/WAW per slot; each half-group's start=True re-marks
            # the whole bank but PSUM reads return raw data for re-marked
            # bytes (hardware-verified), and no other slot is ever
            # mid-accumulation when a start executes (serial emission).
            proj_ps = ps_pool.tile([P, 2, 256], F32, tag="proj", bufs=1)

            class _ProjView:
                """Adapter exposing the proj bank as tile[:, 0, cols]:
                cols 0:256 -> slot 0, 256:512 -> slot 1, full -> wide."""

                def __getitem__(self, idx):
                    c = idx[2]
                    if c == slice(None):
                        return proj_ps[:].rearrange("p s n -> p (s n)")
                    return proj_ps[:, 0 if c.start == 0 else 1, :]

            def proj_tile():
                return _ProjView()

            # ---------- engine-balance ledger ----------
            # planned busy ns for ACT / DVE; drains charge DVE (or ACT) at
            # emission so the per-wave greedy pick stays globally balanced
            est = {"ACT": 0.0, "DVE": 0.0}

            def charge(eng, n_elems, ov=None):
                est[eng] += n_elems * 0.833 + 171 if eng == "ACT" \
                    else n_elems * 1.042 + 61

            def bal_copy(out, in_, n, scale=None):
                """PSUM->SBUF copy (optionally scaled) on whichever of
                ACT/DVE the ledger says is less loaded."""
                if est["ACT"] + n * 0.833 + 171 <= est["DVE"] + n * 1.042 + 61:
                    charge("ACT", n)
                    nc.scalar.activation(
                        out=out, in_=in_,
                        func=mybir.ActivationFunctionType.Copy,
                        scale=1.0 if scale is None else scale)
                else:
                    charge("DVE", n)
                    if scale is None:
                        nc.vector.tensor_copy(out=out, in_=in_)
                    else:
                        nc.vector.tensor_scalar_mul(out, in_, scale)

            # ---------- producer closures ----------
            def dma_w_hp(w_sb, w_dram, hp):
                def run():
                    nc.sync.dma_start(
                        w_sb[:, hp].rearrange("p kt l n -> p (kt l n)"),
                        w_dram[hp])
                return run

            def dma_wv():
                def run():
                    nc.sync.dma_start(wv_sb[:], wv_r)
                return run

            def dma_x(x_sb, x_r, lo, hi):
                def run():
                    for lv in range(2):
                        nc.sync.dma_start(
                            x_sb[:, :, lv, lo:hi],
                            x_r[:, :, lv, lo:hi])
                return run

            xv_tiles = {}

            def dma_xv(c, lv):
                def run():
                    if lv == 0:
                        xv_tiles[c] = xvpool.tile([P, KTC, 2, 512], FP8,
                                                  tag="xvchunk",
                                                  name=f"xv_{c}")
                    nc.sync.dma_start(
                        xv_tiles[c][:, :, lv, :],
                        xv_r[:, :, lv, c * 512:(c + 1) * 512])
                return run

            # warm-up: the PE runs at 0.65/1.2GHz until ~3us of continuous
            # execution; burn dummy matmuls during the prefix DMAs so the
            # first projections run at full clock
            wu_sb = persist.tile([HD, 2, 640], FP8)

            def warmup(n_mm):
                def run():
                    if n_mm < 0:
                        nc.vector.memset(wu_sb[:], 0)
                        return
                    st = ps_pool.tile([P, 2, 512], F32, tag="st", bufs=3,
                                      name=f"wu_{n_mm}")
                    for m in range(n_mm):
                        nc.tensor.matmul(
                            st[:, m % 2, :],
                            lhsT=wu_sb[:, :, 0:P],
                            rhs=wu_sb[:, :, P:P + 512],
                            start=True, stop=True,
                            perf_mode=mybir.MatmulPerfMode.DoubleRow,
                            skip_group_check=True,
                        )
                return run

            # split-emission prefix projections: the hi terms only need the
            # lv0 (hi) half of the x chunk, so they start ~1.5us earlier
            _prefix_ps = {}

            def _kq_half(x_sb, w_sb, hp, tsl, ps, terms, first):
                n = 0
                for lw, lx in terms:
                    for t in range(NDR):
                        nc.tensor.matmul(
                            ps,
                            lhsT=w_sb[:, hp, 2 * t:2 * t + 2, lw, :],
                            rhs=x_sb[:, 2 * t:2 * t + 2, lx, tsl],
                            start=(first and n == 0), stop=False,
                            perf_mode=mybir.MatmulPerfMode.DoubleRow,
                            skip_group_check=True,
                        )
                        n += 1

            def _kq_drain(dst, hp, tsl, ps, split_lo):
                charge("DVE", 512 if split_lo else 256)
                if split_lo:
                    charge("DVE", 256)
                    nc.vector.tensor_copy(out=dst[:, hp, 0, tsl], in_=ps)
                    nc.vector.tensor_sub(dst[:, hp, 1, tsl], ps,
                                         dst[:, hp, 0, tsl])
                else:
                    nc.vector.tensor_copy(out=dst[:, hp, tsl], in_=ps)

            def proj_kq_hi(x_sb, w_sb, hp, c, key):
                def run():
                    tl = proj_tile()
                    _prefix_ps[key] = tl
                    tsl = slice(c * 512, c * 512 + 256)
                    _kq_half(x_sb, w_sb, hp, tsl, tl[:, 0, 0:256],
                             ((0, 0), (1, 0)), True)
                return run

            def proj_kq_lo(x_sb, w_sb, hp, c, dst, split_lo, key):
                def run():
                    tl = _prefix_ps.pop(key)
                    tsl = slice(c * 512, c * 512 + 256)
                    _kq_half(x_sb, w_sb, hp, tsl, tl[:, 0, 0:256],
                             ((0, 1),), False)
                    _kq_drain(dst, hp, tsl, tl[:, 0, 0:256], split_lo)
                    tsl2 = slice(c * 512 + 256, (c + 1) * 512)
                    _kq_half(x_sb, w_sb, hp, tsl2, tl[:, 0, 256:512],
                             ((0, 0), (1, 0), (0, 1)), True)
                    _kq_drain(dst, hp, tsl2, tl[:, 0, 256:512], split_lo)
                return run

            def dma_x_lv(x_sb, x_r, lo, hi, lv):
                def run():
                    nc.sync.dma_start(
                        x_sb[:, :, lv, lo:hi],
                        x_r[:, :, lv, lo:hi])
                return run

            _kq_state = {}

            def _kq_matmuls(x_sb, w_sb, hp, c, tl, half):
                ps = tl[:, 0, half * 256:(half + 1) * 256]
                tsl = slice(c * 512 + half * 256,
                            c * 512 + (half + 1) * 256)
                n = 0
                for lw, lx in TERMS:
                    for t in range(NDR):
                        nc.tensor.matmul(
                            ps,
                            lhsT=w_sb[:, hp, 2 * t:2 * t + 2, lw, :],
                            rhs=x_sb[:, 2 * t:2 * t + 2, lx, tsl],
                            start=(n == 0), stop=False,
                            perf_mode=mybir.MatmulPerfMode.DoubleRow,
                            skip_group_check=True,
                        )
                        n += 1
                    if AUGK:
                        nc.tensor.matmul(
                            ps,
                            lhsT=w_sb[:, hp, KTC - 1, lw, :],
                            rhs=x_sb[:, KTC - 1, lx, tsl],
                            start=(n == 0), stop=False,
                            skip_group_check=True,
                        )
                        n += 1

            def proj_kq_a(x_sb, w_sb, hp, c, key):
                """first 256-half of a K/Q chunk projection -- emitted two
                jobs before the second half so score fills slip between"""
                def run():
                    tl = proj_tile()
                    _kq_state[key] = tl
                    _kq_matmuls(x_sb, w_sb, hp, c, tl, 0)
                return run

            def proj_kq_b(x_sb, w_sb, hp, c, dst, split_lo, key):
                def run():
                    tl = _kq_state.pop(key)
                    _kq_matmuls(x_sb, w_sb, hp, c, tl, 1)
                    wide = tl[:, 0, :]
                    _kq_finish(wide, dst, hp, c, split_lo)
                return run

            def _kq_finish(wide, dst, hp, c, split_lo):
                if True:
                    sl = slice(c * 512, (c + 1) * 512)
                    if split_lo:
                        charge("DVE", 512)
                        charge("DVE", 512)
                        nc.vector.tensor_copy(out=dst[:, hp, 0, sl], in_=wide)
                        nc.vector.tensor_sub(dst[:, hp, 1, sl], wide,
                                             dst[:, hp, 0, sl])
                    else:
                        charge("DVE", 512)
                        nc.vector.tensor_copy(out=dst[:, hp, sl], in_=wide)

            def proj_kq(x_sb, w_sb, hp, c, dst, split_lo):
                def run():
                    tl = proj_tile()
                    _kq_matmuls(x_sb, w_sb, hp, c, tl, 0)
                    _kq_matmuls(x_sb, w_sb, hp, c, tl, 1)
                    _kq_finish(tl[:, 0, :], dst, hp, c, split_lo)
                return run

            _v_ps = {}

            def proj_v(t, part):
                """v_sb[:, t, :] = x-token-tile t @ Wv ([tok, of]).
                part 'hi' takes the two x-hi terms, 'lo' the x-lo term +
                the PSUM drain (finer interleaving in the in-order PE queue,
                and 'hi' only needs the lv0 half of the xv chunk)."""
                def run():
                    xt = xv_tiles[t // 4]
                    tsl = slice((t % 4) * P, (t % 4 + 1) * P)

                    def v_half(ps, osl, terms, first):
                        n = 0
                        for lw, lx in terms:
                            for u in range(NDR):
                                nc.tensor.matmul(
                                    ps,
                                    lhsT=xt[:, 2 * u:2 * u + 2, lx, tsl],
                                    rhs=wv_sb[:, 2 * u:2 * u + 2, lw, osl],
                                    start=(first and n == 0), stop=False,
                                    perf_mode=mybir.MatmulPerfMode.DoubleRow,
                                    skip_group_check=True,
                                )
                                n += 1
                            if AUGK:
                                nc.tensor.matmul(
                                    ps,
                                    lhsT=xt[:, KTC - 1, lx, tsl],
                                    rhs=wv_sb[:, KTC - 1, lw, osl],
                                    start=(first and n == 0), stop=False,
                                    skip_group_check=True,
                                )
                                n += 1

                    if part == "hi":
                        tl = proj_tile()
                        _v_ps[t] = tl
                        v_half(tl[:, 0, 0:256], slice(0, 256),
                               ((0, 0), (1, 0)), True)
                    else:
                        tl = _v_ps.pop(t)
                        v_half(tl[:, 0, 0:256], slice(0, 256),
                               ((0, 1),), False)
                        v_half(tl[:, 0, 256:512], slice(256, 512),
                               ((0, 0), (1, 0), (0, 1)), True)
                        charge("DVE", 512)
                        nc.vector.tensor_copy(
                            out=v_sb[:, t, :], in_=tl[:, 0, :])
                return run

            # ---------- static schedule ----------
            # rounds in head-pair blocks: (2hp,0),(2hp+1,0),(2hp,1),(2hp+1,1)
            # so the early rounds need only qh chunks 0/1 -- xq c2/c3 (2MB)
            # can ship after the whole xk/xv stream.  Round index
            # r(h,qc) = 4*(h//2) + 2*(qc//2) + h%2; job = r*32 + kt*2 + i.
            # Round 0 staggers its second q-chunk by 2 k-tiles so the first
            # waves only need qh[qc0] (whose projection finishes first).
            rounds = [(2 * hp + (s % 2), s // 2)
                      for hp in range(NHP) for s in range(4)]
            jobs = []
            for kt in range(NKT + 2):
                if kt < NKT:
                    jobs.append((0, 0, kt))
                if kt >= 2:
                    jobs.append((0, 1, kt - 2))
            for h, qcp in rounds[1:]:
                jobs += [(h, 2 * qcp + i, kt)
                         for kt in range(NKT) for i in range(2)]
            waves = [jobs[i0:i0 + 2] for i0 in range(0, len(jobs), 2)]

            producers = []  # (due_job, closure, vtile_or_None)
            # prefix + all input DMAs in deadline order (the DMA engines are
            # effectively serial; emission order = transfer order)
            if AUGK == 0:
                producers += [
                    (-99.9, warmup(-1), None),
                    (-99.8, warmup(10), None),
                    (-99.0, dma_w_hp(wk_sb, wk, 0), None),
                    (-98.9, dma_x_lv(xk_sb, xk_r, 0, 512, 0), None),
                    (-98.8, dma_w_hp(wq_sb, wq, 0), None),
                    (-98.7, proj_kq_hi(xk_sb, wk_sb, 0, 0, "k00"), None),
                    (-98.6, dma_x_lv(xk_sb, xk_r, 0, 512, 1), None),
                    (-98.5, proj_kq_lo(xk_sb, wk_sb, 0, 0, kh, True, "k00"),
                     None),
                    (-98.4, dma_x_lv(xq_sb, xq_r, 0, 512, 0), None),
                    (-98.3, warmup(4), None),
                    (-98.2, proj_kq_hi(xq_sb, wq_sb, 0, 0, "q00"), None),
                    (-98.1, dma_x_lv(xq_sb, xq_r, 0, 512, 1), None),
                    (-98.0, proj_kq_lo(xq_sb, wq_sb, 0, 0, qh, False, "q00"),
                     None),
                    (-97.9, dma_x_lv(xq_sb, xq_r, 512, 1024, 0), None),
                    (-97.8, proj_kq_hi(xq_sb, wq_sb, 0, 1, "q01"), None),
                    (-97.7, dma_x_lv(xq_sb, xq_r, 512, 1024, 1), None),
                    (-97.6, proj_kq_lo(xq_sb, wq_sb, 0, 1, qh, False, "q01"),
                     None),
                ]
            else:
                producers += [
                    (-99.0, dma_w_hp(wk_sb, wk, 0), None),
                    (-98.8, dma_x(xk_sb, xk_r, 0, 512), None),
                    (-98.6, dma_w_hp(wq_sb, wq, 0), None),
                    (-98.4, dma_x(xq_sb, xq_r, 0, 512), None),
                    (-98.2, proj_kq(xk_sb, wk_sb, 0, 0, kh, True), None),
                    (-98.0, proj_kq(xq_sb, wq_sb, 0, 0, qh, False), None),
                    (-97.8, dma_x(xq_sb, xq_r, 512, 1024), None),
                    (-97.6, proj_kq(xq_sb, wq_sb, 0, 1, qh, False), None),
                ]
            # Σv per head: ones-lhsT matmuls into the (idle until hp1) proj
            # bank once all v tiles are in SBUF, then one pre-scaled drain.
            _sv = {}

            def sv_accum():
                def run():
                    _sv["tl"] = proj_tile()
                    wide = _sv["tl"][:, 0, :]
                    n = 0
                    for h in range(NH):
                        for kt in range(NKT):
                            nc.tensor.matmul(
                                wide[:, h * HD:(h + 1) * HD],
                                lhsT=ones_sb[:],
                                rhs=v_sb[:, kt, h * HD:(h + 1) * HD],
                                start=(n == 0), stop=(n == NH * NKT - 1),
                                skip_group_check=True,
                            )
                            n += 1
                return run

            def sv_drain():
                def run():
                    charge("DVE", 512)
                    nc.vector.tensor_scalar_mul(
                        sv_sb[:],
                        _sv.pop("tl")[:, 0, :].rearrange(
                            "p (h2 d) -> p h2 d", d=HD),
                        0.5 / WS)
                    _sv["done"] = True
                return run

            # input stream in DMA-arrival order (the DMA engines are
            # effectively serial; emission order = transfer order); PE
            # producers positioned at their consumer's job index so the
            # in-order PE queue never inverts the arrival order.
            if AUGK == 0:
                producers += [
                    # xk c1 -> kh c1 (kt 4-7, needed from job ~8)
                    (-89.8, dma_x_lv(xk_sb, xk_r, 512, 1024, 0), None),
                    (-89.7, proj_kq_hi(xk_sb, wk_sb, 0, 1, "k01"), None),
                    (-89.6, dma_x_lv(xk_sb, xk_r, 512, 1024, 1), None),
                    (-89.5, proj_kq_lo(xk_sb, wk_sb, 0, 1, kh, True, "k01"),
                     None),
                ]
            else:
                producers.append((-89.8, dma_x(xk_sb, xk_r, 512, 1024), None))
                producers.append((2, proj_kq(xk_sb, wk_sb, 0, 1, kh, True),
                                  None))
            # V stream: wv + xv c0 right behind xk c1
            producers.append((-89.4, dma_wv(), None))
            producers.append((-89.3, dma_xv(0, 0), None))
            producers.append((-89.2, dma_xv(0, 1), None))
            for t in range(4):
                producers.append((12 + 0.1 * t, proj_v(t, "hi"), None))
                producers.append((12.05 + 0.1 * t, proj_v(t, "lo"), t))
            # xk c2 -> kh c2 (kt 8-11, needed from job ~16)
            if AUGK == 0:
                producers += [
                    (13.0, dma_x_lv(xk_sb, xk_r, 1024, 1536, 0), None),
                    (13.1, proj_kq_hi(xk_sb, wk_sb, 0, 2, "k02"), None),
                    (13.2, dma_x_lv(xk_sb, xk_r, 1024, 1536, 1), None),
                    (13.3, proj_kq_lo(xk_sb, wk_sb, 0, 2, kh, True, "k02"),
                     None),
                ]
            else:
                producers.append((13.0, dma_x(xk_sb, xk_r, 1024, 1536), None))
                producers.append((13.3, proj_kq(xk_sb, wk_sb, 0, 2, kh, True),
                                  None))
            producers.append((13.4, dma_xv(1, 0), None))
            producers.append((13.5, dma_xv(1, 1), None))
            for t in range(4, 8):
                producers.append((16 + 0.1 * t, proj_v(t, "hi"), None))
                producers.append((16.05 + 0.1 * t, proj_v(t, "lo"), t))
            # xk c3 -> kh c3 (kt 12-15, needed from job ~24)
            if AUGK == 0:
                producers += [
                    (18.0, dma_x_lv(xk_sb, xk_r, 1536, 2048, 0), None),
                    (18.05, proj_kq_hi(xk_sb, wk_sb, 0, 3, "k03"), None),
                    (18.1, dma_x_lv(xk_sb, xk_r, 1536, 2048, 1), None),
                    (18.15, proj_kq_lo(xk_sb, wk_sb, 0, 3, kh, True, "k03"),
                     None),
                ]
            else:
                producers.append((18.0, dma_x(xk_sb, xk_r, 1536, 2048),
                                  None))
                producers.append((18.1,
                                  proj_kq(xk_sb, wk_sb, 0, 3, kh, True),
                                  None))
            producers.append((18.2, dma_xv(2, 0), None))
            producers.append((18.3, dma_xv(2, 1), None))
            for t in range(8, 12):
                producers.append((22 + 0.1 * t, proj_v(t, "hi"), None))
                producers.append((22.05 + 0.1 * t, proj_v(t, "lo"), t))
            producers.append((24.0, dma_xv(3, 0), None))
            producers.append((24.1, dma_xv(3, 1), None))
            for t in range(12, NKT):
                producers.append((26 + 0.1 * t, proj_v(t, "hi"), None))
                producers.append((26.05 + 0.1 * t, proj_v(t, "lo"), t))
            producers.append((30.0, sv_accum(), None))
            producers.append((30.1, sv_drain(), None))
            # xq c2/c3 ship last; qh c2/3 for hp0 first used at job 64
            producers.append((32.0, dma_x(xq_sb, xq_r, 1024, 1536), None))
            producers.append((32.5, dma_x(xq_sb, xq_r, 1536, 2048), None))
            producers.append((46, proj_kq_a(xq_sb, wq_sb, 0, 2, "q02"),
                              None))
            producers.append((48, proj_kq_b(xq_sb, wq_sb, 0, 2, qh, False,
                                            "q02"), None))
            producers.append((50, proj_kq_a(xq_sb, wq_sb, 0, 3, "q03"),
                              None))
            producers.append((52, proj_kq_b(xq_sb, wq_sb, 0, 3, qh, False,
                                            "q03"), None))
            # hp1-3: W DMAs + K/Q projections just-in-time for their blocks
            for hp in range(1, NHP):
                base = 128 * hp
                producers.append((base - 40, dma_w_hp(wk_sb, wk, hp), None))
                producers.append((base - 38, dma_w_hp(wq_sb, wq, hp), None))
                # A/B pairs of one projection share the single proj
                # PSUM bank -- their deadlines must never interleave with
                # another projection's pair (fractional offsets keep each
                # pair adjacent in the producer stream)
                for c in range(TOKC):
                    producers.append((base + 8 * c - 16.0,
                                      proj_kq_a(xk_sb, wk_sb, hp, c,
                                                f"k{hp}{c}"), None))
                    producers.append((base + 8 * c - 15.9,
                                      proj_kq_b(xk_sb, wk_sb, hp, c, kh, True,
                                                f"k{hp}{c}"), None))
                producers.append((base - 13.8,
                                  proj_kq_a(xq_sb, wq_sb, hp, 0, f"q{hp}0"),
                                  None))
                producers.append((base - 13.7,
                                  proj_kq_b(xq_sb, wq_sb, hp, 0, qh, False,
                                            f"q{hp}0"), None))
                producers.append((base - 11.8,
                                  proj_kq_a(xq_sb, wq_sb, hp, 1, f"q{hp}1"),
                                  None))
                producers.append((base - 11.7,
                                  proj_kq_b(xq_sb, wq_sb, hp, 1, qh, False,
                                            f"q{hp}1"), None))
                producers.append((base + 48.0,
                                  proj_kq_a(xq_sb, wq_sb, hp, 2, f"q{hp}2"),
                                  None))
                producers.append((base + 48.1,
                                  proj_kq_b(xq_sb, wq_sb, hp, 2, qh, False,
                                            f"q{hp}2"), None))
                producers.append((base + 52.0,
                                  proj_kq_a(xq_sb, wq_sb, hp, 3, f"q{hp}3"),
                                  None))
                producers.append((base + 52.1,
                                  proj_kq_b(xq_sb, wq_sb, hp, 3, qh, False,
                                            f"q{hp}3"), None))
            producers.sort(key=lambda e: e[0])
            producers = deque(producers)
            v_emit_wave = {}

            # AV bookkeeping
            av_fifo = deque()  # (job_idx, h, qc, kt, a_t, j_in_wave, wave)
            av_state = {"tile": None, "round": -1, "cool": -1}

            def finalize_round(r):
                av = av_state["tile"]
                hp, s = divmod(r, 4)
                h, qcp = 2 * hp + (s % 2), s // 2
                o_sb = opool.tile([P, 2, QC, HD], F32, tag="o_sb",
                                  name=f"osb_{r}")
                # o = (Σ attn'·v + Σv)·(1/(2·WS)) with attn' = tanh(s/2):
                # the sv term supplies the +Σv, pre-scaled by 0.5/WS
                charge("DVE", 512)
                nc.vector.affine_then_add(
                    out=o_sb[:].rearrange("p i qt d -> p (i qt) d"),
                    in0=av[:].rearrange("p (g d) -> p g d", d=HD),
                    in1=sv_sb[:, h].unsqueeze(1).broadcast_to([P, 2 * QC, HD]),
                    scale=0.5 / WS,
                    bias=0.0,
                )
                for i in range(2):
                    qc = 2 * qcp + i
                    dst = o[qc * 512:(qc + 1) * 512,
                            h * HD:(h + 1) * HD].rearrange(
                                "(qt p) d -> p qt d", p=P)
                    nc.sync.dma_start(dst, o_sb[:, i])
                av_state["tile"] = None

            def drain_avs(cur_wave, final=False):
                budget = 2  # cap per-wave AV emission so a backlog burst
                # never parks in front of the score stream in the in-order
                # PE queue
                continue_outer = False
                while av_fifo and not continue_outer:
                    job, h, qc, kt, a_t, j, w = av_fifo[0]
                    if not final:
                        if budget <= 0:
                            break
                        if w >= cur_wave:
                            break
                        vw = v_emit_wave.get(kt)
                        if vw is None or vw >= cur_wave:
                            break
                        budget -= 1
                    r = job // RJOBS
                    if r != av_state["round"]:
                        if not final and not _sv.get("done"):
                            break
                        if av_state["tile"] is not None:
                            finalize_round(av_state["round"])
                            # cool-down: keep the next round's AVs out of the
                            # in-order PE queue until the o-drain has had two
                            # waves to clear the av bank (they would WAR-block
                            # every score fill emitted behind them)
                            av_state["cool"] = cur_wave + 4
                        if not final and cur_wave < av_state["cool"]:
                            continue_outer = True
                            break
                        av_state["tile"] = ps_pool.tile(
                            [P, 512], F32, tag="av", bufs=1, name=f"av_{r}")
                        av_state["round"] = r
                    av_fifo.popleft()
                    av = av_state["tile"]
                    i = qc % 2
                    first = (kt == 0 and i == 0)
                    last = (kt == NKT - 1 and i == 1)
                    for qt in range(4):
                        nc.tensor.matmul(
                            av[:, (i * 4 + qt) * HD:(i * 4 + qt + 1) * HD],
                            lhsT=a_t[:, j, qt * P:(qt + 1) * P],
                            rhs=v_sb[:, kt, h * HD:(h + 1) * HD],
                            start=(first and qt == 0),
                            stop=(last and qt == 3),
                            skip_group_check=True,
                        )

            # ---------- main wave loop ----------
            def drain_producers(w, job_base):
                while producers and producers[0][0] <= job_base + 2:
                    due, closure, vtile = producers.popleft()
                    closure()
                    if vtile is not None:
                        v_emit_wave[vtile] = w

            job_base = 0
            for w, wave in enumerate(waves):
                drain_producers(w, job_base)
                g = len(wave)
                st = ps_pool.tile([P, 2, 512], F32, tag="st", bufs=3,
                                  name=f"st_{w}")
                for j, (h, qc, kt) in enumerate(wave):
                    hp, pb = h // 2, (h % 2) * HD
                    lhsT = kh[pb:pb + HD, hp, :, kt * P:(kt + 1) * P]
                    # two 256-col halves: the moving AP is [HD, 2, 256] = 512
                    # elements, the PE's MAX_MOVING_FREE_DIM_SIZE
                    for half in range(2):
                        rhs = qh[pb:pb + HD, hp,
                                 qc * 512 + half * 256:
                                 qc * 512 + (half + 1) * 256]
                        rhs = rhs.unsqueeze(1).broadcast_to([HD, 2, 256])
                        nc.tensor.matmul(
                            st[:, j, half * 256:(half + 1) * 256],
                            lhsT=lhsT,
                            rhs=rhs,
                            start=True,
                            stop=True,
                            perf_mode=mybir.MatmulPerfMode.DoubleRow,
                            tile_position=(pb, 0),
                            skip_group_check=True,
                        )
                a_t = apool.tile([P, 2, 512], BF16, tag="a_t", name=f"a_{w}")
                n_el = g * 512
                if est["DVE"] + n_el * 1.042 + 61 < est["ACT"] + n_el * 0.833 + 171:
                    # DVE share: clamped odd-quintic ~= tanh(s_true/2)
                    charge("DVE", n_el)
                    nc.vector._custom_dve(
                        TANH_OP,
                        out=a_t[:, :g, :],
                        in0=st[:, :g, :],
                        s0=K_FIT * 0.125 / (WS * WS),
                        s1=C1_FIT,
                        imm2=C2_FIT,
                    )
                else:
                    charge("ACT", n_el)
                    nc.scalar.activation(
                        out=a_t[:, :g, :],
                        in_=st[:, :g, :],
                        func=mybir.ActivationFunctionType.Tanh,
                        scale=0.0625 / (WS * WS),
                    )
                for j, (h, qc, kt) in enumerate(wave):
                    r = 4 * (h // 2) + 2 * (qc // 2) + (h % 2)
                    av_fifo.append((r * RJOBS + kt * 2 + (qc % 2),
                                    h, qc, kt, a_t, j, w))
                drain_avs(w)
                job_base += g
            import os
            if os.environ.get("KDBG"):
                print("EST at end:", est)
            while producers:
                producers.popleft()[1]()
            drain_avs(0, final=True)
            finalize_round(av_state["round"])

    nc.compile()
    return nc


def _prep_core_inputs(q, k, v, Wq, bq, Wk, bk, Wv, bv, KTC):
    """Host-side shard + transpose + split-fp8 packing. in_maps for 8 cores."""
    import ml_dtypes
    E4 = ml_dtypes.float8_e4m3
    KA = KTC * P
    aug = KA > D

    def split8(a):
        """[R, C] fp32 -> [R, 2, C] fp8 (hi, lo)."""
        hi = a.astype(E4)
        lo = (a - hi.astype(np.float32)).astype(E4)
        return np.ascontiguousarray(np.stack([hi, lo], axis=1))

    def x_t(x_b):  # [S, D] -> [KA, 2, S] fp8
        xt = np.ascontiguousarray(x_b.T)
        if aug:
            pad = np.zeros((KA, S), np.float32)
            pad[:D] = xt
            pad[D] = 1.0
            xt = pad
        return split8(xt)

    def w_kq(W, b, half):  # -> [NHP, P, KTC*2*128] fp8, p-major
        ws = np.ascontiguousarray(W[:, half * OF:(half + 1) * OF]) * WS
        if aug:
            pad = np.zeros((KA, OF), np.float32)
            pad[:D] = ws
            pad[D] = b[half * OF:(half + 1) * OF] * WS
            ws = pad
        s8 = split8(ws)  # [KA, 2, OF]
        pm = s8.reshape(KTC, P, 2, NHP, P).transpose(3, 1, 0, 2, 4)
        return np.ascontiguousarray(pm.reshape(NHP, P, KTC * 2 * P))

    def w_v(W, b, half):  # -> [KA, 2, OF] fp8
        ws = np.ascontiguousarray(W[:, half * OF:(half + 1) * OF]) * WS
        if aug:
            pad = np.zeros((KA, OF), np.float32)
            pad[:D] = ws
            pad[D] = b[half * OF:(half + 1) * OF] * WS
            ws = pad
        return split8(ws)

    xts = {}
    in_maps = []
    for c in range(N_CORES):
        b, half = divmod(c, 2)
        if b not in xts:
            xts[b] = (x_t(q[b]), x_t(k[b]), x_t(v[b]))
        xq_c, xk_c, xv_c = xts[b]
        in_maps.append({
            "xq": xq_c,
            "xk": xk_c,
            "xv": xv_c,
            "wq": w_kq(Wq, bq, half),
            "wk": w_kq(Wk, bk, half),
            "wv": w_v(Wv, bv, half),
        })
    return in_maps


def kernel(q, k, v, Wq, bq, Wk, bk, Wv, bv):
    global last_results
    q = np.ascontiguousarray(np.asarray(q, np.float32))
    k = np.ascontiguousarray(np.asarray(k, np.float32))
    v = np.ascontiguousarray(np.asarray(v, np.float32))
    Wq = np.asarray(Wq, np.float32)
    Wk = np.asarray(Wk, np.float32)
    Wv = np.asarray(Wv, np.float32)
    bq = np.asarray(bq, np.float32)
    bk = np.asarray(bk, np.float32)
    bv = np.asarray(bv, np.float32)

    aug = any(np.any(b_) for b_ in (bq, bk, bv))
    KTC = (D // P) + (1 if aug else 0)

    if KTC not in _cache:
        _cache[KTC] = _build(KTC)
    nc = _cache[KTC]

    in_maps = _prep_core_inputs(q, k, v, Wq, bq, Wk, bk, Wv, bv, KTC)
    res = run_bass_kernel_spmd(nc, in_maps, core_ids=list(range(N_CORES)))
    last_results = res

    out = np.empty((B, S, D), np.float32)
    for c in range(N_CORES):
        b, half = divmod(c, 2)
        out[b, :, half * OF:(half + 1) * OF] = res.results[c]["o"]
    return out



# revision 67
# speedup vs baseline: 1.2010x; 1.0007x over previous
"""Trainium2 Bass kernel for nn_MultiHeadAttention_69106023793143.

Reference computation (B=4, S=2048, D=1024, H=16, HD=64):
    qh = split_heads(q @ Wq + bq); kh, vh likewise
    out = merge_heads(sigmoid((qh @ kh^T) / sqrt(HD)) @ vh)

Sharding (8 cores): core c handles batch b = c//2 and the half = c%2 slice of
the feature axis (512 features = 8 heads).  Projections are tensor-parallel on
the output dim of Wq/Wk/Wv; attention is head-parallel.

Device strategy per core (three-way balanced pipeline):
  - The elementwise nonlinearity over 8*2048*2048 scores is split across
    BOTH elementwise engines via the identity sigma(s) = (1+tanh(s/2))/2:
    out = (SUM tanh(s_k/2) v_k + SUM v_k) / 2.  ACT computes exact tanh
    (1 elem/cyc/partition @1.2GHz); the DVE computes a clamped odd-quintic
    approximation of tanh as ONE fused custom-DVE op (TANH_PC5_ANT,
    registered at import: p(u)=u*(C1+u^2*(C2+u^2)), u=clamp(C0*s,+-1),
    8 ALU stages, N(0,1)-weighted RMS 3.5e-3).  A greedy ledger (est/charge)
    assigns each score wave to whichever engine has less planned busy time,
    counting the PSUM->SBUF drains that only the DVE can do.  This turns the
    218us single-engine ACT floor into ~155us across two engines, leaving
    the PE (~178us busy) as the pacing engine.
  - The +SUM v_k term: ones-lhsT matmuls accumulate SUM v per head once into
    the briefly-idle proj bank (sv_sb, prescaled 0.5/WS); each round's o
    drain is AFFINE_THEN_ADD (one DVE op: o = av*(0.5/WS) + sv broadcast).
  - Projections run as split-fp8 DoubleRow matmuls: host ships x and W as
    (hi, lo) fp8e4 pairs and the product takes the three cross terms
    xh*Wh + xh*Wl + xl*Wh - ~bf16 accuracy at 0.75x the bf16 PE cost.
    W is host-scaled by WS=16 to keep fp8 W normal; scales fold into the
    tanh arg and the o drain.
  - Scores use fp8e4 DoubleRow matmuls at 0.5 cyc/row: kh stored as an
    (hi, lo) fp8 pair on the two DoubleRow K-blocks (k-side compensated),
    qh plain fp8 broadcast (stride-0).  Odd heads at tile_position (64,0).
    Each score matmul moves [HD,2,256]=512 elems = MAX_MOVING_FREE_DIM_SIZE
    (a single 512-col matmul moves 1024 and SILENTLY mis-executes on HW).
  - Rounds are ordered in head-pair blocks (2hp,0),(2hp+1,0),(2hp,1),
    (2hp+1,1) so early rounds need only qh chunks 0/1: the DMA stream is
    wk,xk0,wq,xq0,xq1,xk1,wv,xv0,xk2,xv1,xk3,xv2,xv3,xq2,xq3 - kh chunks
    arrive just-in-time for round 0's kt sweep and V-projections/AV start
    ~15us in, while xq c2/c3 (2MB) ship last.  K/Q projections for hp1-3
    are emitted just-in-time before their blocks as adjacent A/B half-pairs
    (pairs must NEVER interleave: they share the single proj PSUM bank).
  - AV in bf16, out[q,d]: lhsT = attn^T tile [128k,128q], rhs = v[128k,64].
    Each round accumulates 8 q-tiles interleaved in ONE PSUM av bank
    (start=True on first, stop=True on last).  AVs drain from a FIFO gated
    on their V-tile's emission, capped at 2/wave, with a 4-wave cool-down
    after each round boundary so the next round's AVs never WAR-block the
    in-order PE queue on the o drain.
  - PSUM: 3 x 2-bank score-wave tiles (2-job waves) + 1 proj bank + 1 av
    bank = 8.  SBUF: 28 attn wave buffers.
  - Warm-up matmuls ramp the PE p-state (0.65->2.4GHz) during the prefix.
  - Nonzero biases fold in via a host-side augmented ones-row (KTC=9).

End-to-end: max rel err ~1.39e-2 (budget 2e-2); TimelineSim 242555ns
(baseline 297223ns).
"""

import sys

if "/opt/trn_rl_repo" not in sys.path:
    sys.path.insert(0, "/opt/trn_rl_repo")

from collections import deque
from contextlib import ExitStack

import numpy as np

import concourse.tile as tile
from concourse import bacc, mybir
from concourse import dve_ops as _dve_ops
from concourse.bass_utils import run_bass_kernel_spmd
from concourse.dve_spec import C0, C1, C2, One, Spec, Src0, Zero, lower, maxx, minn, sq
from concourse.dve_uop import DveOpSpec

# ---- custom DVE op: clamped odd-quintic tanh approximation -----------------
# p(u) = u*(C1 + u^2*(C2 + u^2)), u = clamp(Src0*C0, -1, 1)  [8 ALU stages]
# Approximates tanh(k_fit * s / C0_rel ...): with C0 = K_FIT*raw_scale it
# computes tanh(s_true/2) to 3.5e-3 weighted RMS over s_true ~ N(0,1)
# (max err 0.034 at the |s|~4 clamp shoulder).  The quintic coefficient is
# slaved to 1 in u-units, which keeps the expression inside the DVE's
# 8-stage budget with only 3 scalar slots.
_TANH_NAME = "TANH_PC5_ANT"
K_FIT = 0.25283828
C1_FIT = 1.94641582
C2_FIT = -1.95047264


def _tanh_pc5_ref(in0, in1, s0, s1, imm2):
    u = np.clip(np.asarray(in0, np.float32) * s0, -1.0, 1.0)
    u2 = u * u
    return u * (s1 + u2 * (imm2 + u2))


def _register_tanh_op():
    for op in _dve_ops.OPS:
        if op.name == _TANH_NAME:
            return op
    t = Src0 * C0
    u = maxx(minn(t, One), Zero - One)
    u2 = sq(u)
    spec = Spec(body=u * (C1 + u2 * (C2 + u2)), reference=_tanh_pc5_ref)
    shas = {
        ver: DveOpSpec(name=_TANH_NAME, uops=lower(spec, ver=ver)).sha(ver)
        for ver in ("v3", "v4")
    }
    op = _dve_ops.DveOp(_TANH_NAME, spec, subdim=False, uops_sha=shas)
    _dve_ops.OPS.append(op)
    _dve_ops.CUSTOM_DVE_SPECS[op.name] = spec
    _dve_ops._SUB_OPCODE_FOR_NAME[op.name] = (
        _dve_ops._CUSTOM_DVE_ROW_BASE + len(_dve_ops.OPS) - 1
    )
    return op


TANH_OP = _register_tanh_op()

B, S, D, H = 4, 2048, 1024, 16
HD = D // H  # 64
OF = D // 2  # 512 features (8 heads) per core
N_CORES = 8
P = 128
NH = 8          # heads per core
NHP = 4         # head pairs per core
QC = 4          # q-chunks of 512
NQCP = 2        # q-chunk pairs
NKT = 16        # k token tiles of 128
TOKC = 4        # x token chunks of 512
RJOBS = 2 * NKT  # jobs per round (2 q-chunks x 16 kt)
ABUFS = 28      # attn (a_t) wave buffers
WS = 16.0       # host-side W scale (keeps fp8 W out of subnormals)

F32 = mybir.dt.float32
BF16 = mybir.dt.bfloat16
FP8 = mybir.dt.float8e4

# the three split-fp8 cross terms (w level, x level)
TERMS = ((0, 0), (0, 1), (1, 0))

_cache: dict = {}
last_results = None


def _build(KTC: int):
    """KTC = contraction k-tiles for the projections (8, or 9 when biases are
    folded in via an augmented ones-row)."""
    nc = bacc.Bacc("TRN2", target_bir_lowering=False, debug=False,
                   num_devices=N_CORES, name="mha_sig4")
    KA = KTC * P
    NDR = KTC // 2   # DoubleRow kt-pairs per term
    AUGK = KTC % 2   # leftover kt (the ones-row) as plain fp8 matmul
    WFREE = KTC * 2 * P  # per-partition elements of one head-pair W slice

    xq = nc.dram_tensor("xq", [KA, 2, S], FP8, kind="ExternalInput")
    xk = nc.dram_tensor("xk", [KA, 2, S], FP8, kind="ExternalInput")
    xv = nc.dram_tensor("xv", [KA, 2, S], FP8, kind="ExternalInput")
    # wq/wk p-major: [head-pair, partition, kt*level*128]
    wq = nc.dram_tensor("wq", [NHP, P, WFREE], FP8, kind="ExternalInput")
    wk = nc.dram_tensor("wk", [NHP, P, WFREE], FP8, kind="ExternalInput")
    wv = nc.dram_tensor("wv", [KA, 2, OF], FP8, kind="ExternalInput")
    o = nc.dram_tensor("o", [S, OF], F32, kind="ExternalOutput")

    xq_r = xq.rearrange("(kt p) l t -> p kt l t", p=P)
    xk_r = xk.rearrange("(kt p) l t -> p kt l t", p=P)
    xv_r = xv.rearrange("(kt p) l t -> p kt l t", p=P)
    wv_r = wv.rearrange("(kt p) l n -> p kt l n", p=P)

    abufs = ABUFS if KTC == 8 else 10

    with tile.TileContext(nc) as tc:
        with ExitStack() as ctx:
            persist = ctx.enter_context(tc.tile_pool(name="persist", bufs=1))
            xvpool = ctx.enter_context(tc.tile_pool(name="xvpool", bufs=2))
            apool = ctx.enter_context(tc.tile_pool(name="apool", bufs=abufs))
            opool = ctx.enter_context(tc.tile_pool(name="opool", bufs=2))
            ps_pool = ctx.enter_context(
                tc.tile_pool(name="ps_pool", bufs=2, space="PSUM"))

            wk_sb = persist.tile([P, NHP, KTC, 2, P], FP8)
            wq_sb = persist.tile([P, NHP, KTC, 2, P], FP8)
            wv_sb = persist.tile([P, KTC, 2, OF], FP8)
            xk_sb = persist.tile([P, KTC, 2, S], FP8)
            xq_sb = persist.tile([P, KTC, 2, S], FP8)
            # kh as (hi, lo) fp8 pair, head pairs stacked on partitions;
            # qh plain fp8; v bf16 [tok, of]
            kh = persist.tile([P, NHP, 2, S], FP8)
            qh = persist.tile([P, NHP, S], FP8)
            v_sb = persist.tile([P, NKT, OF], BF16)
            # (Σ_k v)·WS·(0.5/WS) per head, broadcast-added at the o drain
            sv_sb = persist.tile([P, NH, HD], BF16)
            ones_sb = persist.tile([P, P], BF16)
            nc.vector.memset(ones_sb[:], 1.0)

            # ONE PSUM bank for all projections: two [P, 256] slots in a
            # persistent tile, manually rotated. Region-based dep tracking
            # gives WAR/WAW per slot; each half-group's start=True re-marks
            # the whole bank but PSUM reads return raw data for re-marked
            # bytes (hardware-verified), and no other slot is ever
            # mid-accumulation when a start executes (serial emission).
            proj_ps = ps_pool.tile([P, 2, 256], F32, tag="proj", bufs=1)

            class _ProjView:
                """Adapter exposing the proj bank as tile[:, 0, cols]:
                cols 0:256 -> slot 0, 256:512 -> slot 1, full -> wide."""

                def __getitem__(self, idx):
                    c = idx[2]
                    if c == slice(None):
                        return proj_ps[:].rearrange("p s n -> p (s n)")
                    return proj_ps[:, 0 if c.start == 0 else 1, :]

            def proj_tile():
                return _ProjView()

            # ---------- engine-balance ledger ----------
            # planned busy ns for ACT / DVE; drains charge DVE (or ACT) at
            # emission so the per-wave greedy pick stays globally balanced
            est = {"ACT": 0.0, "DVE": 0.0}

            def charge(eng, n_elems, ov=None):
                est[eng] += n_elems * 0.833 + 171 if eng == "ACT" \
                    else n_elems * 1.042 + 61

            def bal_copy(out, in_, n, scale=None):
                """PSUM->SBUF copy (optionally scaled) on whichever of
                ACT/DVE the ledger says is less loaded."""
                if est["ACT"] + n * 0.833 + 171 <= est["DVE"] + n * 1.042 + 61:
                    charge("ACT", n)
                    nc.scalar.activation(
                        out=out, in_=in_,
                        func=mybir.ActivationFunctionType.Copy,
                        scale=1.0 if scale is None else scale)
                else:
                    charge("DVE", n)
                    if scale is None:
                        nc.vector.tensor_copy(out=out, in_=in_)
                    else:
                        nc.vector.tensor_scalar_mul(out, in_, scale)

            # ---------- producer closures ----------
            def dma_w_hp(w_sb, w_dram, hp):
                def run():
                    nc.sync.dma_start(
                        w_sb[:, hp].rearrange("p kt l n -> p (kt l n)"),
                        w_dram[hp])
                return run

            def dma_wv():
                def run():
                    nc.sync.dma_start(wv_sb[:], wv_r)
                return run

            def dma_x(x_sb, x_r, lo, hi):
                def run():
                    for lv in range(2):
                        nc.sync.dma_start(
                            x_sb[:, :, lv, lo:hi],
                            x_r[:, :, lv, lo:hi])
                return run

            xv_tiles = {}

            def dma_xv(c, lv):
                def run():
                    if lv == 0:
                        xv_tiles[c] = xvpool.tile([P, KTC, 2, 512], FP8,
                                                  tag="xvchunk",
                                                  name=f"xv_{c}")
                    nc.sync.dma_start(
                        xv_tiles[c][:, :, lv, :],
                        xv_r[:, :, lv, c * 512:(c + 1) * 512])
                return run

            # warm-up: the PE runs at 0.65/1.2GHz until ~3us of continuous
            # execution; burn dummy matmuls during the prefix DMAs so the
            # first projections run at full clock
            wu_sb = persist.tile([HD, 2, 640], FP8)

            def warmup(n_mm):
                def run():
                    if n_mm < 0:
                        nc.vector.memset(wu_sb[:], 0)
                        return
                    st = ps_pool.tile([P, 2, 512], F32, tag="st", bufs=3,
                                      name=f"wu_{n_mm}")
                    for m in range(n_mm):
                        nc.tensor.matmul(
                            st[:, m % 2, :],
                            lhsT=wu_sb[:, :, 0:P],
                            rhs=wu_sb[:, :, P:P + 512],
                            start=True, stop=True,
                            perf_mode=mybir.MatmulPerfMode.DoubleRow,
                            skip_group_check=True,
                        )
                return run

            # split-emission prefix projections: the hi terms only need the
            # lv0 (hi) half of the x chunk, so they start ~1.5us earlier
            _prefix_ps = {}

            def _kq_half(x_sb, w_sb, hp, tsl, ps, terms, first):
                n = 0
                for lw, lx in terms:
                    for t in range(NDR):
                        nc.tensor.matmul(
                            ps,
                            lhsT=w_sb[:, hp, 2 * t:2 * t + 2, lw, :],
                            rhs=x_sb[:, 2 * t:2 * t + 2, lx, tsl],
                            start=(first and n == 0), stop=False,
                            perf_mode=mybir.MatmulPerfMode.DoubleRow,
                            skip_group_check=True,
                        )
                        n += 1

            def _kq_drain(dst, hp, tsl, ps, split_lo):
                charge("DVE", 512 if split_lo else 256)
                if split_lo:
                    charge("DVE", 256)
                    nc.vector.tensor_copy(out=dst[:, hp, 0, tsl], in_=ps)
                    nc.vector.tensor_sub(dst[:, hp, 1, tsl], ps,
                                         dst[:, hp, 0, tsl])
                else:
                    nc.vector.tensor_copy(out=dst[:, hp, tsl], in_=ps)

            def proj_kq_hi(x_sb, w_sb, hp, c, key):
                def run():
                    tl = proj_tile()
                    _prefix_ps[key] = tl
                    tsl = slice(c * 512, c * 512 + 256)
                    _kq_half(x_sb, w_sb, hp, tsl, tl[:, 0, 0:256],
                             ((0, 0), (1, 0)), True)
                return run

            def proj_kq_lo(x_sb, w_sb, hp, c, dst, split_lo, key):
                def run():
                    tl = _prefix_ps.pop(key)
                    tsl = slice(c * 512, c * 512 + 256)
                    _kq_half(x_sb, w_sb, hp, tsl, tl[:, 0, 0:256],
                             ((0, 1),), False)
                    _kq_drain(dst, hp, tsl, tl[:, 0, 0:256], split_lo)
                    tsl2 = slice(c * 512 + 256, (c + 1) * 512)
                    _kq_half(x_sb, w_sb, hp, tsl2, tl[:, 0, 256:512],
                             ((0, 0), (1, 0), (0, 1)), True)
                    _kq_drain(dst, hp, tsl2, tl[:, 0, 256:512], split_lo)
                return run

            def dma_x_lv(x_sb, x_r, lo, hi, lv):
                def run():
                    nc.sync.dma_start(
                        x_sb[:, :, lv, lo:hi],
                        x_r[:, :, lv, lo:hi])
                return run

            _kq_state = {}

            def _kq_matmuls(x_sb, w_sb, hp, c, tl, half):
                ps = tl[:, 0, half * 256:(half + 1) * 256]
                tsl = slice(c * 512 + half * 256,
                            c * 512 + (half + 1) * 256)
                n = 0
                for lw, lx in TERMS:
                    for t in range(NDR):
                        nc.tensor.matmul(
                            ps,
                            lhsT=w_sb[:, hp, 2 * t:2 * t + 2, lw, :],
                            rhs=x_sb[:, 2 * t:2 * t + 2, lx, tsl],
                            start=(n == 0), stop=False,
                            perf_mode=mybir.MatmulPerfMode.DoubleRow,
                            skip_group_check=True,
                        )
                        n += 1
                    if AUGK:
                        nc.tensor.matmul(
                            ps,
                            lhsT=w_sb[:, hp, KTC - 1, lw, :],
                            rhs=x_sb[:, KTC - 1, lx, tsl],
                            start=(n == 0), stop=False,
                            skip_group_check=True,
                        )
                        n += 1

            def proj_kq_a(x_sb, w_sb, hp, c, key):
                """first 256-half of a K/Q chunk projection -- emitted two
                jobs before the second half so score fills slip between"""
                def run():
                    tl = proj_tile()
                    _kq_state[key] = tl
                    _kq_matmuls(x_sb, w_sb, hp, c, tl, 0)
                return run

            def proj_kq_b(x_sb, w_sb, hp, c, dst, split_lo, key):
                def run():
                    tl = _kq_state.pop(key)
                    _kq_matmuls(x_sb, w_sb, hp, c, tl, 1)
                    wide = tl[:, 0, :]
                    _kq_finish(wide, dst, hp, c, split_lo)
                return run

            def _kq_finish(wide, dst, hp, c, split_lo):
                if True:
                    sl = slice(c * 512, (c + 1) * 512)
                    if split_lo:
                        charge("DVE", 512)
                        charge("DVE", 512)
                        nc.vector.tensor_copy(out=dst[:, hp, 0, sl], in_=wide)
                        nc.vector.tensor_sub(dst[:, hp, 1, sl], wide,
                                             dst[:, hp, 0, sl])
                    else:
                        charge("DVE", 512)
                        nc.vector.tensor_copy(out=dst[:, hp, sl], in_=wide)

            def proj_kq(x_sb, w_sb, hp, c, dst, split_lo):
                def run():
                    tl = proj_tile()
                    _kq_matmuls(x_sb, w_sb, hp, c, tl, 0)
                    _kq_matmuls(x_sb, w_sb, hp, c, tl, 1)
                    _kq_finish(tl[:, 0, :], dst, hp, c, split_lo)
                return run

            _v_ps = {}

            def proj_v(t, part):
                """v_sb[:, t, :] = x-token-tile t @ Wv ([tok, of]).
                part 'hi' takes the two x-hi terms, 'lo' the x-lo term +
                the PSUM drain (finer interleaving in the in-order PE queue,
                and 'hi' only needs the lv0 half of the xv chunk)."""
                def run():
                    xt = xv_tiles[t // 4]
                    tsl = slice((t % 4) * P, (t % 4 + 1) * P)

                    def v_half(ps, osl, terms, first):
                        n = 0
                        for lw, lx in terms:
                            for u in range(NDR):
                                nc.tensor.matmul(
                                    ps,
                                    lhsT=xt[:, 2 * u:2 * u + 2, lx, tsl],
                                    rhs=wv_sb[:, 2 * u:2 * u + 2, lw, osl],
                                    start=(first and n == 0), stop=False,
                                    perf_mode=mybir.MatmulPerfMode.DoubleRow,
                                    skip_group_check=True,
                                )
                                n += 1
                            if AUGK:
                                nc.tensor.matmul(
                                    ps,
                                    lhsT=xt[:, KTC - 1, lx, tsl],
                                    rhs=wv_sb[:, KTC - 1, lw, osl],
                                    start=(first and n == 0), stop=False,
                                    skip_group_check=True,
                                )
                                n += 1

                    if part == "hi":
                        tl = proj_tile()
                        _v_ps[t] = tl
                        v_half(tl[:, 0, 0:256], slice(0, 256),
                               ((0, 0), (1, 0)), True)
                    else:
                        tl = _v_ps.pop(t)
                        v_half(tl[:, 0, 0:256], slice(0, 256),
                               ((0, 1),), False)
                        v_half(tl[:, 0, 256:512], slice(256, 512),
                               ((0, 0), (1, 0), (0, 1)), True)
                        charge("DVE", 512)
                        nc.vector.tensor_copy(
                            out=v_sb[:, t, :], in_=tl[:, 0, :])
                return run

            # ---------- static schedule ----------
            # rounds in head-pair blocks: (2hp,0),(2hp+1,0),(2hp,1),(2hp+1,1)
            # so the early rounds need only qh chunks 0/1 -- xq c2/c3 (2MB)
            # can ship after the whole xk/xv stream.  Round index
            # r(h,qc) = 4*(h//2) + 2*(qc//2) + h%2; job = r*32 + kt*2 + i.
            # Round 0 staggers its second q-chunk by 2 k-tiles so the first
            # waves only need qh[qc0] (whose projection finishes first).
            rounds = [(2 * hp + (s % 2), s // 2)
                      for hp in range(NHP) for s in range(4)]
            jobs = []
            for kt in range(NKT + 2):
                if kt < NKT:
                    jobs.append((0, 0, kt))
                if kt >= 2:
                    jobs.append((0, 1, kt - 2))
            for h, qcp in rounds[1:]:
                jobs += [(h, 2 * qcp + i, kt)
                         for kt in range(NKT) for i in range(2)]
            waves = [jobs[i0:i0 + 2] for i0 in range(0, len(jobs), 2)]

            producers = []  # (due_job, closure, vtile_or_None)
            # prefix + all input DMAs in deadline order (the DMA engines are
            # effectively serial; emission order = transfer order)
            if AUGK == 0:
                producers += [
                    (-99.9, warmup(-1), None),
                    (-99.8, warmup(10), None),
                    (-99.0, dma_w_hp(wk_sb, wk, 0), None),
                    (-98.9, dma_x_lv(xk_sb, xk_r, 0, 512, 0), None),
                    (-98.8, dma_w_hp(wq_sb, wq, 0), None),
                    (-98.7, proj_kq_hi(xk_sb, wk_sb, 0, 0, "k00"), None),
                    (-98.6, dma_x_lv(xk_sb, xk_r, 0, 512, 1), None),
                    (-98.5, proj_kq_lo(xk_sb, wk_sb, 0, 0, kh, True, "k00"),
                     None),
                    (-98.4, dma_x_lv(xq_sb, xq_r, 0, 512, 0), None),
                    (-98.3, warmup(4), None),
                    (-98.2, proj_kq_hi(xq_sb, wq_sb, 0, 0, "q00"), None),
                    (-98.1, dma_x_lv(xq_sb, xq_r, 0, 512, 1), None),
                    (-98.0, proj_kq_lo(xq_sb, wq_sb, 0, 0, qh, False, "q00"),
                     None),
                    (-97.9, dma_x_lv(xq_sb, xq_r, 512, 1024, 0), None),
                    (-97.8, proj_kq_hi(xq_sb, wq_sb, 0, 1, "q01"), None),
                    (-97.7, dma_x_lv(xq_sb, xq_r, 512, 1024, 1), None),
                    (-97.6, proj_kq_lo(xq_sb, wq_sb, 0, 1, qh, False, "q01"),
                     None),
                ]
            else:
                producers += [
                    (-99.0, dma_w_hp(wk_sb, wk, 0), None),
                    (-98.8, dma_x(xk_sb, xk_r, 0, 512), None),
                    (-98.6, dma_w_hp(wq_sb, wq, 0), None),
                    (-98.4, dma_x(xq_sb, xq_r, 0, 512), None),
                    (-98.2, proj_kq(xk_sb, wk_sb, 0, 0, kh, True), None),
                    (-98.0, proj_kq(xq_sb, wq_sb, 0, 0, qh, False), None),
                    (-97.8, dma_x(xq_sb, xq_r, 512, 1024), None),
                    (-97.6, proj_kq(xq_sb, wq_sb, 0, 1, qh, False), None),
                ]
            # Σv per head: ones-lhsT matmuls into the (idle until hp1) proj
            # bank once all v tiles are in SBUF, then one pre-scaled drain.
            _sv = {}

            def sv_accum():
                def run():
                    _sv["tl"] = proj_tile()
                    wide = _sv["tl"][:, 0, :]
                    n = 0
                    for h in range(NH):
                        for kt in range(NKT):
                            nc.tensor.matmul(
                                wide[:, h * HD:(h + 1) * HD],
                                lhsT=ones_sb[:],
                                rhs=v_sb[:, kt, h * HD:(h + 1) * HD],
                                start=(n == 0), stop=(n == NH * NKT - 1),
                                skip_group_check=True,
                            )
                            n += 1
                return run

            def sv_drain():
                def run():
                    charge("DVE", 512)
                    nc.vector.tensor_scalar_mul(
                        sv_sb[:],
                        _sv.pop("tl")[:, 0, :].rearrange(
                            "p (h2 d) -> p h2 d", d=HD),
                        0.5 / WS)
                    _sv["done"] = True
                return run

            # input stream in DMA-arrival order (the DMA engines are
            # effectively serial; emission order = transfer order); PE
            # producers positioned at their consumer's job index so the
            # in-order PE queue never inverts the arrival order.
            if AUGK == 0:
                producers += [
                    # xk c1 -> kh c1 (kt 4-7, needed from job ~8)
                    (-89.8, dma_x_lv(xk_sb, xk_r, 512, 1024, 0), None),
                    (-89.7, proj_kq_hi(xk_sb, wk_sb, 0, 1, "k01"), None),
                    (-89.6, dma_x_lv(xk_sb, xk_r, 512, 1024, 1), None),
                    (-89.5, proj_kq_lo(xk_sb, wk_sb, 0, 1, kh, True, "k01"),
                     None),
                ]
            else:
                producers.append((-89.8, dma_x(xk_sb, xk_r, 512, 1024), None))
                producers.append((2, proj_kq(xk_sb, wk_sb, 0, 1, kh, True),
                                  None))
            # V stream: wv + xv c0 right behind xk c1
            producers.append((-89.4, dma_wv(), None))
            producers.append((-89.3, dma_xv(0, 0), None))
            producers.append((-89.2, dma_xv(0, 1), None))
            for t in range(4):
                producers.append((12 + 0.1 * t, proj_v(t, "hi"), None))
                producers.append((12.05 + 0.1 * t, proj_v(t, "lo"), t))
            # xk c2 -> kh c2 (kt 8-11, needed from job ~16)
            if AUGK == 0:
                producers += [
                    (13.0, dma_x_lv(xk_sb, xk_r, 1024, 1536, 0), None),
                    (13.1, proj_kq_hi(xk_sb, wk_sb, 0, 2, "k02"), None),
                    (13.2, dma_x_lv(xk_sb, xk_r, 1024, 1536, 1), None),
                    (13.3, proj_kq_lo(xk_sb, wk_sb, 0, 2, kh, True, "k02"),
                     None),
                ]
            else:
                producers.append((13.0, dma_x(xk_sb, xk_r, 1024, 1536), None))
                producers.append((13.3, proj_kq(xk_sb, wk_sb, 0, 2, kh, True),
                                  None))
            producers.append((13.4, dma_xv(1, 0), None))
            producers.append((13.5, dma_xv(1, 1), None))
            for t in range(4, 8):
                producers.append((16 + 0.1 * t, proj_v(t, "hi"), None))
                producers.append((16.05 + 0.1 * t, proj_v(t, "lo"), t))
            # xk c3 -> kh c3 (kt 12-15, needed from job ~24)
            if AUGK == 0:
                producers += [
                    (18.0, dma_x_lv(xk_sb, xk_r, 1536, 2048, 0), None),
                    (18.05, proj_kq_hi(xk_sb, wk_sb, 0, 3, "k03"), None),
                    (18.1, dma_x_lv(xk_sb, xk_r, 1536, 2048, 1), None),
                    (18.15, proj_kq_lo(xk_sb, wk_sb, 0, 3, kh, True, "k03"),
                     None),
                ]
            else:
                producers.append((18.0, dma_x(xk_sb, xk_r, 1536, 2048),
                                  None))
                producers.append((18.1,
                                  proj_kq(xk_sb, wk_sb, 0, 3, kh, True),
                                  None))
            producers.append((18.2, dma_xv(2, 0), None))
            producers.append((18.3, dma_xv(2, 1), None))
            for t in range(8, 12):
                producers.append((22 + 0.1 * t, proj_v(t, "hi"), None))
                producers.append((22.05 + 0.1 * t, proj_v(t, "lo"), t))
            producers.append((24.0, dma_xv(3, 0), None))
            producers.append((24.1, dma_xv(3, 1), None))
            for t in range(12, NKT):
                producers.append((26 + 0.1 * t, proj_v(t, "hi"), None))
                producers.append((26.05 + 0.1 * t, proj_v(t, "lo"), t))
            producers.append((30.0, sv_accum(), None))
            producers.append((30.1, sv_drain(), None))
            # xq c2/c3 ship last; qh c2/3 for hp0 first used at job 64
            producers.append((32.0, dma_x(xq_sb, xq_r, 1024, 1536), None))
            producers.append((32.5, dma_x(xq_sb, xq_r, 1536, 2048), None))
            producers.append((46, proj_kq_a(xq_sb, wq_sb, 0, 2, "q02"),
                              None))
            producers.append((48, proj_kq_b(xq_sb, wq_sb, 0, 2, qh, False,
                                            "q02"), None))
            producers.append((50, proj_kq_a(xq_sb, wq_sb, 0, 3, "q03"),
                              None))
            producers.append((52, proj_kq_b(xq_sb, wq_sb, 0, 3, qh, False,
                                            "q03"), None))
            # hp1-3: W DMAs + K/Q projections just-in-time for their blocks
            for hp in range(1, NHP):
                base = 128 * hp
                producers.append((base - 40, dma_w_hp(wk_sb, wk, hp), None))
                producers.append((base - 38, dma_w_hp(wq_sb, wq, hp), None))
                # A/B pairs of one projection share the single proj
                # PSUM bank -- their deadlines must never interleave with
                # another projection's pair (fractional offsets keep each
                # pair adjacent in the producer stream)
                for c in range(TOKC):
                    producers.append((base + 8 * c - 16.0,
                                      proj_kq_a(xk_sb, wk_sb, hp, c,
                                                f"k{hp}{c}"), None))
                    producers.append((base + 8 * c - 15.9,
                                      proj_kq_b(xk_sb, wk_sb, hp, c, kh, True,
                                                f"k{hp}{c}"), None))
                producers.append((base - 13.8,
                                  proj_kq_a(xq_sb, wq_sb, hp, 0, f"q{hp}0"),
                                  None))
                producers.append((base - 13.7,
                                  proj_kq_b(xq_sb, wq_sb, hp, 0, qh, False,
                                            f"q{hp}0"), None))
                producers.append((base - 11.8,
                                  proj_kq_a(xq_sb, wq_sb, hp, 1, f"q{hp}1"),
                                  None))
                producers.append((base - 11.7,
                                  proj_kq_b(xq_sb, wq_sb, hp, 1, qh, False,
                                            f"q{hp}1"), None))
                producers.append((base + 48.0,
                                  proj_kq_a(xq_sb, wq_sb, hp, 2, f"q{hp}2"),
                                  None))
                producers.append((base + 48.1,
                                  proj_kq_b(xq_sb, wq_sb, hp, 2, qh, False,
                                            f"q{hp}2"), None))
                producers.append((base + 52.0,
                                  proj_kq_a(xq_sb, wq_sb, hp, 3, f"q{hp}3"),
                                  None))
                producers.append((base + 52.1,
                                  proj_kq_b(xq_sb, wq_sb, hp, 3, qh, False,
                                            f"q{hp}3"), None))
            producers.sort(key=lambda e: e[0])
            producers = deque(producers)
            v_emit_wave = {}

            # AV bookkeeping
            av_fifo = deque()  # (job_idx, h, qc, kt, a_t, j_in_wave, wave)
            av_state = {"tile": None, "round": -1, "cool": -1}

            def finalize_round(r):
                av = av_state["tile"]
                hp, s = divmod(r, 4)
                h, qcp = 2 * hp + (s % 2), s // 2
                o_sb = opool.tile([P, 2, QC, HD], F32, tag="o_sb",
                                  name=f"osb_{r}")
                # o = (Σ attn'·v + Σv)·(1/(2·WS)) with attn' = tanh(s/2):
                # the sv term supplies the +Σv, pre-scaled by 0.5/WS
                charge("DVE", 512)
                nc.vector.affine_then_add(
                    out=o_sb[:].rearrange("p i qt d -> p (i qt) d"),
                    in0=av[:].rearrange("p (g d) -> p g d", d=HD),
                    in1=sv_sb[:, h].unsqueeze(1).broadcast_to([P, 2 * QC, HD]),
                    scale=0.5 / WS,
                    bias=0.0,
                )
                for i in range(2):
                    qc = 2 * qcp + i
                    dst = o[qc * 512:(qc + 1) * 512,
                            h * HD:(h + 1) * HD].rearrange(
                                "(qt p) d -> p qt d", p=P)
                    nc.sync.dma_start(dst, o_sb[:, i])
                av_state["tile"] = None

            def drain_avs(cur_wave, final=False):
                budget = 2  # cap per-wave AV emission so a backlog burst
                # never parks in front of the score stream in the in-order
                # PE queue
                continue_outer = False
                while av_fifo and not continue_outer:
                    job, h, qc, kt, a_t, j, w = av_fifo[0]
                    if not final:
                        if budget <= 0:
                            break
                        if w >= cur_wave:
                            break
                        vw = v_emit_wave.get(kt)
                        if vw is None or vw >= cur_wave:
                            break
                        budget -= 1
                    r = job // RJOBS
                    if r != av_state["round"]:
                        if not final and not _sv.get("done"):
                            break
                        if av_state["tile"] is not None:
                            finalize_round(av_state["round"])
                            # cool-down: keep the next round's AVs out of the
                            # in-order PE queue until the o-drain has had two
                            # waves to clear the av bank (they would WAR-block
                            # every score fill emitted behind them)
                            av_state["cool"] = cur_wave + 4
                        if not final and cur_wave < av_state["cool"]:
                            continue_outer = True
                            break
                        av_state["tile"] = ps_pool.tile(
                            [P, 512], F32, tag="av", bufs=1, name=f"av_{r}")
                        av_state["round"] = r
                    av_fifo.popleft()
                    av = av_state["tile"]
                    i = qc % 2
                    first = (kt == 0 and i == 0)
                    last = (kt == NKT - 1 and i == 1)
                    for qt in range(4):
                        nc.tensor.matmul(
                            av[:, (i * 4 + qt) * HD:(i * 4 + qt + 1) * HD],
                            lhsT=a_t[:, j, qt * P:(qt + 1) * P],
                            rhs=v_sb[:, kt, h * HD:(h + 1) * HD],
                            start=(first and qt == 0),
                            stop=(last and qt == 3),
                            skip_group_check=True,
                        )

            # ---------- main wave loop ----------
            def drain_producers(w, job_base):
                while producers and producers[0][0] <= job_base + 2:
                    due, closure, vtile = producers.popleft()
                    closure()
                    if vtile is not None:
                        v_emit_wave[vtile] = w

            job_base = 0
            for w, wave in enumerate(waves):
                drain_producers(w, job_base)
                g = len(wave)
                st = ps_pool.tile([P, 2, 512], F32, tag="st", bufs=3,
                                  name=f"st_{w}")
                for j, (h, qc, kt) in enumerate(wave):
                    hp, pb = h // 2, (h % 2) * HD
                    lhsT = kh[pb:pb + HD, hp, :, kt * P:(kt + 1) * P]
                    # two 256-col halves: the moving AP is [HD, 2, 256] = 512
                    # elements, the PE's MAX_MOVING_FREE_DIM_SIZE
                    for half in range(2):
                        rhs = qh[pb:pb + HD, hp,
                                 qc * 512 + half * 256:
                                 qc * 512 + (half + 1) * 256]
                        rhs = rhs.unsqueeze(1).broadcast_to([HD, 2, 256])
                        nc.tensor.matmul(
                            st[:, j, half * 256:(half + 1) * 256],
                            lhsT=lhsT,
                            rhs=rhs,
                            start=True,
                            stop=True,
                            perf_mode=mybir.MatmulPerfMode.DoubleRow,
                            tile_position=(pb, 0),
                            skip_group_check=True,
                        )
                a_t = apool.tile([P, 2, 512], BF16, tag="a_t", name=f"a_{w}")
                n_el = g * 512
                if est["DVE"] + n_el * 1.042 + 61 - 80 < est["ACT"] + n_el * 0.833 + 171:
                    # DVE share: clamped odd-quintic ~= tanh(s_true/2)
                    charge("DVE", n_el)
                    nc.vector._custom_dve(
                        TANH_OP,
                        out=a_t[:, :g, :],
                        in0=st[:, :g, :],
                        s0=K_FIT * 0.125 / (WS * WS),
                        s1=C1_FIT,
                        imm2=C2_FIT,
                    )
                else:
                    charge("ACT", n_el)
                    nc.scalar.activation(
                        out=a_t[:, :g, :],
                        in_=st[:, :g, :],
                        func=mybir.ActivationFunctionType.Tanh,
                        scale=0.0625 / (WS * WS),
                    )
                for j, (h, qc, kt) in enumerate(wave):
                    r = 4 * (h // 2) + 2 * (qc // 2) + (h % 2)
                    av_fifo.append((r * RJOBS + kt * 2 + (qc % 2),
                                    h, qc, kt, a_t, j, w))
                drain_avs(w)
                job_base += g
            import os
            if os.environ.get("KDBG"):
                print("EST at end:", est)
            while producers:
                producers.popleft()[1]()
            drain_avs(0, final=True)
            finalize_round(av_state["round"])

    nc.compile()
    return nc


def _prep_core_inputs(q, k, v, Wq, bq, Wk, bk, Wv, bv, KTC):
    """Host-side shard + transpose + split-fp8 packing. in_maps for 8 cores."""
    import ml_dtypes
    E4 = ml_dtypes.float8_e4m3
    KA = KTC * P
    aug = KA > D

    def split8(a):
        """[R, C] fp32 -> [R, 2, C] fp8 (hi, lo)."""
        hi = a.astype(E4)
        lo = (a - hi.astype(np.float32)).astype(E4)
        return np.ascontiguousarray(np.stack([hi, lo], axis=1))

    def x_t(x_b):  # [S, D] -> [KA, 2, S] fp8
        xt = np.ascontiguousarray(x_b.T)
        if aug:
            pad = np.zeros((KA, S), np.float32)
            pad[:D] = xt
            pad[D] = 1.0
            xt = pad
        return split8(xt)

    def w_kq(W, b, half):  # -> [NHP, P, KTC*2*128] fp8, p-major
        ws = np.ascontiguousarray(W[:, half * OF:(half + 1) * OF]) * WS
        if aug:
            pad = np.zeros((KA, OF), np.float32)
            pad[:D] = ws
            pad[D] = b[half * OF:(half + 1) * OF] * WS
            ws = pad
        s8 = split8(ws)  # [KA, 2, OF]
        pm = s8.reshape(KTC, P, 2, NHP, P).transpose(3, 1, 0, 2, 4)
        return np.ascontiguousarray(pm.reshape(NHP, P, KTC * 2 * P))

    def w_v(W, b, half):  # -> [KA, 2, OF] fp8
        ws = np.ascontiguousarray(W[:, half * OF:(half + 1) * OF]) * WS
        if aug:
            pad = np.zeros((KA, OF), np.float32)
            pad[:D] = ws
            pad[D] = b[half * OF:(half + 1) * OF] * WS
            ws = pad
        return split8(ws)

    xts = {}
    in_maps = []
    for c in range(N_CORES):
        b, half = divmod(c, 2)
        if b not in xts:
            xts[b] = (x_t(q[b]), x_t(k[b]), x_t(v[b]))
        xq_c, xk_c, xv_c = xts[b]
        in_maps.append({
            "xq": xq_c,
            "xk": xk_c,
            "xv": xv_c,
            "wq": w_kq(Wq, bq, half),
            "wk": w_kq(Wk, bk, half),
            "wv": w_v(Wv, bv, half),
        })
    return in_maps


def kernel(q, k, v, Wq, bq, Wk, bk, Wv, bv):
    global last_results
    q = np.ascontiguousarray(np.asarray(q, np.float32))
    k = np.ascontiguousarray(np.asarray(k, np.float32))
    v = np.ascontiguousarray(np.asarray(v, np.float32))
    Wq = np.asarray(Wq, np.float32)
    Wk = np.asarray(Wk, np.float32)
    Wv = np.asarray(Wv, np.float32)
    bq = np.asarray(bq, np.float32)
    bk = np.asarray(bk, np.float32)
    bv = np.asarray(bv, np.float32)

    aug = any(np.any(b_) for b_ in (bq, bk, bv))
    KTC = (D // P) + (1 if aug else 0)

    if KTC not in _cache:
        _cache[KTC] = _build(KTC)
    nc = _cache[KTC]

    in_maps = _prep_core_inputs(q, k, v, Wq, bq, Wk, bk, Wv, bv, KTC)
    res = run_bass_kernel_spmd(nc, in_maps, core_ids=list(range(N_CORES)))
    last_results = res

    out = np.empty((B, S, D), np.float32)
    for c in range(N_CORES):
        b, half = divmod(c, 2)
        out[b, :, half * OF:(half + 1) * OF] = res.results[c]["o"]
    return out



# revision 71
# speedup vs baseline: 1.2029x; 1.0015x over previous
"""Trainium2 Bass kernel for nn_MultiHeadAttention_69106023793143.

Reference computation (B=4, S=2048, D=1024, H=16, HD=64):
    qh = split_heads(q @ Wq + bq); kh, vh likewise
    out = merge_heads(sigmoid((qh @ kh^T) / sqrt(HD)) @ vh)

Sharding (8 cores): core c handles batch b = c//2 and the half = c%2 slice of
the feature axis (512 features = 8 heads).  Projections are tensor-parallel on
the output dim of Wq/Wk/Wv; attention is head-parallel.

Device strategy per core (three-way balanced pipeline):
  - The elementwise nonlinearity over 8*2048*2048 scores is split across
    BOTH elementwise engines via the identity sigma(s) = (1+tanh(s/2))/2:
    out = (SUM tanh(s_k/2) v_k + SUM v_k) / 2.  ACT computes exact tanh
    (1 elem/cyc/partition @1.2GHz); the DVE computes a clamped odd-quintic
    approximation of tanh as ONE fused custom-DVE op (TANH_PC5_ANT,
    registered at import: p(u)=u*(C1+u^2*(C2+u^2)), u=clamp(C0*s,+-1),
    8 ALU stages, N(0,1)-weighted RMS 3.5e-3).  A greedy ledger (est/charge)
    assigns each score wave to whichever engine has less planned busy time,
    counting the PSUM->SBUF drains that only the DVE can do.  This turns the
    218us single-engine ACT floor into ~155us across two engines, leaving
    the PE (~178us busy) as the pacing engine.
  - The +SUM v_k term: ones-lhsT matmuls accumulate SUM v per head once into
    the briefly-idle proj bank (sv_sb, prescaled 0.5/WS); each round's o
    drain is AFFINE_THEN_ADD (one DVE op: o = av*(0.5/WS) + sv broadcast).
  - Projections run as split-fp8 DoubleRow matmuls: host ships x and W as
    (hi, lo) fp8e4 pairs and the product takes the three cross terms
    xh*Wh + xh*Wl + xl*Wh - ~bf16 accuracy at 0.75x the bf16 PE cost.
    W is host-scaled by WS=16 to keep fp8 W normal; scales fold into the
    tanh arg and the o drain.
  - Scores use fp8e4 DoubleRow matmuls at 0.5 cyc/row: kh stored as an
    (hi, lo) fp8 pair on the two DoubleRow K-blocks (k-side compensated),
    qh plain fp8 broadcast (stride-0).  Odd heads at tile_position (64,0).
    Each score matmul moves [HD,2,256]=512 elems = MAX_MOVING_FREE_DIM_SIZE
    (a single 512-col matmul moves 1024 and SILENTLY mis-executes on HW).
  - Rounds are ordered in head-pair blocks (2hp,0),(2hp+1,0),(2hp,1),
    (2hp+1,1) so early rounds need only qh chunks 0/1: the DMA stream is
    wk,xk0,wq,xq0,xq1,xk1,wv,xv0,xk2,xv1,xk3,xv2,xv3,xq2,xq3 - kh chunks
    arrive just-in-time for round 0's kt sweep and V-projections/AV start
    ~15us in, while xq c2/c3 (2MB) ship last.  K/Q projections for hp1-3
    are emitted just-in-time before their blocks as adjacent A/B half-pairs
    (pairs must NEVER interleave: they share the single proj PSUM bank).
  - AV in bf16, out[q,d]: lhsT = attn^T tile [128k,128q], rhs = v[128k,64].
    Each round accumulates 8 q-tiles interleaved in ONE PSUM av bank
    (start=True on first, stop=True on last).  AVs drain from a FIFO gated
    on their V-tile's emission, capped at 2/wave, with a 4-wave cool-down
    after each round boundary so the next round's AVs never WAR-block the
    in-order PE queue on the o drain.
  - PSUM: 3 x 2-bank score-wave tiles (2-job waves) + 1 proj bank + 1 av
    bank = 8.  SBUF: 28 attn wave buffers.
  - Warm-up matmuls ramp the PE p-state (0.65->2.4GHz) during the prefix.
  - Nonzero biases fold in via a host-side augmented ones-row (KTC=9).

End-to-end: max rel err ~1.39e-2 (budget 2e-2); TimelineSim 242555ns
(baseline 297223ns).
"""

import sys

if "/opt/trn_rl_repo" not in sys.path:
    sys.path.insert(0, "/opt/trn_rl_repo")

from collections import deque
from contextlib import ExitStack

import numpy as np

import concourse.tile as tile
from concourse import bacc, mybir
from concourse import dve_ops as _dve_ops
from concourse.bass_utils import run_bass_kernel_spmd
from concourse.dve_spec import C0, C1, C2, One, Spec, Src0, Zero, lower, maxx, minn, sq
from concourse.dve_uop import DveOpSpec

# ---- custom DVE op: clamped odd-quintic tanh approximation -----------------
# p(u) = u*(C1 + u^2*(C2 + u^2)), u = clamp(Src0*C0, -1, 1)  [8 ALU stages]
# Approximates tanh(k_fit * s / C0_rel ...): with C0 = K_FIT*raw_scale it
# computes tanh(s_true/2) to 3.5e-3 weighted RMS over s_true ~ N(0,1)
# (max err 0.034 at the |s|~4 clamp shoulder).  The quintic coefficient is
# slaved to 1 in u-units, which keeps the expression inside the DVE's
# 8-stage budget with only 3 scalar slots.
_TANH_NAME = "TANH_PC5_ANT"
K_FIT = 0.25283828
C1_FIT = 1.94641582
C2_FIT = -1.95047264


def _tanh_pc5_ref(in0, in1, s0, s1, imm2):
    u = np.clip(np.asarray(in0, np.float32) * s0, -1.0, 1.0)
    u2 = u * u
    return u * (s1 + u2 * (imm2 + u2))


def _register_tanh_op():
    for op in _dve_ops.OPS:
        if op.name == _TANH_NAME:
            return op
    t = Src0 * C0
    u = maxx(minn(t, One), Zero - One)
    u2 = sq(u)
    spec = Spec(body=u * (C1 + u2 * (C2 + u2)), reference=_tanh_pc5_ref)
    shas = {
        ver: DveOpSpec(name=_TANH_NAME, uops=lower(spec, ver=ver)).sha(ver)
        for ver in ("v3", "v4")
    }
    op = _dve_ops.DveOp(_TANH_NAME, spec, subdim=False, uops_sha=shas)
    _dve_ops.OPS.append(op)
    _dve_ops.CUSTOM_DVE_SPECS[op.name] = spec
    _dve_ops._SUB_OPCODE_FOR_NAME[op.name] = (
        _dve_ops._CUSTOM_DVE_ROW_BASE + len(_dve_ops.OPS) - 1
    )
    return op


TANH_OP = _register_tanh_op()

B, S, D, H = 4, 2048, 1024, 16
HD = D // H  # 64
OF = D // 2  # 512 features (8 heads) per core
N_CORES = 8
P = 128
NH = 8          # heads per core
NHP = 4         # head pairs per core
QC = 4          # q-chunks of 512
NQCP = 2        # q-chunk pairs
NKT = 16        # k token tiles of 128
TOKC = 4        # x token chunks of 512
RJOBS = 2 * NKT  # jobs per round (2 q-chunks x 16 kt)
ABUFS = 28      # attn (a_t) wave buffers
WS = 16.0       # host-side W scale (keeps fp8 W out of subnormals)

F32 = mybir.dt.float32
BF16 = mybir.dt.bfloat16
FP8 = mybir.dt.float8e4

# the three split-fp8 cross terms (w level, x level)
TERMS = ((0, 0), (0, 1), (1, 0))

_cache: dict = {}
last_results = None


def _build(KTC: int):
    """KTC = contraction k-tiles for the projections (8, or 9 when biases are
    folded in via an augmented ones-row)."""
    nc = bacc.Bacc("TRN2", target_bir_lowering=False, debug=False,
                   num_devices=N_CORES, name="mha_sig4")
    KA = KTC * P
    NDR = KTC // 2   # DoubleRow kt-pairs per term
    AUGK = KTC % 2   # leftover kt (the ones-row) as plain fp8 matmul
    WFREE = KTC * 2 * P  # per-partition elements of one head-pair W slice

    xq = nc.dram_tensor("xq", [KA, 2, S], FP8, kind="ExternalInput")
    xk = nc.dram_tensor("xk", [KA, 2, S], FP8, kind="ExternalInput")
    xv = nc.dram_tensor("xv", [KA, 2, S], FP8, kind="ExternalInput")
    # wq/wk p-major: [head-pair, partition, kt*level*128]
    wq = nc.dram_tensor("wq", [NHP, P, WFREE], FP8, kind="ExternalInput")
    wk = nc.dram_tensor("wk", [NHP, P, WFREE], FP8, kind="ExternalInput")
    wv = nc.dram_tensor("wv", [KA, 2, OF], FP8, kind="ExternalInput")
    o = nc.dram_tensor("o", [S, OF], F32, kind="ExternalOutput")

    xq_r = xq.rearrange("(kt p) l t -> p kt l t", p=P)
    xk_r = xk.rearrange("(kt p) l t -> p kt l t", p=P)
    xv_r = xv.rearrange("(kt p) l t -> p kt l t", p=P)
    wv_r = wv.rearrange("(kt p) l n -> p kt l n", p=P)

    abufs = ABUFS if KTC == 8 else 10

    with tile.TileContext(nc) as tc:
        with ExitStack() as ctx:
            persist = ctx.enter_context(tc.tile_pool(name="persist", bufs=1))
            xvpool = ctx.enter_context(tc.tile_pool(name="xvpool", bufs=2))
            apool = ctx.enter_context(tc.tile_pool(name="apool", bufs=abufs))
            opool = ctx.enter_context(tc.tile_pool(name="opool", bufs=2))
            ps_pool = ctx.enter_context(
                tc.tile_pool(name="ps_pool", bufs=2, space="PSUM"))

            wk_sb = persist.tile([P, NHP, KTC, 2, P], FP8)
            wq_sb = persist.tile([P, NHP, KTC, 2, P], FP8)
            wv_sb = persist.tile([P, KTC, 2, OF], FP8)
            xk_sb = persist.tile([P, KTC, 2, S], FP8)
            xq_sb = persist.tile([P, KTC, 2, S], FP8)
            # kh as (hi, lo) fp8 pair, head pairs stacked on partitions;
            # qh plain fp8; v bf16 [tok, of]
            kh = persist.tile([P, NHP, 2, S], FP8)
            qh = persist.tile([P, NHP, S], FP8)
            v_sb = persist.tile([P, NKT, OF], BF16)
            # (Σ_k v)·WS·(0.5/WS) per head, broadcast-added at the o drain
            sv_sb = persist.tile([P, NH, HD], BF16)
            ones_sb = persist.tile([P, P], BF16)
            nc.vector.memset(ones_sb[:], 1.0)

            # ONE PSUM bank for all projections: two [P, 256] slots in a
            # persistent tile, manually rotated. Region-based dep tracking
            # gives WAR/WAW per slot; each half-group's start=True re-marks
            # the whole bank but PSUM reads return raw data for re-marked
            # bytes (hardware-verified), and no other slot is ever
            # mid-accumulation when a start executes (serial emission).
            proj_ps = ps_pool.tile([P, 2, 256], F32, tag="proj", bufs=1)

            class _ProjView:
                """Adapter exposing the proj bank as tile[:, 0, cols]:
                cols 0:256 -> slot 0, 256:512 -> slot 1, full -> wide."""

                def __getitem__(self, idx):
                    c = idx[2]
                    if c == slice(None):
                        return proj_ps[:].rearrange("p s n -> p (s n)")
                    return proj_ps[:, 0 if c.start == 0 else 1, :]

            def proj_tile():
                return _ProjView()

            # ---------- engine-balance ledger ----------
            # planned busy ns for ACT / DVE; drains charge DVE (or ACT) at
            # emission so the per-wave greedy pick stays globally balanced
            est = {"ACT": 0.0, "DVE": 0.0}

            def charge(eng, n_elems, ov=None):
                est[eng] += n_elems * 0.833 + 171 if eng == "ACT" \
                    else n_elems * 1.042 + 61

            def bal_copy(out, in_, n, scale=None):
                """PSUM->SBUF copy (optionally scaled) on whichever of
                ACT/DVE the ledger says is less loaded."""
                if est["ACT"] + n * 0.833 + 171 <= est["DVE"] + n * 1.042 + 61:
                    charge("ACT", n)
                    nc.scalar.activation(
                        out=out, in_=in_,
                        func=mybir.ActivationFunctionType.Copy,
                        scale=1.0 if scale is None else scale)
                else:
                    charge("DVE", n)
                    if scale is None:
                        nc.vector.tensor_copy(out=out, in_=in_)
                    else:
                        nc.vector.tensor_scalar_mul(out, in_, scale)

            # ---------- producer closures ----------
            def dma_w_hp(w_sb, w_dram, hp):
                def run():
                    nc.sync.dma_start(
                        w_sb[:, hp].rearrange("p kt l n -> p (kt l n)"),
                        w_dram[hp])
                return run

            def dma_wv():
                def run():
                    nc.sync.dma_start(wv_sb[:], wv_r)
                return run

            def dma_x(x_sb, x_r, lo, hi):
                def run():
                    for lv in range(2):
                        nc.sync.dma_start(
                            x_sb[:, :, lv, lo:hi],
                            x_r[:, :, lv, lo:hi])
                return run

            xv_tiles = {}

            def dma_xv(c, lv):
                def run():
                    if lv == 0:
                        xv_tiles[c] = xvpool.tile([P, KTC, 2, 512], FP8,
                                                  tag="xvchunk",
                                                  name=f"xv_{c}")
                    nc.sync.dma_start(
                        xv_tiles[c][:, :, lv, :],
                        xv_r[:, :, lv, c * 512:(c + 1) * 512])
                return run

            # warm-up: the PE runs at 0.65/1.2GHz until ~3us of continuous
            # execution; burn dummy matmuls during the prefix DMAs so the
            # first projections run at full clock
            wu_sb = persist.tile([HD, 2, 640], FP8)

            def warmup(n_mm):
                def run():
                    if n_mm < 0:
                        nc.vector.memset(wu_sb[:], 0)
                        return
                    st = ps_pool.tile([P, 2, 512], F32, tag="st", bufs=3,
                                      name=f"wu_{n_mm}")
                    for m in range(n_mm):
                        nc.tensor.matmul(
                            st[:, m % 2, :],
                            lhsT=wu_sb[:, :, 0:P],
                            rhs=wu_sb[:, :, P:P + 512],
                            start=True, stop=True,
                            perf_mode=mybir.MatmulPerfMode.DoubleRow,
                            skip_group_check=True,
                        )
                return run

            # split-emission prefix projections: the hi terms only need the
            # lv0 (hi) half of the x chunk, so they start ~1.5us earlier
            _prefix_ps = {}

            def _kq_half(x_sb, w_sb, hp, tsl, ps, terms, first):
                n = 0
                for lw, lx in terms:
                    for t in range(NDR):
                        nc.tensor.matmul(
                            ps,
                            lhsT=w_sb[:, hp, 2 * t:2 * t + 2, lw, :],
                            rhs=x_sb[:, 2 * t:2 * t + 2, lx, tsl],
                            start=(first and n == 0), stop=False,
                            perf_mode=mybir.MatmulPerfMode.DoubleRow,
                            skip_group_check=True,
                        )
                        n += 1

            def _kq_drain(dst, hp, tsl, ps, split_lo):
                charge("DVE", 512 if split_lo else 256)
                if split_lo:
                    charge("DVE", 256)
                    nc.vector.tensor_copy(out=dst[:, hp, 0, tsl], in_=ps)
                    nc.vector.tensor_sub(dst[:, hp, 1, tsl], ps,
                                         dst[:, hp, 0, tsl])
                else:
                    nc.vector.tensor_copy(out=dst[:, hp, tsl], in_=ps)

            def proj_kq_hi(x_sb, w_sb, hp, c, key):
                def run():
                    tl = proj_tile()
                    _prefix_ps[key] = tl
                    tsl = slice(c * 512, c * 512 + 256)
                    _kq_half(x_sb, w_sb, hp, tsl, tl[:, 0, 0:256],
                             ((0, 0), (1, 0)), True)
                return run

            def proj_kq_lo(x_sb, w_sb, hp, c, dst, split_lo, key):
                def run():
                    tl = _prefix_ps.pop(key)
                    tsl = slice(c * 512, c * 512 + 256)
                    _kq_half(x_sb, w_sb, hp, tsl, tl[:, 0, 0:256],
                             ((0, 1),), False)
                    _kq_drain(dst, hp, tsl, tl[:, 0, 0:256], split_lo)
                    tsl2 = slice(c * 512 + 256, (c + 1) * 512)
                    _kq_half(x_sb, w_sb, hp, tsl2, tl[:, 0, 256:512],
                             ((0, 0), (1, 0), (0, 1)), True)
                    _kq_drain(dst, hp, tsl2, tl[:, 0, 256:512], split_lo)
                return run

            def dma_x_lv(x_sb, x_r, lo, hi, lv):
                def run():
                    nc.sync.dma_start(
                        x_sb[:, :, lv, lo:hi],
                        x_r[:, :, lv, lo:hi])
                return run

            _kq_state = {}

            def _kq_matmuls(x_sb, w_sb, hp, c, tl, half):
                ps = tl[:, 0, half * 256:(half + 1) * 256]
                tsl = slice(c * 512 + half * 256,
                            c * 512 + (half + 1) * 256)
                n = 0
                for lw, lx in TERMS:
                    for t in range(NDR):
                        nc.tensor.matmul(
                            ps,
                            lhsT=w_sb[:, hp, 2 * t:2 * t + 2, lw, :],
                            rhs=x_sb[:, 2 * t:2 * t + 2, lx, tsl],
                            start=(n == 0), stop=False,
                            perf_mode=mybir.MatmulPerfMode.DoubleRow,
                            skip_group_check=True,
                        )
                        n += 1
                    if AUGK:
                        nc.tensor.matmul(
                            ps,
                            lhsT=w_sb[:, hp, KTC - 1, lw, :],
                            rhs=x_sb[:, KTC - 1, lx, tsl],
                            start=(n == 0), stop=False,
                            skip_group_check=True,
                        )
                        n += 1

            def proj_kq_a(x_sb, w_sb, hp, c, key):
                """first 256-half of a K/Q chunk projection -- emitted two
                jobs before the second half so score fills slip between"""
                def run():
                    tl = proj_tile()
                    _kq_state[key] = tl
                    _kq_matmuls(x_sb, w_sb, hp, c, tl, 0)
                return run

            def proj_kq_b(x_sb, w_sb, hp, c, dst, split_lo, key):
                def run():
                    tl = _kq_state.pop(key)
                    _kq_matmuls(x_sb, w_sb, hp, c, tl, 1)
                    wide = tl[:, 0, :]
                    _kq_finish(wide, dst, hp, c, split_lo)
                return run

            def _kq_finish(wide, dst, hp, c, split_lo):
                if True:
                    sl = slice(c * 512, (c + 1) * 512)
                    if split_lo:
                        charge("DVE", 512)
                        charge("DVE", 512)
                        nc.vector.tensor_copy(out=dst[:, hp, 0, sl], in_=wide)
                        nc.vector.tensor_sub(dst[:, hp, 1, sl], wide,
                                             dst[:, hp, 0, sl])
                    else:
                        charge("DVE", 512)
                        nc.vector.tensor_copy(out=dst[:, hp, sl], in_=wide)

            def proj_kq(x_sb, w_sb, hp, c, dst, split_lo):
                def run():
                    tl = proj_tile()
                    _kq_matmuls(x_sb, w_sb, hp, c, tl, 0)
                    _kq_matmuls(x_sb, w_sb, hp, c, tl, 1)
                    _kq_finish(tl[:, 0, :], dst, hp, c, split_lo)
                return run

            _v_ps = {}

            def proj_v(t, part):
                """v_sb[:, t, :] = x-token-tile t @ Wv ([tok, of]).
                part 'hi' takes the two x-hi terms, 'lo' the x-lo term +
                the PSUM drain (finer interleaving in the in-order PE queue,
                and 'hi' only needs the lv0 half of the xv chunk)."""
                def run():
                    xt = xv_tiles[t // 4]
                    tsl = slice((t % 4) * P, (t % 4 + 1) * P)

                    def v_half(ps, osl, terms, first):
                        n = 0
                        for lw, lx in terms:
                            for u in range(NDR):
                                nc.tensor.matmul(
                                    ps,
                                    lhsT=xt[:, 2 * u:2 * u + 2, lx, tsl],
                                    rhs=wv_sb[:, 2 * u:2 * u + 2, lw, osl],
                                    start=(first and n == 0), stop=False,
                                    perf_mode=mybir.MatmulPerfMode.DoubleRow,
                                    skip_group_check=True,
                                )
                                n += 1
                            if AUGK:
                                nc.tensor.matmul(
                                    ps,
                                    lhsT=xt[:, KTC - 1, lx, tsl],
                                    rhs=wv_sb[:, KTC - 1, lw, osl],
                                    start=(first and n == 0), stop=False,
                                    skip_group_check=True,
                                )
                                n += 1

                    if part == "hi":
                        tl = proj_tile()
                        _v_ps[t] = tl
                        v_half(tl[:, 0, 0:256], slice(0, 256),
                               ((0, 0), (1, 0)), True)
                    else:
                        tl = _v_ps.pop(t)
                        v_half(tl[:, 0, 0:256], slice(0, 256),
                               ((0, 1),), False)
                        v_half(tl[:, 0, 256:512], slice(256, 512),
                               ((0, 0), (1, 0), (0, 1)), True)
                        charge("DVE", 512)
                        nc.vector.tensor_copy(
                            out=v_sb[:, t, :], in_=tl[:, 0, :])
                return run

            # ---------- static schedule ----------
            # rounds in head-pair blocks: (2hp,0),(2hp+1,0),(2hp,1),(2hp+1,1)
            # so the early rounds need only qh chunks 0/1 -- xq c2/c3 (2MB)
            # can ship after the whole xk/xv stream.  Round index
            # r(h,qc) = 4*(h//2) + 2*(qc//2) + h%2; job = r*32 + kt*2 + i.
            # Round 0 staggers its second q-chunk by 2 k-tiles so the first
            # waves only need qh[qc0] (whose projection finishes first).
            rounds = [(2 * hp + (s % 2), s // 2)
                      for hp in range(NHP) for s in range(4)]
            jobs = []
            for kt in range(NKT + 2):
                if kt < NKT:
                    jobs.append((0, 0, kt))
                if kt >= 2:
                    jobs.append((0, 1, kt - 2))
            for h, qcp in rounds[1:]:
                jobs += [(h, 2 * qcp + i, kt)
                         for kt in range(NKT) for i in range(2)]
            waves = [jobs[i0:i0 + 2] for i0 in range(0, len(jobs), 2)]

            producers = []  # (due_job, closure, vtile_or_None)
            # prefix + all input DMAs in deadline order (the DMA engines are
            # effectively serial; emission order = transfer order)
            if AUGK == 0:
                producers += [
                    (-99.9, warmup(-1), None),
                    (-99.8, warmup(10), None),
                    (-99.0, dma_w_hp(wk_sb, wk, 0), None),
                    (-98.9, dma_x_lv(xk_sb, xk_r, 0, 512, 0), None),
                    (-98.8, dma_w_hp(wq_sb, wq, 0), None),
                    (-98.7, proj_kq_hi(xk_sb, wk_sb, 0, 0, "k00"), None),
                    (-98.6, dma_x_lv(xk_sb, xk_r, 0, 512, 1), None),
                    (-98.5, proj_kq_lo(xk_sb, wk_sb, 0, 0, kh, True, "k00"),
                     None),
                    (-98.4, dma_x_lv(xq_sb, xq_r, 0, 512, 0), None),
                    (-98.3, warmup(4), None),
                    (-98.2, proj_kq_hi(xq_sb, wq_sb, 0, 0, "q00"), None),
                    (-98.1, dma_x_lv(xq_sb, xq_r, 0, 512, 1), None),
                    (-98.0, proj_kq_lo(xq_sb, wq_sb, 0, 0, qh, False, "q00"),
                     None),
                    (-97.9, dma_x_lv(xq_sb, xq_r, 512, 1024, 0), None),
                    (-97.8, proj_kq_hi(xq_sb, wq_sb, 0, 1, "q01"), None),
                    (-97.7, dma_x_lv(xq_sb, xq_r, 512, 1024, 1), None),
                    (-97.6, proj_kq_lo(xq_sb, wq_sb, 0, 1, qh, False, "q01"),
                     None),
                ]
            else:
                producers += [
                    (-99.0, dma_w_hp(wk_sb, wk, 0), None),
                    (-98.8, dma_x(xk_sb, xk_r, 0, 512), None),
                    (-98.6, dma_w_hp(wq_sb, wq, 0), None),
                    (-98.4, dma_x(xq_sb, xq_r, 0, 512), None),
                    (-98.2, proj_kq(xk_sb, wk_sb, 0, 0, kh, True), None),
                    (-98.0, proj_kq(xq_sb, wq_sb, 0, 0, qh, False), None),
                    (-97.8, dma_x(xq_sb, xq_r, 512, 1024), None),
                    (-97.6, proj_kq(xq_sb, wq_sb, 0, 1, qh, False), None),
                ]
            # Σv per head: ones-lhsT matmuls into the (idle until hp1) proj
            # bank once all v tiles are in SBUF, then one pre-scaled drain.
            _sv = {}

            def sv_accum():
                def run():
                    _sv["tl"] = proj_tile()
                    wide = _sv["tl"][:, 0, :]
                    n = 0
                    for h in range(NH):
                        for kt in range(NKT):
                            nc.tensor.matmul(
                                wide[:, h * HD:(h + 1) * HD],
                                lhsT=ones_sb[:],
                                rhs=v_sb[:, kt, h * HD:(h + 1) * HD],
                                start=(n == 0), stop=(n == NH * NKT - 1),
                                skip_group_check=True,
                            )
                            n += 1
                return run

            def sv_drain():
                def run():
                    charge("DVE", 512)
                    nc.vector.tensor_scalar_mul(
                        sv_sb[:],
                        _sv.pop("tl")[:, 0, :].rearrange(
                            "p (h2 d) -> p h2 d", d=HD),
                        0.5 / WS)
                    _sv["done"] = True
                return run

            # input stream in DMA-arrival order (the DMA engines are
            # effectively serial; emission order = transfer order); PE
            # producers positioned at their consumer's job index so the
            # in-order PE queue never inverts the arrival order.
            if AUGK == 0:
                producers += [
                    # xk c1 -> kh c1 (kt 4-7, needed from job ~8)
                    (-89.8, dma_x_lv(xk_sb, xk_r, 512, 1024, 0), None),
                    (-89.7, proj_kq_hi(xk_sb, wk_sb, 0, 1, "k01"), None),
                    (-89.6, dma_x_lv(xk_sb, xk_r, 512, 1024, 1), None),
                    (-89.5, proj_kq_lo(xk_sb, wk_sb, 0, 1, kh, True, "k01"),
                     None),
                ]
            else:
                producers.append((-89.8, dma_x(xk_sb, xk_r, 512, 1024), None))
                producers.append((2, proj_kq(xk_sb, wk_sb, 0, 1, kh, True),
                                  None))
            # V stream: wv + xv c0 right behind xk c1
            producers.append((-89.4, dma_wv(), None))
            producers.append((-89.3, dma_xv(0, 0), None))
            producers.append((-89.2, dma_xv(0, 1), None))
            for t in range(4):
                producers.append((12 + 0.1 * t, proj_v(t, "hi"), None))
                producers.append((12.05 + 0.1 * t, proj_v(t, "lo"), t))
            # xk c2 -> kh c2 (kt 8-11, needed from job ~16)
            if AUGK == 0:
                producers += [
                    (13.0, dma_x_lv(xk_sb, xk_r, 1024, 1536, 0), None),
                    (13.1, proj_kq_hi(xk_sb, wk_sb, 0, 2, "k02"), None),
                    (13.2, dma_x_lv(xk_sb, xk_r, 1024, 1536, 1), None),
                    (13.3, proj_kq_lo(xk_sb, wk_sb, 0, 2, kh, True, "k02"),
                     None),
                ]
            else:
                producers.append((13.0, dma_x(xk_sb, xk_r, 1024, 1536), None))
                producers.append((13.3, proj_kq(xk_sb, wk_sb, 0, 2, kh, True),
                                  None))
            producers.append((13.4, dma_xv(1, 0), None))
            producers.append((13.5, dma_xv(1, 1), None))
            for t in range(4, 8):
                producers.append((16 + 0.1 * t, proj_v(t, "hi"), None))
                producers.append((16.05 + 0.1 * t, proj_v(t, "lo"), t))
            # xk c3 -> kh c3 (kt 12-15, needed from job ~24)
            if AUGK == 0:
                producers += [
                    (18.0, dma_x_lv(xk_sb, xk_r, 1536, 2048, 0), None),
                    (18.05, proj_kq_hi(xk_sb, wk_sb, 0, 3, "k03"), None),
                    (18.1, dma_x_lv(xk_sb, xk_r, 1536, 2048, 1), None),
                    (18.15, proj_kq_lo(xk_sb, wk_sb, 0, 3, kh, True, "k03"),
                     None),
                ]
            else:
                producers.append((18.0, dma_x(xk_sb, xk_r, 1536, 2048),
                                  None))
                producers.append((18.1,
                                  proj_kq(xk_sb, wk_sb, 0, 3, kh, True),
                                  None))
            producers.append((18.2, dma_xv(2, 0), None))
            producers.append((18.3, dma_xv(2, 1), None))
            for t in range(8, 12):
                producers.append((22 + 0.1 * t, proj_v(t, "hi"), None))
                producers.append((22.05 + 0.1 * t, proj_v(t, "lo"), t))
            producers.append((24.0, dma_xv(3, 0), None))
            producers.append((24.1, dma_xv(3, 1), None))
            for t in range(12, NKT):
                producers.append((26 + 0.1 * t, proj_v(t, "hi"), None))
                producers.append((26.05 + 0.1 * t, proj_v(t, "lo"), t))
            producers.append((28.0, sv_accum(), None))
            producers.append((28.1, sv_drain(), None))
            # xq c2/c3 ship last; qh c2/3 for hp0 first used at job 64
            producers.append((32.0, dma_x(xq_sb, xq_r, 1024, 1536), None))
            producers.append((32.5, dma_x(xq_sb, xq_r, 1536, 2048), None))
            producers.append((46, proj_kq_a(xq_sb, wq_sb, 0, 2, "q02"),
                              None))
            producers.append((48, proj_kq_b(xq_sb, wq_sb, 0, 2, qh, False,
                                            "q02"), None))
            producers.append((50, proj_kq_a(xq_sb, wq_sb, 0, 3, "q03"),
                              None))
            producers.append((52, proj_kq_b(xq_sb, wq_sb, 0, 3, qh, False,
                                            "q03"), None))
            # hp1-3: W DMAs + K/Q projections just-in-time for their blocks
            for hp in range(1, NHP):
                base = 128 * hp
                producers.append((base - 40, dma_w_hp(wk_sb, wk, hp), None))
                producers.append((base - 38, dma_w_hp(wq_sb, wq, hp), None))
                # A/B pairs of one projection share the single proj
                # PSUM bank -- their deadlines must never interleave with
                # another projection's pair (fractional offsets keep each
                # pair adjacent in the producer stream)
                for c in range(TOKC):
                    producers.append((base + 8 * c - 16.0,
                                      proj_kq_a(xk_sb, wk_sb, hp, c,
                                                f"k{hp}{c}"), None))
                    producers.append((base + 8 * c - 15.9,
                                      proj_kq_b(xk_sb, wk_sb, hp, c, kh, True,
                                                f"k{hp}{c}"), None))
                producers.append((base - 13.8,
                                  proj_kq_a(xq_sb, wq_sb, hp, 0, f"q{hp}0"),
                                  None))
                producers.append((base - 13.7,
                                  proj_kq_b(xq_sb, wq_sb, hp, 0, qh, False,
                                            f"q{hp}0"), None))
                producers.append((base - 11.8,
                                  proj_kq_a(xq_sb, wq_sb, hp, 1, f"q{hp}1"),
                                  None))
                producers.append((base - 11.7,
                                  proj_kq_b(xq_sb, wq_sb, hp, 1, qh, False,
                                            f"q{hp}1"), None))
                producers.append((base + 48.0,
                                  proj_kq_a(xq_sb, wq_sb, hp, 2, f"q{hp}2"),
                                  None))
                producers.append((base + 48.1,
                                  proj_kq_b(xq_sb, wq_sb, hp, 2, qh, False,
                                            f"q{hp}2"), None))
                producers.append((base + 52.0,
                                  proj_kq_a(xq_sb, wq_sb, hp, 3, f"q{hp}3"),
                                  None))
                producers.append((base + 52.1,
                                  proj_kq_b(xq_sb, wq_sb, hp, 3, qh, False,
                                            f"q{hp}3"), None))
            producers.sort(key=lambda e: e[0])
            producers = deque(producers)
            v_emit_wave = {}

            # AV bookkeeping
            av_fifo = deque()  # (job_idx, h, qc, kt, a_t, j_in_wave, wave)
            av_state = {"tile": None, "round": -1, "cool": -1}

            def finalize_round(r):
                av = av_state["tile"]
                hp, s = divmod(r, 4)
                h, qcp = 2 * hp + (s % 2), s // 2
                o_sb = opool.tile([P, 2, QC, HD], F32, tag="o_sb",
                                  name=f"osb_{r}")
                # o = (Σ attn'·v + Σv)·(1/(2·WS)) with attn' = tanh(s/2):
                # the sv term supplies the +Σv, pre-scaled by 0.5/WS
                charge("DVE", 512)
                nc.vector.affine_then_add(
                    out=o_sb[:].rearrange("p i qt d -> p (i qt) d"),
                    in0=av[:].rearrange("p (g d) -> p g d", d=HD),
                    in1=sv_sb[:, h].unsqueeze(1).broadcast_to([P, 2 * QC, HD]),
                    scale=0.5 / WS,
                    bias=0.0,
                )
                for i in range(2):
                    qc = 2 * qcp + i
                    dst = o[qc * 512:(qc + 1) * 512,
                            h * HD:(h + 1) * HD].rearrange(
                                "(qt p) d -> p qt d", p=P)
                    nc.sync.dma_start(dst, o_sb[:, i])
                av_state["tile"] = None

            def drain_avs(cur_wave, final=False):
                budget = 2  # cap per-wave AV emission so a backlog burst
                # never parks in front of the score stream in the in-order
                # PE queue
                continue_outer = False
                while av_fifo and not continue_outer:
                    job, h, qc, kt, a_t, j, w = av_fifo[0]
                    if not final:
                        if budget <= 0:
                            break
                        if w >= cur_wave:
                            break
                        vw = v_emit_wave.get(kt)
                        if vw is None or vw >= cur_wave:
                            break
                        budget -= 1
                    r = job // RJOBS
                    if r != av_state["round"]:
                        if not final and not _sv.get("done"):
                            break
                        if av_state["tile"] is not None:
                            finalize_round(av_state["round"])
                            # cool-down: keep the next round's AVs out of the
                            # in-order PE queue until the o-drain has had two
                            # waves to clear the av bank (they would WAR-block
                            # every score fill emitted behind them)
                            av_state["cool"] = cur_wave + 4
                        if not final and cur_wave < av_state["cool"]:
                            continue_outer = True
                            break
                        av_state["tile"] = ps_pool.tile(
                            [P, 512], F32, tag="av", bufs=1, name=f"av_{r}")
                        av_state["round"] = r
                    av_fifo.popleft()
                    av = av_state["tile"]
                    i = qc % 2
                    first = (kt == 0 and i == 0)
                    last = (kt == NKT - 1 and i == 1)
                    for qt in range(4):
                        nc.tensor.matmul(
                            av[:, (i * 4 + qt) * HD:(i * 4 + qt + 1) * HD],
                            lhsT=a_t[:, j, qt * P:(qt + 1) * P],
                            rhs=v_sb[:, kt, h * HD:(h + 1) * HD],
                            start=(first and qt == 0),
                            stop=(last and qt == 3),
                            skip_group_check=True,
                        )

            # ---------- main wave loop ----------
            def drain_producers(w, job_base):
                while producers and producers[0][0] <= job_base + 2:
                    due, closure, vtile = producers.popleft()
                    closure()
                    if vtile is not None:
                        v_emit_wave[vtile] = w

            job_base = 0
            for w, wave in enumerate(waves):
                drain_producers(w, job_base)
                g = len(wave)
                st = ps_pool.tile([P, 2, 512], F32, tag="st", bufs=3,
                                  name=f"st_{w}")
                for j, (h, qc, kt) in enumerate(wave):
                    hp, pb = h // 2, (h % 2) * HD
                    lhsT = kh[pb:pb + HD, hp, :, kt * P:(kt + 1) * P]
                    # two 256-col halves: the moving AP is [HD, 2, 256] = 512
                    # elements, the PE's MAX_MOVING_FREE_DIM_SIZE
                    for half in range(2):
                        rhs = qh[pb:pb + HD, hp,
                                 qc * 512 + half * 256:
                                 qc * 512 + (half + 1) * 256]
                        rhs = rhs.unsqueeze(1).broadcast_to([HD, 2, 256])
                        nc.tensor.matmul(
                            st[:, j, half * 256:(half + 1) * 256],
                            lhsT=lhsT,
                            rhs=rhs,
                            start=True,
                            stop=True,
                            perf_mode=mybir.MatmulPerfMode.DoubleRow,
                            tile_position=(pb, 0),
                            skip_group_check=True,
                        )
                a_t = apool.tile([P, 2, 512], BF16, tag="a_t", name=f"a_{w}")
                n_el = g * 512
                if est["DVE"] + n_el * 1.042 + 61 - 80 < est["ACT"] + n_el * 0.833 + 171:
                    # DVE share: clamped odd-quintic ~= tanh(s_true/2)
                    charge("DVE", n_el)
                    nc.vector._custom_dve(
                        TANH_OP,
                        out=a_t[:, :g, :],
                        in0=st[:, :g, :],
                        s0=K_FIT * 0.125 / (WS * WS),
                        s1=C1_FIT,
                        imm2=C2_FIT,
                    )
                else:
                    charge("ACT", n_el)
                    nc.scalar.activation(
                        out=a_t[:, :g, :],
                        in_=st[:, :g, :],
                        func=mybir.ActivationFunctionType.Tanh,
                        scale=0.0625 / (WS * WS),
                    )
                for j, (h, qc, kt) in enumerate(wave):
                    r = 4 * (h // 2) + 2 * (qc // 2) + (h % 2)
                    av_fifo.append((r * RJOBS + kt * 2 + (qc % 2),
                                    h, qc, kt, a_t, j, w))
                drain_avs(w)
                job_base += g
            import os
            if os.environ.get("KDBG"):
                print("EST at end:", est)
            while producers:
                producers.popleft()[1]()
            drain_avs(0, final=True)
            finalize_round(av_state["round"])

    nc.compile()
    return nc


def _prep_core_inputs(q, k, v, Wq, bq, Wk, bk, Wv, bv, KTC):
    """Host-side shard + transpose + split-fp8 packing. in_maps for 8 cores."""
    import ml_dtypes
    E4 = ml_dtypes.float8_e4m3
    KA = KTC * P
    aug = KA > D

    def split8(a):
        """[R, C] fp32 -> [R, 2, C] fp8 (hi, lo)."""
        hi = a.astype(E4)
        lo = (a - hi.astype(np.float32)).astype(E4)
        return np.ascontiguousarray(np.stack([hi, lo], axis=1))

    def x_t(x_b):  # [S, D] -> [KA, 2, S] fp8
        xt = np.ascontiguousarray(x_b.T)
        if aug:
            pad = np.zeros((KA, S), np.float32)
            pad[:D] = xt
            pad[D] = 1.0
            xt = pad
        return split8(xt)

    def w_kq(W, b, half):  # -> [NHP, P, KTC*2*128] fp8, p-major
        ws = np.ascontiguousarray(W[:, half * OF:(half + 1) * OF]) * WS
        if aug:
            pad = np.zeros((KA, OF), np.float32)
            pad[:D] = ws
            pad[D] = b[half * OF:(half + 1) * OF] * WS
            ws = pad
        s8 = split8(ws)  # [KA, 2, OF]
        pm = s8.reshape(KTC, P, 2, NHP, P).transpose(3, 1, 0, 2, 4)
        return np.ascontiguousarray(pm.reshape(NHP, P, KTC * 2 * P))

    def w_v(W, b, half):  # -> [KA, 2, OF] fp8
        ws = np.ascontiguousarray(W[:, half * OF:(half + 1) * OF]) * WS
        if aug:
            pad = np.zeros((KA, OF), np.float32)
            pad[:D] = ws
            pad[D] = b[half * OF:(half + 1) * OF] * WS
            ws = pad
        return split8(ws)

    xts = {}
    in_maps = []
    for c in range(N_CORES):
        b, half = divmod(c, 2)
        if b not in xts:
            xts[b] = (x_t(q[b]), x_t(k[b]), x_t(v[b]))
        xq_c, xk_c, xv_c = xts[b]
        in_maps.append({
            "xq": xq_c,
            "xk": xk_c,
            "xv": xv_c,
            "wq": w_kq(Wq, bq, half),
            "wk": w_kq(Wk, bk, half),
            "wv": w_v(Wv, bv, half),
        })
    return in_maps


def kernel(q, k, v, Wq, bq, Wk, bk, Wv, bv):
    global last_results
    q = np.ascontiguousarray(np.asarray(q, np.float32))
    k = np.ascontiguousarray(np.asarray(k, np.float32))
    v = np.ascontiguousarray(np.asarray(v, np.float32))
    Wq = np.asarray(Wq, np.float32)
    Wk = np.asarray(Wk, np.float32)
    Wv = np.asarray(Wv, np.float32)
    bq = np.asarray(bq, np.float32)
    bk = np.asarray(bk, np.float32)
    bv = np.asarray(bv, np.float32)

    aug = any(np.any(b_) for b_ in (bq, bk, bv))
    KTC = (D // P) + (1 if aug else 0)

    if KTC not in _cache:
        _cache[KTC] = _build(KTC)
    nc = _cache[KTC]

    in_maps = _prep_core_inputs(q, k, v, Wq, bq, Wk, bk, Wv, bv, KTC)
    res = run_bass_kernel_spmd(nc, in_maps, core_ids=list(range(N_CORES)))
    last_results = res

    out = np.empty((B, S, D), np.float32)
    for c in range(N_CORES):
        b, half = divmod(c, 2)
        out[b, :, half * OF:(half + 1) * OF] = res.results[c]["o"]
    return out

